# revision 8
# baseline (speedup 1.0000x reference)
"""Trainium2 Bass kernel for nn_RadialModel (forward NUFFT, radial MRI).

Per-core (1 frame, all 8 coils):
  1. coil multiply (DVE, bf16 out)       cimage = (xr+ixi)*(cr+ici)
  2. DFT via PE bf16 matmuls (two stages): G[v,u] = A @ (M^T @ A^T) with
     apodization + fftshift phases folded into the constant A matrices
  3. store grid to a DRAM table (bf16), coil-interleaved cells
     [p=v_pad(517), q=u_pad(517), cri(16)] with 2/3-cell wraparound halo
  4. Kaiser-Bessel 6x6 interpolation: indirect-DMA gathers (one index per
     partition per call is a HW limit of the SWDGE indirect path; each
     index fetches a contiguous 6-cell x 16-cri 192B run, 768 calls).
  5. weighted reduce on DVE, sqrt(w) scale + store

Sharding: one frame (nt) per NeuronCore, 8 cores. Host does only
shard/reshape/unshuffle; all math on device.
"""
import math
import numpy as np

import concourse.bass as bass
import concourse.bacc as bacc
import concourse.mybir as mybir
import concourse.tile as tile
from concourse.bass_utils import run_bass_kernel_spmd
from concourse.masks import make_identity

F32 = mybir.dt.float32
I32 = mybir.dt.int32
AX = mybir.AxisListType
OP = mybir.AluOpType

IM = 256
G = 512
J = 6
ALPHA = 2.34 * J
TWO_PI = 2.0 * np.pi
PAD = 517          # 512 + 2 left halo + 3 right halo
NT, NC, K = 8, 8, 16384
CELL = NC * 2      # floats per (p,q) cell = 16 (all coils interleaved)
TW = PAD * CELL    # table row width in elements = 8272
NTILE = 16         # point tiles of 1024 points (8 groups x 128 partitions)
GRP = 8            # groups per tile
DEG = 8            # KB weight polynomial degree (in t); abs err ~8.5e-6
NCELLS = PAD * PAD # flat cell count of the table


# ---------------------------------------------------------------- host consts
def _host_consts():
    # apodization correction 1/FT(kb)
    f = (np.arange(IM) - IM // 2) / G
    z = (np.pi * J * f) ** 2 - ALPHA ** 2
    s = np.sqrt(np.abs(z))
    val = np.where(z < 0, np.sinh(s) / np.maximum(s, 1e-12), np.sinc(s / np.pi))
    ftkb = (J / np.i0(ALPHA)) * val
    scal = 1.0 / ftkb
    # A[u, x'] = e^{i pi u/2 - 2 pi i u x'/G} * scal[x'] / sqrt(G)
    u = np.arange(G)[:, None].astype(np.float64)
    xp = np.arange(IM)[None, :].astype(np.float64)
    A = np.exp(1j * np.pi * u / 2 - 2j * np.pi * u * xp / G) * scal[None, :] / np.sqrt(G)
    art = np.ascontiguousarray(A.T.real, dtype=np.float32)   # [256, 512]
    ait = np.ascontiguousarray(A.T.imag, dtype=np.float32)
    aitn = np.ascontiguousarray(-A.T.imag, dtype=np.float32)
    # polynomial fit of w(t) = i0(ALPHA*sqrt(t))/i0(ALPHA) on t in [0,1]
    n = 512
    x = (1 - np.cos(np.pi * (np.arange(n) + 0.5) / n)) / 2
    w = np.i0(ALPHA * np.sqrt(x)) / np.i0(ALPHA)
    V = np.vander(x, DEG + 1, increasing=True)
    c, *_ = np.linalg.lstsq(V, w, rcond=None)
    return art, ait, aitn, c.astype(np.float64)


_ART, _AIT, _AITN, _CHEB = _host_consts()


# ---------------------------------------------------------------- bass build
def build_bass(debug=False):
    nc = bacc.Bacc()

    x_in = nc.declare_dram_parameter("x", [2, IM, IM], F32, isOutput=False)
    k_in = nc.declare_dram_parameter("kk", [2, K], F32, isOutput=False)
    c_in = nc.declare_dram_parameter("coil", [NC, 2, IM, IM], F32, isOutput=False)
    w_in = nc.declare_dram_parameter("wr", [128, NTILE * 128], F32, isOutput=False)
    art_in = nc.declare_dram_parameter("art", [IM, G], F32, isOutput=False)
    ait_in = nc.declare_dram_parameter("ait", [IM, G], F32, isOutput=False)
    aitn_in = nc.declare_dram_parameter("aitn", [IM, G], F32, isOutput=False)
    y_out = nc.declare_dram_parameter("yr", [128, NTILE * 128], F32, isOutput=True)

    BF16 = mybir.dt.bfloat16
    T_dram = nc.dram_tensor("T0", [PAD, TW], BF16)

    CH = _CHEB
    with tile.TileContext(nc) as tc:
        with (
            tc.tile_pool(name="const", bufs=1) as constp,
            tc.tile_pool(name="work", bufs=1) as workp,
            tc.tile_pool(name="ctile", bufs=2) as coilp,
            tc.tile_pool(name="mtile", bufs=4) as mp,
            tc.tile_pool(name="bt", bufs=8) as btp,
            tc.tile_pool(name="stg", bufs=1) as stgp,
            tc.tile_pool(name="patch", bufs=2) as patchp,
            tc.tile_pool(name="w36", bufs=2) as w36p,
            tc.tile_pool(name="wp", bufs=2) as wpp,
            tc.tile_pool(name="ps1", bufs=4, space="PSUM") as ps1,
            tc.tile_pool(name="ps2", bufs=4, space="PSUM") as ps2,
        ):
            # ---------------- constants ----------------
            ident = constp.tile([128, 128], F32, tag="ident")
            make_identity(nc, ident[:])
            # A matrices: DMA f32, convert once to bf16 for PE
            art = []
            for name, src in (("art", art_in), ("ait", ait_in), ("aitn", aitn_in)):
                ts_ = []
                for xt in range(2):
                    tf = constp.tile([128, G], F32, tag=f"{name}f{xt}")
                    nc.sync.dma_start(out=tf[:], in_=src[xt * 128:(xt + 1) * 128, :])
                    tb = constp.tile([128, G], BF16, tag=f"{name}b{xt}")
                    nc.scalar.copy(out=tb[:], in_=tf[:])
                    ts_.append(tb)
                art.append(ts_)
            artT, aitT, aitnT = art

            offs = constp.tile([128, J], F32, tag="offs")
            for a in range(J):
                nc.vector.memset(offs[:, a:a + 1], float(3 - (a + 1)))

            # ---------------- k -> [p, c] transpose ----------------
            kg = workp.tile([128, 256], F32, tag="kg")  # [p, (d, c)]
            for d in range(2):
                kt_in = workp.tile([128, 128], F32, tag="ktin")
                nc.sync.dma_start(
                    out=kt_in[:], in_=k_in[d].rearrange("(c p) -> c p", p=128)
                )
                ktp = ps2.tile([128, 128], F32, tag="psb")
                nc.tensor.transpose(ktp[:], kt_in[:], ident[:])
                nc.scalar.copy(out=kg[:, d * 128:(d + 1) * 128], in_=ktp[:])

            # ---------------- w load + sqrt ----------------
            wsq = workp.tile([128, NTILE * 128], F32, tag="wsq")
            nc.sync.dma_start(out=wsq[:], in_=w_in[:])
            nc.scalar.activation(
                out=wsq[:], in_=wsq[:],
                func=mybir.ActivationFunctionType.Sqrt,
            )

            # ---------------- index & weight math (DVE) ----------------
            # gx = om*(G/2pi); gx += 512 if gx < 0  -> [0, 512)
            gx0 = workp.tile([128, 256], F32, tag="gx0")
            nc.vector.tensor_scalar_mul(gx0[:], kg[:], float(G / TWO_PI))
            msk = workp.tile([128, 256], F32, tag="msk")
            nc.vector.tensor_scalar(
                out=msk[:], in0=gx0[:], scalar1=0.0, scalar2=None, op0=OP.is_lt
            )
            gxy = workp.tile([128, 256], F32, tag="gxy")
            nc.vector.scalar_tensor_tensor(
                out=gxy[:], in0=msk[:], scalar=float(G), in1=gx0[:],
                op0=OP.mult, op1=OP.add,
            )
            # gm3 = gxy - 3 ; f = rne(gm3 - 0.498) via 2^23 trick ; r = gm3 - f
            gm3 = workp.tile([128, 256], F32, tag="gm3")
            nc.vector.tensor_scalar(
                out=gm3[:], in0=gxy[:], scalar1=3.0, scalar2=None, op0=OP.subtract
            )
            fl = workp.tile([128, 256], F32, tag="fl")
            nc.vector.tensor_scalar(
                out=fl[:], in0=gm3[:],
                scalar1=-0.498046875, scalar2=12582912.0,
                op0=OP.add, op1=OP.add,
            )
            nc.vector.tensor_scalar(
                out=fl[:], in0=fl[:], scalar1=12582912.0, scalar2=None,
                op0=OP.subtract,
            )
            rr = workp.tile([128, 256], F32, tag="rr")
            nc.vector.tensor_sub(rr[:], gm3[:], fl[:])

            # U[p, (dc, a)] = r + (3 - a_idx)
            ut = workp.tile([128, 256 * J], F32, tag="ut")
            ut3 = ut[:].rearrange("p (dc a) -> p dc a", a=J)
            nc.vector.tensor_tensor(
                out=ut3,
                in0=rr[:].unsqueeze(2).broadcast_to([128, 256, J]),
                in1=offs[:].unsqueeze(1).broadcast_to([128, 256, J]),
                op=OP.add,
            )
            # t = max(0, 1 - (U/3)^2)
            tsq = workp.tile([128, 256 * J], F32, tag="tsq")
            nc.vector.tensor_mul(tsq[:], ut[:], ut[:])
            nc.vector.tensor_scalar(
                out=tsq[:], in0=tsq[:], scalar1=float(-1.0 / 9.0), scalar2=1.0,
                op0=OP.mult, op1=OP.add,
            )
            nc.vector.tensor_scalar_max(tsq[:], tsq[:], 0.0)
            # Horner in t
            acc = workp.tile([128, 256 * J], F32, tag="acc")
            nc.vector.tensor_scalar(
                out=acc[:], in0=tsq[:], scalar1=float(CH[DEG]),
                scalar2=float(CH[DEG - 1]), op0=OP.mult, op1=OP.add,
            )
            for d in range(DEG - 2, -1, -1):
                nc.vector.tensor_mul(acc[:], acc[:], tsq[:])
                nc.vector.tensor_scalar_add(acc[:], acc[:], float(CH[d]))
            # acc = W_all [p, (d, c, a)]: d=0 -> wx taps, d=1 -> wy taps

            # gather cell indices: flat = fy*517 + (b+2)*517 + 3 + fx
            cbt = constp.tile([128, J], F32, tag="cbt")
            for a in range(J):
                nc.vector.memset(cbt[:, a:a + 1], float(((a + 1) + 2) * PAD + 3))
            fy517 = workp.tile([128, 128], F32, tag="fy517")
            nc.vector.tensor_scalar_mul(fy517[:], fl[:, 128:256], float(PAD))
            idxf = workp.tile([128, 128 * J], F32, tag="idxf")
            idxf3 = idxf[:].rearrange("p (c b) -> p c b", b=J)
            nc.vector.tensor_tensor(
                out=idxf3,
                in0=fy517[:].unsqueeze(2).broadcast_to([128, 128, J]),
                in1=cbt[:].unsqueeze(1).broadcast_to([128, 128, J]),
                op=OP.add,
            )
            nc.vector.tensor_tensor(
                out=idxf3,
                in0=idxf3,
                in1=fl[:, 0:128].unsqueeze(2).broadcast_to([128, 128, J]),
                op=OP.add,
            )
            idx32 = workp.tile([128, 128 * J], I32, tag="idx32")
            nc.vector.tensor_copy(out=idx32[:], in_=idxf[:])

            # ---------------- res buffer ----------------
            res = workp.tile([128, NTILE * 128], F32, tag="res")

            # x image tiles (persist across all coils)
            xts = []
            for xt in range(2):
                xt_t = workp.tile([128, 2 * IM], F32, tag=f"xt{xt}")
                nc.sync.dma_start(
                    out=xt_t[:],
                    in_=x_in[:, xt * 128:(xt + 1) * 128, :]
                    .rearrange("ri x y -> x ri y"),
                )
                xts.append(xt_t)

            # 4 persistent bf16 stagings (one per v-tile), filled across coils
            stgs = []
            for vt in range(4):
                stg = stgp.tile([128, G * CELL], BF16, tag=f"stg{vt}")
                stgs.append(stg)

            for c in range(NC):
                # ---- coil multiply (bf16 out for PE) ----
                mt = []
                for xt in range(2):
                    ct = coilp.tile([128, 2 * IM], F32, tag="ct")
                    nc.sync.dma_start(
                        out=ct[:],
                        in_=c_in[c, :, xt * 128:(xt + 1) * 128, :]
                        .rearrange("ri x y -> x ri y"),
                    )
                    xt_t = xts[xt]
                    m = mp.tile([128, 2 * IM], BF16, tag="m")
                    xr, xi = xt_t[:, 0:IM], xt_t[:, IM:2 * IM]
                    cr, ci = ct[:, 0:IM], ct[:, IM:2 * IM]
                    mr, mi = m[:, 0:IM], m[:, IM:2 * IM]
                    t1 = mp.tile([128, IM], F32, tag="cm1")
                    t2 = mp.tile([128, IM], F32, tag="cm2")
                    nc.vector.tensor_mul(t1[:], xr, cr)
                    nc.vector.tensor_mul(t2[:], xi, ci)
                    nc.vector.tensor_sub(mr, t1[:], t2[:])
                    nc.vector.tensor_mul(t1[:], xr, ci)
                    nc.vector.tensor_mul(t2[:], xi, cr)
                    nc.vector.tensor_add(mi, t1[:], t2[:])
                    mt.append(m)
                # ---- stage 1: BT[y, u] per (ri, Yt) ----
                bt = {}
                for yt in range(2):
                    pr = ps1.tile([128, G], F32, tag="psa")
                    pi = ps1.tile([128, G], F32, tag="psa")
                    for xt in range(2):
                        mrb = mt[xt][:, yt * 128:yt * 128 + 128]
                        mib = mt[xt][:, IM + yt * 128:IM + yt * 128 + 128]
                        st = xt == 0
                        sp = xt == 1
                        nc.tensor.matmul(pr[:], mrb, artT[xt][:], start=st, stop=False)
                        nc.tensor.matmul(pi[:], mrb, aitT[xt][:], start=st, stop=False)
                        nc.tensor.matmul(pr[:], mib, aitnT[xt][:], start=False, stop=sp)
                        nc.tensor.matmul(pi[:], mib, artT[xt][:], start=False, stop=sp)
                    btr = btp.tile([128, G], BF16, tag="bt")
                    bti = btp.tile([128, G], BF16, tag="bt")
                    nc.scalar.copy(out=btr[:], in_=pr[:])
                    nc.scalar.copy(out=bti[:], in_=pi[:])
                    bt[(0, yt)] = btr
                    bt[(1, yt)] = bti
                # ---- stage 2: G[v, u], drain into stagings at cri slot ----
                for vt in range(4):
                    stg3 = stgs[vt][:].rearrange("p (u e) -> p u e", e=CELL)
                    gr = ps2.tile([128, G], F32, tag="psb")
                    gi = ps2.tile([128, G], F32, tag="psb")
                    for yt in range(2):
                        av = artT[yt][:, vt * 128:(vt + 1) * 128]
                        aiv = aitT[yt][:, vt * 128:(vt + 1) * 128]
                        ainv = aitnT[yt][:, vt * 128:(vt + 1) * 128]
                        btr = bt[(0, yt)]
                        bti = bt[(1, yt)]
                        st = yt == 0
                        sp = yt == 1
                        nc.tensor.matmul(gr[:], av, btr[:], start=st, stop=False)
                        nc.tensor.matmul(gi[:], aiv, btr[:], start=st, stop=False)
                        nc.tensor.matmul(gr[:], ainv, bti[:], start=False, stop=sp)
                        nc.tensor.matmul(gi[:], av, bti[:], start=False, stop=sp)
                    # split strided drains across Scalar and Vector engines
                    if c % 2 == 0:
                        nc.scalar.copy(out=stg3[:, :, 2 * c:2 * c + 1], in_=gr[:].unsqueeze(2))
                        nc.vector.tensor_copy(out=stg3[:, :, 2 * c + 1:2 * c + 2], in_=gi[:].unsqueeze(2))
                    else:
                        nc.vector.tensor_copy(out=stg3[:, :, 2 * c:2 * c + 1], in_=gr[:].unsqueeze(2))
                        nc.scalar.copy(out=stg3[:, :, 2 * c + 1:2 * c + 2], in_=gi[:].unsqueeze(2))

            # ---- table stores: main + q halos (+ p halos at vt 0 / 3) ----
            t_stores = []
            for vt in range(4):
                stg = stgs[vt]
                Th = T_dram
                r0 = vt * 128 + 2
                t_stores.append(nc.sync.dma_start(
                    out=Th[r0:r0 + 128, 2 * CELL:2 * CELL + G * CELL], in_=stg[:]
                ))
                t_stores.append(nc.sync.dma_start(
                    out=Th[r0:r0 + 128, 514 * CELL:514 * CELL + 3 * CELL],
                    in_=stg[:, 0:3 * CELL],
                ))
                t_stores.append(nc.sync.dma_start(
                    out=Th[r0:r0 + 128, 0:2 * CELL],
                    in_=stg[:, 510 * CELL:512 * CELL],
                ))
                if vt == 0:
                    t_stores += [
                        nc.sync.dma_start(
                            out=Th[514:517, 2 * CELL:2 * CELL + G * CELL],
                            in_=stg[0:3, :],
                        ),
                        nc.sync.dma_start(
                            out=Th[514:517, 514 * CELL:514 * CELL + 3 * CELL],
                            in_=stg[0:3, 0:3 * CELL],
                        ),
                        nc.sync.dma_start(
                            out=Th[514:517, 0:2 * CELL],
                            in_=stg[0:3, 510 * CELL:512 * CELL],
                        ),
                    ]
                if vt == 3:
                    t_stores += [
                        nc.sync.dma_start(
                            out=Th[0:2, 2 * CELL:2 * CELL + G * CELL],
                            in_=stg[126:128, :],
                        ),
                        nc.sync.dma_start(
                            out=Th[0:2, 514 * CELL:514 * CELL + 3 * CELL],
                            in_=stg[126:128, 0:3 * CELL],
                        ),
                        nc.sync.dma_start(
                            out=Th[0:2, 0:2 * CELL],
                            in_=stg[126:128, 510 * CELL:512 * CELL],
                        ),
                    ]

            # ======== gather + combine ========
            # per index: 96 contiguous elements (6 cells x 16 cri = 192B);
            # 48 indices per partition per call -> 16 gather instructions
            tab_flat = T_dram[:].rearrange("r (q e) -> (r q) e", e=CELL)
            all_gathers = []
            for t in range(NTILE):
                w36 = w36p.tile([128, GRP * J * J], F32, tag="w36")
                w363 = w36[:].rearrange("p (g b a) -> p g b a", b=J, a=J)
                wys = acc[:, 768 + t * 48: 768 + (t + 1) * 48].rearrange(
                    "p (g b) -> p g b", b=J)
                wxs = acc[:, t * 48:(t + 1) * 48].rearrange(
                    "p (g a) -> p g a", a=J)
                nc.vector.tensor_tensor(
                    out=w363,
                    in0=wys.unsqueeze(3).broadcast_to([128, GRP, J, J]),
                    in1=wxs.unsqueeze(2).broadcast_to([128, GRP, J, J]),
                    op=OP.mult,
                )
                patch = patchp.tile([128, GRP * J * J * CELL], BF16, tag="patch")
                for g in range(GRP):
                    for b in range(J):
                        col = (t * GRP + g) * J + b
                        gi_ = nc.gpsimd.indirect_dma_start(
                            out=patch[:, (g * J + b) * J * CELL:
                                      (g * J + b + 1) * J * CELL],
                            out_offset=None,
                            in_=tab_flat,
                            in_offset=bass.IndirectOffsetOnAxis(
                                ap=idx32[:, col:col + 1], axis=0
                            ),
                        )
                        all_gathers.append(gi_)
                # WP[p, (g, cr, ba)] = patch[p, (g, b, a, cr)] * W36
                wp = wpp.tile([128, GRP * J * J * CELL], BF16, tag="wpt")
                pv = bass.AP(
                    patch[:].tensor, patch[:].offset,
                    [patch[:].ap[0],
                     [J * J * CELL, GRP], [1, CELL], [CELL, J * J]],
                )
                wv = bass.AP(
                    w36[:].tensor, w36[:].offset,
                    [w36[:].ap[0], [J * J, GRP], [0, CELL], [1, J * J]],
                )
                ov = bass.AP(
                    wp[:].tensor, wp[:].offset,
                    [wp[:].ap[0],
                     [J * J * CELL, GRP], [J * J, CELL], [1, J * J]],
                )
                nc.vector.tensor_tensor(out=ov, in0=pv, in1=wv, op=OP.mult)
                # reduce innermost (b,a)=36 -> res[:, t*128 + g*16 + cr]
                rv = bass.AP(
                    res[:].tensor, res[:].offset + t * 128,
                    [res[:].ap[0], [16, GRP], [1, CELL]],
                )
                wp3 = wp[:].rearrange("p (g cr ba) -> p g cr ba", cr=CELL, ba=J * J)
                nc.vector.tensor_reduce(out=rv, in_=wp3, axis=AX.X, op=OP.add)

            # explicit RAW edges: gathers after table stores
            for gi_ in all_gathers:
                for si in t_stores:
                    tile.add_dep_helper(gi_.ins, si.ins, reason="T RAW")

            # ======== sqrt(w) scale + store ========
            nc.vector.tensor_mul(res[:], res[:], wsq[:])
            nc.sync.dma_start(out=y_out[:], in_=res[:])

            if debug:
                dbg_outs = {
                    "kgo": kg, "acco": acc, "idxo": idx32, "flo": fl, "rro": rr,
                }
                for nm, t_ in dbg_outs.items():
                    o = nc.dram_tensor(nm, list(t_[:].shape), t_[:].dtype,
                                       kind="ExternalOutput")
                    nc.sync.dma_start(out=o[:], in_=t_[:])
                o = nc.dram_tensor("t0o", [PAD, TW], BF16, kind="ExternalOutput")
                di = nc.sync.dma_start(out=o[:], in_=T_dram[:])
                for si in t_stores:
                    tile.add_dep_helper(di.ins, si.ins, reason="T dump RAW")

    nc.compile()
    return nc


_NC_CACHE = None


def _get_nc():
    global _NC_CACHE
    if _NC_CACHE is None:
        _NC_CACHE = build_bass()
    return _NC_CACHE


# ---------------------------------------------------------------- host glue
def _shuffle_w(w_t):
    # w[c, ri, K] -> [p, (t, g, c, ri)] with K = t*1024 + g*128 + p
    v = w_t.reshape(NC, 2, NTILE, GRP, 128)
    return np.ascontiguousarray(v.transpose(4, 2, 3, 0, 1).reshape(128, NTILE * 128))


def _unshuffle_y(yr):
    # [p, (t, g, c, ri)] -> y[c, ri, K]
    v = yr.reshape(128, NTILE, GRP, NC, 2)
    return np.ascontiguousarray(v.transpose(3, 4, 1, 2, 0).reshape(NC, 2, K))


def make_in_maps(x, k, coil_sensitivities, w):
    in_maps = []
    coil0 = np.ascontiguousarray(coil_sensitivities[0], dtype=np.float32)
    for t in range(NT):
        in_maps.append({
            "x": np.ascontiguousarray(x[t], dtype=np.float32),
            "kk": np.ascontiguousarray(k[t], dtype=np.float32),
            "coil": coil0,
            "wr": _shuffle_w(np.asarray(w[t], dtype=np.float32)),
            "art": _ART, "ait": _AIT, "aitn": _AITN,
        })
    return in_maps


def run(x, k, coil_sensitivities, w, trace=False, **spmd_kwargs):
    nc = _get_nc()
    in_maps = make_in_maps(x, k, coil_sensitivities, w)
    r = run_bass_kernel_spmd(nc, in_maps, list(range(NT)), trace=trace, **spmd_kwargs)
    y = np.stack([_unshuffle_y(r.results[t]["yr"]) for t in range(NT)], axis=0)
    return y.astype(np.float32), r


def kernel(x, k, coil_sensitivities, w):
    y, _ = run(x, k, coil_sensitivities, w, trace=False)
    return y


# revision 13
# speedup vs baseline: 1.1704x; 1.1704x over previous
"""Trainium2 Bass kernel for nn_RadialModel (forward NUFFT, radial MRI).

Per-core (1 frame, all 8 coils):
  1. coil multiply (DVE, bf16 out)       cimage = (xr+ixi)*(cr+ici)
  2. DFT via PE bf16 matmuls (two stages): G[v,u] = A @ (M^T @ A^T) with
     apodization + fftshift phases folded into the constant A matrices
  3. store grid to a DRAM table (bf16), coil-interleaved cells
     [p=v_pad(517), q=u_pad(517), cri(16)] with 2/3-cell wraparound halo
  4. Kaiser-Bessel 6x6 interpolation: indirect-DMA gathers (one index per
     partition per call is a HW limit of the SWDGE indirect path; each
     index fetches a contiguous 6-cell x 16-cri 192B run, 768 calls).
  5. weighted reduce on DVE, sqrt(w) scale + store

Sharding: one frame (nt) per NeuronCore, 8 cores. Host does only
shard/reshape/unshuffle; all math on device.
"""
import math
import numpy as np

import concourse.bass as bass
import concourse.bacc as bacc
import concourse.mybir as mybir
import concourse.tile as tile
from concourse.bass_utils import run_bass_kernel_spmd
from concourse.masks import make_identity

F32 = mybir.dt.float32
I32 = mybir.dt.int32
AX = mybir.AxisListType
OP = mybir.AluOpType

IM = 256
G = 512
J = 6
JT = 5             # taps actually gathered per dim: the offs=-3 tap always
                   # has |U| >= 2.5 where the KB weight is <= 2.6e-3 -> drop
ALPHA = 2.34 * J
TWO_PI = 2.0 * np.pi
PAD = 517          # 512 + 2 left halo + 3 right halo
NT, NC, K = 8, 8, 16384
CELL = NC * 2      # floats per (p,q) cell = 16 (all coils interleaved)
TW = PAD * CELL    # table row width in elements = 8272
NTILE = 16         # point tiles of 1024 points (8 groups x 128 partitions)
GRP = 8            # groups per tile
DEG = 8            # KB weight polynomial degree (in t); abs err ~8.5e-6
NCELLS = PAD * PAD # flat cell count of the table


# ---------------------------------------------------------------- host consts
def _host_consts():
    # apodization correction 1/FT(kb)
    f = (np.arange(IM) - IM // 2) / G
    z = (np.pi * J * f) ** 2 - ALPHA ** 2
    s = np.sqrt(np.abs(z))
    val = np.where(z < 0, np.sinh(s) / np.maximum(s, 1e-12), np.sinc(s / np.pi))
    ftkb = (J / np.i0(ALPHA)) * val
    scal = 1.0 / ftkb
    # A[u, x'] = e^{i pi u/2 - 2 pi i u x'/G} * scal[x'] / sqrt(G)
    u = np.arange(G)[:, None].astype(np.float64)
    xp = np.arange(IM)[None, :].astype(np.float64)
    A = np.exp(1j * np.pi * u / 2 - 2j * np.pi * u * xp / G) * scal[None, :] / np.sqrt(G)
    art = np.ascontiguousarray(A.T.real, dtype=np.float32)   # [256, 512]
    ait = np.ascontiguousarray(A.T.imag, dtype=np.float32)
    aitn = np.ascontiguousarray(-A.T.imag, dtype=np.float32)
    # polynomial fit of w(t) = i0(ALPHA*sqrt(t))/i0(ALPHA) on t in [0,1]
    n = 512
    x = (1 - np.cos(np.pi * (np.arange(n) + 0.5) / n)) / 2
    w = np.i0(ALPHA * np.sqrt(x)) / np.i0(ALPHA)
    V = np.vander(x, DEG + 1, increasing=True)
    c, *_ = np.linalg.lstsq(V, w, rcond=None)
    return art, ait, aitn, c.astype(np.float64)


_ART, _AIT, _AITN, _CHEB = _host_consts()


# ---------------------------------------------------------------- bass build
def build_bass(debug=False):
    nc = bacc.Bacc()

    x_in = nc.declare_dram_parameter("x", [2, IM, IM], F32, isOutput=False)
    k_in = nc.declare_dram_parameter("kk", [2, K], F32, isOutput=False)
    c_in = nc.declare_dram_parameter("coil", [NC, 2, IM, IM], F32, isOutput=False)
    w_in = nc.declare_dram_parameter("wr", [128, NTILE * 128], F32, isOutput=False)
    art_in = nc.declare_dram_parameter("art", [IM, G], F32, isOutput=False)
    ait_in = nc.declare_dram_parameter("ait", [IM, G], F32, isOutput=False)
    aitn_in = nc.declare_dram_parameter("aitn", [IM, G], F32, isOutput=False)
    y_out = nc.declare_dram_parameter("yr", [128, NTILE * 128], F32, isOutput=True)

    BF16 = mybir.dt.bfloat16
    T_dram = nc.dram_tensor("T0", [PAD, TW], BF16)

    CH = _CHEB
    with tile.TileContext(nc) as tc:
        with (
            tc.tile_pool(name="const", bufs=1) as constp,
            tc.tile_pool(name="work", bufs=1) as workp,
            tc.tile_pool(name="ctile", bufs=2) as coilp,
            tc.tile_pool(name="mtile", bufs=4) as mp,
            tc.tile_pool(name="bt", bufs=8) as btp,
            tc.tile_pool(name="stg", bufs=1) as stgp,
            tc.tile_pool(name="patch", bufs=2) as patchp,
            tc.tile_pool(name="w36", bufs=2) as w36p,
            tc.tile_pool(name="wp", bufs=2) as wpp,
            tc.tile_pool(name="ps1", bufs=4, space="PSUM") as ps1,
            tc.tile_pool(name="ps2", bufs=4, space="PSUM") as ps2,
        ):
            # ---------------- constants ----------------
            ident = constp.tile([128, 128], F32, tag="ident")
            make_identity(nc, ident[:])
            # A matrices: DMA f32, convert once to bf16 for PE
            art = []
            for name, src in (("art", art_in), ("ait", ait_in), ("aitn", aitn_in)):
                ts_ = []
                for xt in range(2):
                    tf = constp.tile([128, G], F32, tag=f"{name}f{xt}")
                    nc.sync.dma_start(out=tf[:], in_=src[xt * 128:(xt + 1) * 128, :])
                    tb = constp.tile([128, G], BF16, tag=f"{name}b{xt}")
                    nc.scalar.copy(out=tb[:], in_=tf[:])
                    ts_.append(tb)
                art.append(ts_)
            artT, aitT, aitnT = art

            offs = constp.tile([128, J], F32, tag="offs")
            for a in range(J):
                nc.vector.memset(offs[:, a:a + 1], float(3 - (a + 1)))

            # ---------------- k -> [p, c] transpose ----------------
            kg = workp.tile([128, 256], F32, tag="kg")  # [p, (d, c)]
            for d in range(2):
                kt_in = workp.tile([128, 128], F32, tag="ktin")
                nc.sync.dma_start(
                    out=kt_in[:], in_=k_in[d].rearrange("(c p) -> c p", p=128)
                )
                ktp = ps2.tile([128, 128], F32, tag="psb")
                nc.tensor.transpose(ktp[:], kt_in[:], ident[:])
                nc.scalar.copy(out=kg[:, d * 128:(d + 1) * 128], in_=ktp[:])

            # ---------------- w load + sqrt ----------------
            wsq = workp.tile([128, NTILE * 128], F32, tag="wsq")
            nc.sync.dma_start(out=wsq[:], in_=w_in[:])
            nc.scalar.activation(
                out=wsq[:], in_=wsq[:],
                func=mybir.ActivationFunctionType.Sqrt,
            )

            # ---------------- index & weight math (DVE) ----------------
            # gx = om*(G/2pi); gx += 512 if gx < 0  -> [0, 512)
            gx0 = workp.tile([128, 256], F32, tag="gx0")
            nc.vector.tensor_scalar_mul(gx0[:], kg[:], float(G / TWO_PI))
            msk = workp.tile([128, 256], F32, tag="msk")
            nc.vector.tensor_scalar(
                out=msk[:], in0=gx0[:], scalar1=0.0, scalar2=None, op0=OP.is_lt
            )
            gxy = workp.tile([128, 256], F32, tag="gxy")
            nc.vector.scalar_tensor_tensor(
                out=gxy[:], in0=msk[:], scalar=float(G), in1=gx0[:],
                op0=OP.mult, op1=OP.add,
            )
            # gm3 = gxy - 3 ; f = rne(gm3 - 0.498) via 2^23 trick ; r = gm3 - f
            gm3 = workp.tile([128, 256], F32, tag="gm3")
            nc.vector.tensor_scalar(
                out=gm3[:], in0=gxy[:], scalar1=3.0, scalar2=None, op0=OP.subtract
            )
            fl = workp.tile([128, 256], F32, tag="fl")
            nc.vector.tensor_scalar(
                out=fl[:], in0=gm3[:],
                scalar1=-0.498046875, scalar2=12582912.0,
                op0=OP.add, op1=OP.add,
            )
            nc.vector.tensor_scalar(
                out=fl[:], in0=fl[:], scalar1=12582912.0, scalar2=None,
                op0=OP.subtract,
            )
            rr = workp.tile([128, 256], F32, tag="rr")
            nc.vector.tensor_sub(rr[:], gm3[:], fl[:])

            # U[p, (dc, a)] = r + (3 - a_idx)
            ut = workp.tile([128, 256 * J], F32, tag="ut")
            ut3 = ut[:].rearrange("p (dc a) -> p dc a", a=J)
            nc.vector.tensor_tensor(
                out=ut3,
                in0=rr[:].unsqueeze(2).broadcast_to([128, 256, J]),
                in1=offs[:].unsqueeze(1).broadcast_to([128, 256, J]),
                op=OP.add,
            )
            # t = max(0, 1 - (U/3)^2)
            tsq = workp.tile([128, 256 * J], F32, tag="tsq")
            nc.vector.tensor_mul(tsq[:], ut[:], ut[:])
            nc.vector.tensor_scalar(
                out=tsq[:], in0=tsq[:], scalar1=float(-1.0 / 9.0), scalar2=1.0,
                op0=OP.mult, op1=OP.add,
            )
            nc.vector.tensor_scalar_max(tsq[:], tsq[:], 0.0)
            # Horner in t
            acc = workp.tile([128, 256 * J], F32, tag="acc")
            nc.vector.tensor_scalar(
                out=acc[:], in0=tsq[:], scalar1=float(CH[DEG]),
                scalar2=float(CH[DEG - 1]), op0=OP.mult, op1=OP.add,
            )
            for d in range(DEG - 2, -1, -1):
                nc.vector.tensor_mul(acc[:], acc[:], tsq[:])
                nc.vector.tensor_scalar_add(acc[:], acc[:], float(CH[d]))
            # acc = W_all [p, (d, c, a)]: d=0 -> wx taps, d=1 -> wy taps

            # gather cell indices: flat = fy*517 + (b+2)*517 + 3 + fx
            cbt = constp.tile([128, JT], F32, tag="cbt")
            for a in range(JT):
                nc.vector.memset(cbt[:, a:a + 1], float(((a + 1) + 2) * PAD + 3))
            fy517 = workp.tile([128, 128], F32, tag="fy517")
            nc.vector.tensor_scalar_mul(fy517[:], fl[:, 128:256], float(PAD))
            idxf = workp.tile([128, 128 * JT], F32, tag="idxf")
            idxf3 = idxf[:].rearrange("p (c b) -> p c b", b=JT)
            nc.vector.tensor_tensor(
                out=idxf3,
                in0=fy517[:].unsqueeze(2).broadcast_to([128, 128, JT]),
                in1=cbt[:].unsqueeze(1).broadcast_to([128, 128, JT]),
                op=OP.add,
            )
            nc.vector.tensor_tensor(
                out=idxf3,
                in0=idxf3,
                in1=fl[:, 0:128].unsqueeze(2).broadcast_to([128, 128, JT]),
                op=OP.add,
            )
            idx32 = workp.tile([128, 128 * JT], I32, tag="idx32")
            nc.vector.tensor_copy(out=idx32[:], in_=idxf[:])

            # ---------------- res buffer ----------------
            res = workp.tile([128, NTILE * 128], F32, tag="res")

            # x image tiles (persist across all coils)
            xts = []
            for xt in range(2):
                xt_t = workp.tile([128, 2 * IM], F32, tag=f"xt{xt}")
                nc.sync.dma_start(
                    out=xt_t[:],
                    in_=x_in[:, xt * 128:(xt + 1) * 128, :]
                    .rearrange("ri x y -> x ri y"),
                )
                xts.append(xt_t)

            # 4 persistent bf16 stagings (one per v-tile), filled across coils
            stgs = []
            for vt in range(4):
                stg = stgp.tile([128, G * CELL], BF16, tag=f"stg{vt}")
                stgs.append(stg)

            for c in range(NC):
                # ---- coil multiply (bf16 out for PE) ----
                mt = []
                for xt in range(2):
                    ct = coilp.tile([128, 2 * IM], F32, tag="ct")
                    nc.sync.dma_start(
                        out=ct[:],
                        in_=c_in[c, :, xt * 128:(xt + 1) * 128, :]
                        .rearrange("ri x y -> x ri y"),
                    )
                    xt_t = xts[xt]
                    m = mp.tile([128, 2 * IM], BF16, tag="m")
                    xr, xi = xt_t[:, 0:IM], xt_t[:, IM:2 * IM]
                    cr, ci = ct[:, 0:IM], ct[:, IM:2 * IM]
                    mr, mi = m[:, 0:IM], m[:, IM:2 * IM]
                    t1 = mp.tile([128, IM], F32, tag="cm1")
                    t2 = mp.tile([128, IM], F32, tag="cm2")
                    nc.vector.tensor_mul(t1[:], xr, cr)
                    nc.vector.tensor_mul(t2[:], xi, ci)
                    nc.vector.tensor_sub(mr, t1[:], t2[:])
                    nc.vector.tensor_mul(t1[:], xr, ci)
                    nc.vector.tensor_mul(t2[:], xi, cr)
                    nc.vector.tensor_add(mi, t1[:], t2[:])
                    mt.append(m)
                # ---- stage 1: BT[y, u] per (ri, Yt) ----
                bt = {}
                for yt in range(2):
                    pr = ps1.tile([128, G], F32, tag="psa")
                    pi = ps1.tile([128, G], F32, tag="psa")
                    for xt in range(2):
                        mrb = mt[xt][:, yt * 128:yt * 128 + 128]
                        mib = mt[xt][:, IM + yt * 128:IM + yt * 128 + 128]
                        st = xt == 0
                        sp = xt == 1
                        nc.tensor.matmul(pr[:], mrb, artT[xt][:], start=st, stop=False)
                        nc.tensor.matmul(pi[:], mrb, aitT[xt][:], start=st, stop=False)
                        nc.tensor.matmul(pr[:], mib, aitnT[xt][:], start=False, stop=sp)
                        nc.tensor.matmul(pi[:], mib, artT[xt][:], start=False, stop=sp)
                    btr = btp.tile([128, G], BF16, tag="bt")
                    bti = btp.tile([128, G], BF16, tag="bt")
                    nc.scalar.copy(out=btr[:], in_=pr[:])
                    nc.scalar.copy(out=bti[:], in_=pi[:])
                    bt[(0, yt)] = btr
                    bt[(1, yt)] = bti
                # ---- stage 2: G[v, u], drain into stagings at cri slot ----
                for vt in range(4):
                    stg3 = stgs[vt][:].rearrange("p (u e) -> p u e", e=CELL)
                    gr = ps2.tile([128, G], F32, tag="psb")
                    gi = ps2.tile([128, G], F32, tag="psb")
                    for yt in range(2):
                        av = artT[yt][:, vt * 128:(vt + 1) * 128]
                        aiv = aitT[yt][:, vt * 128:(vt + 1) * 128]
                        ainv = aitnT[yt][:, vt * 128:(vt + 1) * 128]
                        btr = bt[(0, yt)]
                        bti = bt[(1, yt)]
                        st = yt == 0
                        sp = yt == 1
                        nc.tensor.matmul(gr[:], av, btr[:], start=st, stop=False)
                        nc.tensor.matmul(gi[:], aiv, btr[:], start=st, stop=False)
                        nc.tensor.matmul(gr[:], ainv, bti[:], start=False, stop=sp)
                        nc.tensor.matmul(gi[:], av, bti[:], start=False, stop=sp)
                    # split strided drains across Scalar and Vector engines
                    if c % 2 == 0:
                        nc.scalar.copy(out=stg3[:, :, 2 * c:2 * c + 1], in_=gr[:].unsqueeze(2))
                        nc.vector.tensor_copy(out=stg3[:, :, 2 * c + 1:2 * c + 2], in_=gi[:].unsqueeze(2))
                    else:
                        nc.vector.tensor_copy(out=stg3[:, :, 2 * c:2 * c + 1], in_=gr[:].unsqueeze(2))
                        nc.scalar.copy(out=stg3[:, :, 2 * c + 1:2 * c + 2], in_=gi[:].unsqueeze(2))

            # ---- table stores: main + q halos (+ p halos at vt 0 / 3) ----
            t_stores = []
            for vt in range(4):
                stg = stgs[vt]
                Th = T_dram
                r0 = vt * 128 + 2
                t_stores.append(nc.sync.dma_start(
                    out=Th[r0:r0 + 128, 2 * CELL:2 * CELL + G * CELL], in_=stg[:]
                ))
                t_stores.append(nc.sync.dma_start(
                    out=Th[r0:r0 + 128, 514 * CELL:514 * CELL + 3 * CELL],
                    in_=stg[:, 0:3 * CELL],
                ))
                t_stores.append(nc.sync.dma_start(
                    out=Th[r0:r0 + 128, 0:2 * CELL],
                    in_=stg[:, 510 * CELL:512 * CELL],
                ))
                if vt == 0:
                    t_stores += [
                        nc.sync.dma_start(
                            out=Th[514:517, 2 * CELL:2 * CELL + G * CELL],
                            in_=stg[0:3, :],
                        ),
                        nc.sync.dma_start(
                            out=Th[514:517, 514 * CELL:514 * CELL + 3 * CELL],
                            in_=stg[0:3, 0:3 * CELL],
                        ),
                        nc.sync.dma_start(
                            out=Th[514:517, 0:2 * CELL],
                            in_=stg[0:3, 510 * CELL:512 * CELL],
                        ),
                    ]
                if vt == 3:
                    t_stores += [
                        nc.sync.dma_start(
                            out=Th[0:2, 2 * CELL:2 * CELL + G * CELL],
                            in_=stg[126:128, :],
                        ),
                        nc.sync.dma_start(
                            out=Th[0:2, 514 * CELL:514 * CELL + 3 * CELL],
                            in_=stg[126:128, 0:3 * CELL],
                        ),
                        nc.sync.dma_start(
                            out=Th[0:2, 0:2 * CELL],
                            in_=stg[126:128, 510 * CELL:512 * CELL],
                        ),
                    ]

            # ======== gather + combine ========
            # per index: 96 contiguous elements (6 cells = all x-taps, 192B);
            # 5 y-row taps per point, one index/partition/call -> 640 calls
            tab_flat = T_dram[:].rearrange("r (q e) -> (r q) e", e=CELL)
            all_gathers = []
            for t in range(NTILE):
                w36 = w36p.tile([128, GRP * JT * J], F32, tag="w36")
                w363 = w36[:].rearrange("p (g b a) -> p g b a", b=JT, a=J)
                wys = bass.AP(
                    acc[:].tensor, acc[:].offset + 128 * J + t * GRP * J,
                    [acc[:].ap[0], [J, GRP], [1, JT]],
                )
                wxs = bass.AP(
                    acc[:].tensor, acc[:].offset + t * GRP * J,
                    [acc[:].ap[0], [J, GRP], [1, J]],
                )
                nc.vector.tensor_tensor(
                    out=w363,
                    in0=wys.unsqueeze(3).broadcast_to([128, GRP, JT, J]),
                    in1=wxs.unsqueeze(2).broadcast_to([128, GRP, JT, J]),
                    op=OP.mult,
                )
                patch = patchp.tile([128, GRP * JT * J * CELL], BF16, tag="patch")
                for g in range(GRP):
                    for b in range(JT):
                        col = (t * GRP + g) * JT + b
                        gi_ = nc.gpsimd.indirect_dma_start(
                            out=patch[:, (g * JT + b) * J * CELL:
                                      (g * JT + b + 1) * J * CELL],
                            out_offset=None,
                            in_=tab_flat,
                            in_offset=bass.IndirectOffsetOnAxis(
                                ap=idx32[:, col:col + 1], axis=0
                            ),
                        )
                        all_gathers.append(gi_)
                # WP[p, (g, cr, ba)] = patch[p, (g, b, a, cr)] * W36
                wp = wpp.tile([128, GRP * JT * J * CELL], BF16, tag="wpt")
                pv = bass.AP(
                    patch[:].tensor, patch[:].offset,
                    [patch[:].ap[0],
                     [JT * J * CELL, GRP], [1, CELL], [CELL, JT * J]],
                )
                wv = bass.AP(
                    w36[:].tensor, w36[:].offset,
                    [w36[:].ap[0], [JT * J, GRP], [0, CELL], [1, JT * J]],
                )
                ov = bass.AP(
                    wp[:].tensor, wp[:].offset,
                    [wp[:].ap[0],
                     [JT * J * CELL, GRP], [JT * J, CELL], [1, JT * J]],
                )
                nc.vector.tensor_tensor(out=ov, in0=pv, in1=wv, op=OP.mult)
                # reduce innermost (b,a)=36 -> res[:, t*128 + g*16 + cr]
                rv = bass.AP(
                    res[:].tensor, res[:].offset + t * 128,
                    [res[:].ap[0], [16, GRP], [1, CELL]],
                )
                wp3 = wp[:].rearrange("p (g cr ba) -> p g cr ba", cr=CELL, ba=JT * J)
                nc.vector.tensor_reduce(out=rv, in_=wp3, axis=AX.X, op=OP.add)

            # explicit RAW edges: gathers after table stores
            for gi_ in all_gathers:
                for si in t_stores:
                    tile.add_dep_helper(gi_.ins, si.ins, reason="T RAW")

            # ======== sqrt(w) scale + store ========
            nc.vector.tensor_mul(res[:], res[:], wsq[:])
            nc.sync.dma_start(out=y_out[:], in_=res[:])

            if debug:
                dbg_outs = {
                    "kgo": kg, "acco": acc, "idxo": idx32, "flo": fl, "rro": rr,
                }
                for nm, t_ in dbg_outs.items():
                    o = nc.dram_tensor(nm, list(t_[:].shape), t_[:].dtype,
                                       kind="ExternalOutput")
                    nc.sync.dma_start(out=o[:], in_=t_[:])
                o = nc.dram_tensor("t0o", [PAD, TW], BF16, kind="ExternalOutput")
                di = nc.sync.dma_start(out=o[:], in_=T_dram[:])
                for si in t_stores:
                    tile.add_dep_helper(di.ins, si.ins, reason="T dump RAW")

    nc.compile()
    return nc


_NC_CACHE = None


def _get_nc():
    global _NC_CACHE
    if _NC_CACHE is None:
        _NC_CACHE = build_bass()
    return _NC_CACHE


# ---------------------------------------------------------------- host glue
def _shuffle_w(w_t):
    # w[c, ri, K] -> [p, (t, g, c, ri)] with K = t*1024 + g*128 + p
    v = w_t.reshape(NC, 2, NTILE, GRP, 128)
    return np.ascontiguousarray(v.transpose(4, 2, 3, 0, 1).reshape(128, NTILE * 128))


def _unshuffle_y(yr):
    # [p, (t, g, c, ri)] -> y[c, ri, K]
    v = yr.reshape(128, NTILE, GRP, NC, 2)
    return np.ascontiguousarray(v.transpose(3, 4, 1, 2, 0).reshape(NC, 2, K))


def make_in_maps(x, k, coil_sensitivities, w):
    in_maps = []
    coil0 = np.ascontiguousarray(coil_sensitivities[0], dtype=np.float32)
    for t in range(NT):
        in_maps.append({
            "x": np.ascontiguousarray(x[t], dtype=np.float32),
            "kk": np.ascontiguousarray(k[t], dtype=np.float32),
            "coil": coil0,
            "wr": _shuffle_w(np.asarray(w[t], dtype=np.float32)),
            "art": _ART, "ait": _AIT, "aitn": _AITN,
        })
    return in_maps


def run(x, k, coil_sensitivities, w, trace=False, **spmd_kwargs):
    nc = _get_nc()
    in_maps = make_in_maps(x, k, coil_sensitivities, w)
    r = run_bass_kernel_spmd(nc, in_maps, list(range(NT)), trace=trace, **spmd_kwargs)
    y = np.stack([_unshuffle_y(r.results[t]["yr"]) for t in range(NT)], axis=0)
    return y.astype(np.float32), r


def kernel(x, k, coil_sensitivities, w):
    y, _ = run(x, k, coil_sensitivities, w, trace=False)
    return y


# revision 14
# speedup vs baseline: 1.1985x; 1.0240x over previous
"""Trainium2 Bass kernel for nn_RadialModel (forward NUFFT, radial MRI).

Per-core (1 frame, all 8 coils):
  1. coil multiply (DVE, bf16 out)       cimage = (xr+ixi)*(cr+ici)
  2. DFT via PE bf16 matmuls (two stages): G[v,u] = A @ (M^T @ A^T) with
     apodization + fftshift phases folded into the constant A matrices
  3. store grid to a DRAM table (bf16), coil-interleaved cells
     [p=v_pad(517), q=u_pad(517), cri(16)] with 2/3-cell wraparound halo
  4. Kaiser-Bessel 6x6 interpolation: indirect-DMA gathers (one index per
     partition per call is a HW limit of the SWDGE indirect path; each
     index fetches a contiguous 6-cell x 16-cri 192B run, 768 calls).
  5. weighted reduce on DVE, sqrt(w) scale + store

Sharding: one frame (nt) per NeuronCore, 8 cores. Host does only
shard/reshape/unshuffle; all math on device.
"""
import math
import numpy as np

import concourse.bass as bass
import concourse.bacc as bacc
import concourse.mybir as mybir
import concourse.tile as tile
from concourse.bass_utils import run_bass_kernel_spmd
from concourse.masks import make_identity

F32 = mybir.dt.float32
I32 = mybir.dt.int32
AX = mybir.AxisListType
OP = mybir.AluOpType

IM = 256
G = 512
J = 6
JT = 5             # taps actually gathered per dim: the offs=-3 tap always
                   # has |U| >= 2.5 where the KB weight is <= 2.6e-3 -> drop
ALPHA = 2.34 * J
TWO_PI = 2.0 * np.pi
PAD = 517          # 512 + 2 left halo + 3 right halo
NT, NC, K = 8, 8, 16384
CELL = NC * 2      # floats per (p,q) cell = 16 (all coils interleaved)
TW = PAD * CELL    # table row width in elements = 8272
NTILE = 16         # point tiles of 1024 points (8 groups x 128 partitions)
GRP = 8            # groups per tile
DEG = 8            # KB weight polynomial degree (in t); abs err ~8.5e-6
NCELLS = PAD * PAD # flat cell count of the table


# ---------------------------------------------------------------- host consts
def _host_consts():
    # apodization correction 1/FT(kb)
    f = (np.arange(IM) - IM // 2) / G
    z = (np.pi * J * f) ** 2 - ALPHA ** 2
    s = np.sqrt(np.abs(z))
    val = np.where(z < 0, np.sinh(s) / np.maximum(s, 1e-12), np.sinc(s / np.pi))
    ftkb = (J / np.i0(ALPHA)) * val
    scal = 1.0 / ftkb
    # A[u, x'] = e^{i pi u/2 - 2 pi i u x'/G} * scal[x'] / sqrt(G)
    u = np.arange(G)[:, None].astype(np.float64)
    xp = np.arange(IM)[None, :].astype(np.float64)
    A = np.exp(1j * np.pi * u / 2 - 2j * np.pi * u * xp / G) * scal[None, :] / np.sqrt(G)
    art = np.ascontiguousarray(A.T.real, dtype=np.float32)   # [256, 512]
    ait = np.ascontiguousarray(A.T.imag, dtype=np.float32)
    aitn = np.ascontiguousarray(-A.T.imag, dtype=np.float32)
    # polynomial fit of w(t) = i0(ALPHA*sqrt(t))/i0(ALPHA) on t in [0,1]
    n = 512
    x = (1 - np.cos(np.pi * (np.arange(n) + 0.5) / n)) / 2
    w = np.i0(ALPHA * np.sqrt(x)) / np.i0(ALPHA)
    V = np.vander(x, DEG + 1, increasing=True)
    c, *_ = np.linalg.lstsq(V, w, rcond=None)
    return art, ait, aitn, c.astype(np.float64)


_ART, _AIT, _AITN, _CHEB = _host_consts()


# ---------------------------------------------------------------- bass build
def build_bass(debug=False):
    nc = bacc.Bacc()

    x_in = nc.declare_dram_parameter("x", [2, IM, IM], F32, isOutput=False)
    k_in = nc.declare_dram_parameter("kk", [2, K], F32, isOutput=False)
    c_in = nc.declare_dram_parameter("coil", [NC, 2, IM, IM], F32, isOutput=False)
    w_in = nc.declare_dram_parameter("wr", [128, NTILE * 128], F32, isOutput=False)
    art_in = nc.declare_dram_parameter("art", [IM, G], F32, isOutput=False)
    ait_in = nc.declare_dram_parameter("ait", [IM, G], F32, isOutput=False)
    aitn_in = nc.declare_dram_parameter("aitn", [IM, G], F32, isOutput=False)
    y_out = nc.declare_dram_parameter("yr", [128, NTILE * 128], F32, isOutput=True)

    BF16 = mybir.dt.bfloat16
    T_dram = nc.dram_tensor("T0", [PAD, TW], BF16)

    CH = _CHEB
    with tile.TileContext(nc) as tc:
        with (
            tc.tile_pool(name="const", bufs=1) as constp,
            tc.tile_pool(name="work", bufs=1) as workp,
            tc.tile_pool(name="ctile", bufs=2) as coilp,
            tc.tile_pool(name="mtile", bufs=4) as mp,
            tc.tile_pool(name="bt", bufs=8) as btp,
            tc.tile_pool(name="stg", bufs=1) as stgp,
            tc.tile_pool(name="patch", bufs=2) as patchp,
            tc.tile_pool(name="w36", bufs=2) as w36p,
            tc.tile_pool(name="wp", bufs=2) as wpp,
            tc.tile_pool(name="ps1", bufs=4, space="PSUM") as ps1,
            tc.tile_pool(name="ps2", bufs=4, space="PSUM") as ps2,
        ):
            # ---------------- constants ----------------
            ident = constp.tile([128, 128], F32, tag="ident")
            make_identity(nc, ident[:])
            # A matrices: DMA f32, convert once to bf16 for PE
            art = []
            for name, src in (("art", art_in), ("ait", ait_in), ("aitn", aitn_in)):
                ts_ = []
                for xt in range(2):
                    tf = constp.tile([128, G], F32, tag=f"{name}f{xt}")
                    nc.sync.dma_start(out=tf[:], in_=src[xt * 128:(xt + 1) * 128, :])
                    tb = constp.tile([128, G], BF16, tag=f"{name}b{xt}")
                    nc.scalar.copy(out=tb[:], in_=tf[:])
                    ts_.append(tb)
                art.append(ts_)
            artT, aitT, aitnT = art

            offs = constp.tile([128, JT], F32, tag="offs")
            for a in range(JT):
                nc.vector.memset(offs[:, a:a + 1], float(3 - (a + 1)))

            # ---------------- k -> [p, c] transpose ----------------
            kg = workp.tile([128, 256], F32, tag="kg")  # [p, (d, c)]
            for d in range(2):
                kt_in = workp.tile([128, 128], F32, tag="ktin")
                nc.sync.dma_start(
                    out=kt_in[:], in_=k_in[d].rearrange("(c p) -> c p", p=128)
                )
                ktp = ps2.tile([128, 128], F32, tag="psb")
                nc.tensor.transpose(ktp[:], kt_in[:], ident[:])
                nc.scalar.copy(out=kg[:, d * 128:(d + 1) * 128], in_=ktp[:])

            # ---------------- w load + sqrt ----------------
            wsq = workp.tile([128, NTILE * 128], F32, tag="wsq")
            nc.sync.dma_start(out=wsq[:], in_=w_in[:])
            nc.scalar.activation(
                out=wsq[:], in_=wsq[:],
                func=mybir.ActivationFunctionType.Sqrt,
            )

            # ---------------- index & weight math (DVE) ----------------
            # gx = om*(G/2pi); gx += 512 if gx < 0  -> [0, 512)
            gx0 = workp.tile([128, 256], F32, tag="gx0")
            nc.vector.tensor_scalar_mul(gx0[:], kg[:], float(G / TWO_PI))
            msk = workp.tile([128, 256], F32, tag="msk")
            nc.vector.tensor_scalar(
                out=msk[:], in0=gx0[:], scalar1=0.0, scalar2=None, op0=OP.is_lt
            )
            gxy = workp.tile([128, 256], F32, tag="gxy")
            nc.vector.scalar_tensor_tensor(
                out=gxy[:], in0=msk[:], scalar=float(G), in1=gx0[:],
                op0=OP.mult, op1=OP.add,
            )
            # gm3 = gxy - 3 ; f = rne(gm3 - 0.498) via 2^23 trick ; r = gm3 - f
            gm3 = workp.tile([128, 256], F32, tag="gm3")
            nc.vector.tensor_scalar(
                out=gm3[:], in0=gxy[:], scalar1=3.0, scalar2=None, op0=OP.subtract
            )
            fl = workp.tile([128, 256], F32, tag="fl")
            nc.vector.tensor_scalar(
                out=fl[:], in0=gm3[:],
                scalar1=0.0, scalar2=12582912.0,
                op0=OP.add, op1=OP.add,
            )
            nc.vector.tensor_scalar(
                out=fl[:], in0=fl[:], scalar1=12582912.0, scalar2=None,
                op0=OP.subtract,
            )
            rr = workp.tile([128, 256], F32, tag="rr")
            nc.vector.tensor_sub(rr[:], gm3[:], fl[:])

            # U[p, (dc, a)] = r + (3 - a_idx)
            ut = workp.tile([128, 256 * JT], F32, tag="ut")
            ut3 = ut[:].rearrange("p (dc a) -> p dc a", a=JT)
            nc.vector.tensor_tensor(
                out=ut3,
                in0=rr[:].unsqueeze(2).broadcast_to([128, 256, JT]),
                in1=offs[:].unsqueeze(1).broadcast_to([128, 256, JT]),
                op=OP.add,
            )
            # t = max(0, 1 - (U/3)^2)
            tsq = workp.tile([128, 256 * JT], F32, tag="tsq")
            nc.vector.tensor_mul(tsq[:], ut[:], ut[:])
            nc.vector.tensor_scalar(
                out=tsq[:], in0=tsq[:], scalar1=float(-1.0 / 9.0), scalar2=1.0,
                op0=OP.mult, op1=OP.add,
            )
            nc.vector.tensor_scalar_max(tsq[:], tsq[:], 0.0)
            # Horner in t
            acc = workp.tile([128, 256 * JT], F32, tag="acc")
            nc.vector.tensor_scalar(
                out=acc[:], in0=tsq[:], scalar1=float(CH[DEG]),
                scalar2=float(CH[DEG - 1]), op0=OP.mult, op1=OP.add,
            )
            for d in range(DEG - 2, -1, -1):
                nc.vector.tensor_mul(acc[:], acc[:], tsq[:])
                nc.vector.tensor_scalar_add(acc[:], acc[:], float(CH[d]))
            # acc = W_all [p, (d, c, a)]: d=0 -> wx taps, d=1 -> wy taps

            # gather cell indices: flat = fy*517 + (b+2)*517 + 3 + fx
            cbt = constp.tile([128, JT], F32, tag="cbt")
            for a in range(JT):
                nc.vector.memset(cbt[:, a:a + 1], float(((a + 1) + 2) * PAD + 3))
            fy517 = workp.tile([128, 128], F32, tag="fy517")
            nc.vector.tensor_scalar_mul(fy517[:], fl[:, 128:256], float(PAD))
            idxf = workp.tile([128, 128 * JT], F32, tag="idxf")
            idxf3 = idxf[:].rearrange("p (c b) -> p c b", b=JT)
            nc.vector.tensor_tensor(
                out=idxf3,
                in0=fy517[:].unsqueeze(2).broadcast_to([128, 128, JT]),
                in1=cbt[:].unsqueeze(1).broadcast_to([128, 128, JT]),
                op=OP.add,
            )
            nc.vector.tensor_tensor(
                out=idxf3,
                in0=idxf3,
                in1=fl[:, 0:128].unsqueeze(2).broadcast_to([128, 128, JT]),
                op=OP.add,
            )
            idx32 = workp.tile([128, 128 * JT], I32, tag="idx32")
            nc.vector.tensor_copy(out=idx32[:], in_=idxf[:])

            # ---------------- res buffer ----------------
            res = workp.tile([128, NTILE * 128], F32, tag="res")

            # x image tiles (persist across all coils)
            xts = []
            for xt in range(2):
                xt_t = workp.tile([128, 2 * IM], F32, tag=f"xt{xt}")
                nc.sync.dma_start(
                    out=xt_t[:],
                    in_=x_in[:, xt * 128:(xt + 1) * 128, :]
                    .rearrange("ri x y -> x ri y"),
                )
                xts.append(xt_t)

            # 4 persistent bf16 stagings (one per v-tile), filled across coils
            stgs = []
            for vt in range(4):
                stg = stgp.tile([128, G * CELL], BF16, tag=f"stg{vt}")
                stgs.append(stg)

            for c in range(NC):
                # ---- coil multiply (bf16 out for PE) ----
                mt = []
                for xt in range(2):
                    ct = coilp.tile([128, 2 * IM], F32, tag="ct")
                    nc.sync.dma_start(
                        out=ct[:],
                        in_=c_in[c, :, xt * 128:(xt + 1) * 128, :]
                        .rearrange("ri x y -> x ri y"),
                    )
                    xt_t = xts[xt]
                    m = mp.tile([128, 2 * IM], BF16, tag="m")
                    xr, xi = xt_t[:, 0:IM], xt_t[:, IM:2 * IM]
                    cr, ci = ct[:, 0:IM], ct[:, IM:2 * IM]
                    mr, mi = m[:, 0:IM], m[:, IM:2 * IM]
                    t1 = mp.tile([128, IM], F32, tag="cm1")
                    t2 = mp.tile([128, IM], F32, tag="cm2")
                    nc.vector.tensor_mul(t1[:], xr, cr)
                    nc.vector.tensor_mul(t2[:], xi, ci)
                    nc.vector.tensor_sub(mr, t1[:], t2[:])
                    nc.vector.tensor_mul(t1[:], xr, ci)
                    nc.vector.tensor_mul(t2[:], xi, cr)
                    nc.vector.tensor_add(mi, t1[:], t2[:])
                    mt.append(m)
                # ---- stage 1: BT[y, u] per (ri, Yt) ----
                bt = {}
                for yt in range(2):
                    pr = ps1.tile([128, G], F32, tag="psa")
                    pi = ps1.tile([128, G], F32, tag="psa")
                    for xt in range(2):
                        mrb = mt[xt][:, yt * 128:yt * 128 + 128]
                        mib = mt[xt][:, IM + yt * 128:IM + yt * 128 + 128]
                        st = xt == 0
                        sp = xt == 1
                        nc.tensor.matmul(pr[:], mrb, artT[xt][:], start=st, stop=False)
                        nc.tensor.matmul(pi[:], mrb, aitT[xt][:], start=st, stop=False)
                        nc.tensor.matmul(pr[:], mib, aitnT[xt][:], start=False, stop=sp)
                        nc.tensor.matmul(pi[:], mib, artT[xt][:], start=False, stop=sp)
                    btr = btp.tile([128, G], BF16, tag="bt")
                    bti = btp.tile([128, G], BF16, tag="bt")
                    nc.scalar.copy(out=btr[:], in_=pr[:])
                    nc.scalar.copy(out=bti[:], in_=pi[:])
                    bt[(0, yt)] = btr
                    bt[(1, yt)] = bti
                # ---- stage 2: G[v, u], drain into stagings at cri slot ----
                for vt in range(4):
                    stg3 = stgs[vt][:].rearrange("p (u e) -> p u e", e=CELL)
                    gr = ps2.tile([128, G], F32, tag="psb")
                    gi = ps2.tile([128, G], F32, tag="psb")
                    for yt in range(2):
                        av = artT[yt][:, vt * 128:(vt + 1) * 128]
                        aiv = aitT[yt][:, vt * 128:(vt + 1) * 128]
                        ainv = aitnT[yt][:, vt * 128:(vt + 1) * 128]
                        btr = bt[(0, yt)]
                        bti = bt[(1, yt)]
                        st = yt == 0
                        sp = yt == 1
                        nc.tensor.matmul(gr[:], av, btr[:], start=st, stop=False)
                        nc.tensor.matmul(gi[:], aiv, btr[:], start=st, stop=False)
                        nc.tensor.matmul(gr[:], ainv, bti[:], start=False, stop=sp)
                        nc.tensor.matmul(gi[:], av, bti[:], start=False, stop=sp)
                    # split strided drains across Scalar and Vector engines
                    if c % 2 == 0:
                        nc.scalar.copy(out=stg3[:, :, 2 * c:2 * c + 1], in_=gr[:].unsqueeze(2))
                        nc.vector.tensor_copy(out=stg3[:, :, 2 * c + 1:2 * c + 2], in_=gi[:].unsqueeze(2))
                    else:
                        nc.vector.tensor_copy(out=stg3[:, :, 2 * c:2 * c + 1], in_=gr[:].unsqueeze(2))
                        nc.scalar.copy(out=stg3[:, :, 2 * c + 1:2 * c + 2], in_=gi[:].unsqueeze(2))

            # ---- table stores: main + q halos (+ p halos at vt 0 / 3) ----
            t_stores = []
            for vt in range(4):
                stg = stgs[vt]
                Th = T_dram
                r0 = vt * 128 + 2
                t_stores.append(nc.sync.dma_start(
                    out=Th[r0:r0 + 128, 2 * CELL:2 * CELL + G * CELL], in_=stg[:]
                ))
                t_stores.append(nc.sync.dma_start(
                    out=Th[r0:r0 + 128, 514 * CELL:514 * CELL + 3 * CELL],
                    in_=stg[:, 0:3 * CELL],
                ))
                t_stores.append(nc.sync.dma_start(
                    out=Th[r0:r0 + 128, 0:2 * CELL],
                    in_=stg[:, 510 * CELL:512 * CELL],
                ))
                if vt == 0:
                    t_stores += [
                        nc.sync.dma_start(
                            out=Th[514:517, 2 * CELL:2 * CELL + G * CELL],
                            in_=stg[0:3, :],
                        ),
                        nc.sync.dma_start(
                            out=Th[514:517, 514 * CELL:514 * CELL + 3 * CELL],
                            in_=stg[0:3, 0:3 * CELL],
                        ),
                        nc.sync.dma_start(
                            out=Th[514:517, 0:2 * CELL],
                            in_=stg[0:3, 510 * CELL:512 * CELL],
                        ),
                    ]
                if vt == 3:
                    t_stores += [
                        nc.sync.dma_start(
                            out=Th[0:2, 2 * CELL:2 * CELL + G * CELL],
                            in_=stg[126:128, :],
                        ),
                        nc.sync.dma_start(
                            out=Th[0:2, 514 * CELL:514 * CELL + 3 * CELL],
                            in_=stg[126:128, 0:3 * CELL],
                        ),
                        nc.sync.dma_start(
                            out=Th[0:2, 0:2 * CELL],
                            in_=stg[126:128, 510 * CELL:512 * CELL],
                        ),
                    ]

            # ======== gather + combine ========
            # per index: 80 contiguous elements (5 cells x 16 cri = 160B);
            # one index per partition per call (HW limit) -> 640 calls
            tab_flat = T_dram[:].rearrange("r (q e) -> (r q) e", e=CELL)
            all_gathers = []
            for t in range(NTILE):
                w36 = w36p.tile([128, GRP * JT * JT], F32, tag="w36")
                w363 = w36[:].rearrange("p (g b a) -> p g b a", b=JT, a=JT)
                wys = acc[:, 128 * JT + t * GRP * JT:
                          128 * JT + (t + 1) * GRP * JT].rearrange(
                    "p (g b) -> p g b", b=JT)
                wxs = acc[:, t * GRP * JT:(t + 1) * GRP * JT].rearrange(
                    "p (g a) -> p g a", a=JT)
                nc.vector.tensor_tensor(
                    out=w363,
                    in0=wys.unsqueeze(3).broadcast_to([128, GRP, JT, JT]),
                    in1=wxs.unsqueeze(2).broadcast_to([128, GRP, JT, JT]),
                    op=OP.mult,
                )
                patch = patchp.tile([128, GRP * JT * JT * CELL], BF16, tag="patch")
                for g in range(GRP):
                    for b in range(JT):
                        col = (t * GRP + g) * JT + b
                        gi_ = nc.gpsimd.indirect_dma_start(
                            out=patch[:, (g * JT + b) * JT * CELL:
                                      (g * JT + b + 1) * JT * CELL],
                            out_offset=None,
                            in_=tab_flat,
                            in_offset=bass.IndirectOffsetOnAxis(
                                ap=idx32[:, col:col + 1], axis=0
                            ),
                        )
                        all_gathers.append(gi_)
                # WP[p, (g, cr, ba)] = patch[p, (g, b, a, cr)] * W36
                wp = wpp.tile([128, GRP * JT * JT * CELL], BF16, tag="wpt")
                pv = bass.AP(
                    patch[:].tensor, patch[:].offset,
                    [patch[:].ap[0],
                     [JT * JT * CELL, GRP], [1, CELL], [CELL, JT * JT]],
                )
                wv = bass.AP(
                    w36[:].tensor, w36[:].offset,
                    [w36[:].ap[0], [JT * JT, GRP], [0, CELL], [1, JT * JT]],
                )
                ov = bass.AP(
                    wp[:].tensor, wp[:].offset,
                    [wp[:].ap[0],
                     [JT * JT * CELL, GRP], [JT * JT, CELL], [1, JT * JT]],
                )
                nc.vector.tensor_tensor(out=ov, in0=pv, in1=wv, op=OP.mult)
                # reduce innermost (b,a)=36 -> res[:, t*128 + g*16 + cr]
                rv = bass.AP(
                    res[:].tensor, res[:].offset + t * 128,
                    [res[:].ap[0], [16, GRP], [1, CELL]],
                )
                wp3 = wp[:].rearrange("p (g cr ba) -> p g cr ba", cr=CELL, ba=JT * JT)
                nc.vector.tensor_reduce(out=rv, in_=wp3, axis=AX.X, op=OP.add)

            # explicit RAW edges: gathers after table stores
            for gi_ in all_gathers:
                for si in t_stores:
                    tile.add_dep_helper(gi_.ins, si.ins, reason="T RAW")

            # ======== sqrt(w) scale + store ========
            nc.vector.tensor_mul(res[:], res[:], wsq[:])
            nc.sync.dma_start(out=y_out[:], in_=res[:])

            if debug:
                dbg_outs = {
                    "kgo": kg, "acco": acc, "idxo": idx32, "flo": fl, "rro": rr,
                }
                for nm, t_ in dbg_outs.items():
                    o = nc.dram_tensor(nm, list(t_[:].shape), t_[:].dtype,
                                       kind="ExternalOutput")
                    nc.sync.dma_start(out=o[:], in_=t_[:])
                o = nc.dram_tensor("t0o", [PAD, TW], BF16, kind="ExternalOutput")
                di = nc.sync.dma_start(out=o[:], in_=T_dram[:])
                for si in t_stores:
                    tile.add_dep_helper(di.ins, si.ins, reason="T dump RAW")

    nc.compile()
    return nc


_NC_CACHE = None


def _get_nc():
    global _NC_CACHE
    if _NC_CACHE is None:
        _NC_CACHE = build_bass()
    return _NC_CACHE


# ---------------------------------------------------------------- host glue
def _shuffle_w(w_t):
    # w[c, ri, K] -> [p, (t, g, c, ri)] with K = t*1024 + g*128 + p
    v = w_t.reshape(NC, 2, NTILE, GRP, 128)
    return np.ascontiguousarray(v.transpose(4, 2, 3, 0, 1).reshape(128, NTILE * 128))


def _unshuffle_y(yr):
    # [p, (t, g, c, ri)] -> y[c, ri, K]
    v = yr.reshape(128, NTILE, GRP, NC, 2)
    return np.ascontiguousarray(v.transpose(3, 4, 1, 2, 0).reshape(NC, 2, K))


def make_in_maps(x, k, coil_sensitivities, w):
    in_maps = []
    coil0 = np.ascontiguousarray(coil_sensitivities[0], dtype=np.float32)
    for t in range(NT):
        in_maps.append({
            "x": np.ascontiguousarray(x[t], dtype=np.float32),
            "kk": np.ascontiguousarray(k[t], dtype=np.float32),
            "coil": coil0,
            "wr": _shuffle_w(np.asarray(w[t], dtype=np.float32)),
            "art": _ART, "ait": _AIT, "aitn": _AITN,
        })
    return in_maps


def run(x, k, coil_sensitivities, w, trace=False, **spmd_kwargs):
    nc = _get_nc()
    in_maps = make_in_maps(x, k, coil_sensitivities, w)
    r = run_bass_kernel_spmd(nc, in_maps, list(range(NT)), trace=trace, **spmd_kwargs)
    y = np.stack([_unshuffle_y(r.results[t]["yr"]) for t in range(NT)], axis=0)
    return y.astype(np.float32), r


def kernel(x, k, coil_sensitivities, w):
    y, _ = run(x, k, coil_sensitivities, w, trace=False)
    return y


# revision 16
# speedup vs baseline: 1.7198x; 1.4350x over previous
"""Trainium2 Bass kernel for nn_RadialModel (forward NUFFT, radial MRI).

Per-core (1 frame, all 8 coils):
  1. coil multiply (DVE, bf16 out)       cimage = (xr+ixi)*(cr+ici)
  2. DFT via PE bf16 matmuls (two stages): G[v,u] = A @ (M^T @ A^T) with
     apodization + fftshift phases folded into the constant A matrices
  3. store grid to a DRAM table (bf16), coil-interleaved cells
     [p=v_pad(517), q=u_pad(517), cri(16)] with 2/3-cell wraparound halo
  4. Kaiser-Bessel interpolation with round-to-nearest centering so the
     fractional offset is in [-0.5, 0.5]: a 5x5 tap window then captures
     every tap with weight > 2.6e-3 (the 6th tap of the reference's 6x6
     always has |U| >= 2.5 there).  Indirect-DMA gathers: one index per
     partition per call is a HW limit of the SWDGE indirect path (~1.25us
     fixed cost each); each index fetches a contiguous 5-cell x 16-cri
     160B run -> 640 calls, the dominant cost of the kernel.
  5. weighted reduce on DVE, sqrt(w) scale + store

Sharding: one frame (nt) per NeuronCore, 8 cores. Host does only
shard/reshape/unshuffle; all math on device.
"""
import math
import numpy as np

import concourse.bass as bass
import concourse.bacc as bacc
import concourse.mybir as mybir
import concourse.tile as tile
from concourse.bass_utils import run_bass_kernel_spmd
from concourse.masks import make_identity

F32 = mybir.dt.float32
I32 = mybir.dt.int32
AX = mybir.AxisListType
OP = mybir.AluOpType

IM = 256
G = 512
J = 6
JT = 5             # taps actually gathered per dim: the offs=-3 tap always
                   # has |U| >= 2.5 where the KB weight is <= 2.6e-3 -> drop
ALPHA = 2.34 * J
TWO_PI = 2.0 * np.pi
PAD = 517          # 512 + 2 left halo + 3 right halo
NT, NC, K = 8, 8, 16384
CELL = NC * 2      # floats per (p,q) cell = 16 (all coils interleaved)
TW = PAD * CELL    # table row width in elements = 8272
NTILE = 16         # point tiles of 1024 points (8 groups x 128 partitions)
GRP = 8            # groups per tile
DEG = 8            # KB weight polynomial degree (in t); abs err ~8.5e-6
NCELLS = PAD * PAD # flat cell count of the table


# ---------------------------------------------------------------- host consts
def _host_consts():
    # apodization correction 1/FT(kb)
    f = (np.arange(IM) - IM // 2) / G
    z = (np.pi * J * f) ** 2 - ALPHA ** 2
    s = np.sqrt(np.abs(z))
    val = np.where(z < 0, np.sinh(s) / np.maximum(s, 1e-12), np.sinc(s / np.pi))
    ftkb = (J / np.i0(ALPHA)) * val
    scal = 1.0 / ftkb
    # A[u, x'] = e^{i pi u/2 - 2 pi i u x'/G} * scal[x'] / sqrt(G)
    u = np.arange(G)[:, None].astype(np.float64)
    xp = np.arange(IM)[None, :].astype(np.float64)
    A = np.exp(1j * np.pi * u / 2 - 2j * np.pi * u * xp / G) * scal[None, :] / np.sqrt(G)
    art = np.ascontiguousarray(A.T.real, dtype=np.float32)   # [256, 512]
    ait = np.ascontiguousarray(A.T.imag, dtype=np.float32)
    aitn = np.ascontiguousarray(-A.T.imag, dtype=np.float32)
    # polynomial fit of w(t) = i0(ALPHA*sqrt(t))/i0(ALPHA) on t in [0,1]
    n = 512
    x = (1 - np.cos(np.pi * (np.arange(n) + 0.5) / n)) / 2
    w = np.i0(ALPHA * np.sqrt(x)) / np.i0(ALPHA)
    V = np.vander(x, DEG + 1, increasing=True)
    c, *_ = np.linalg.lstsq(V, w, rcond=None)
    return art, ait, aitn, c.astype(np.float64)


_ART, _AIT, _AITN, _CHEB = _host_consts()


# ---------------------------------------------------------------- bass build
def build_bass(debug=False):
    nc = bacc.Bacc()

    x_in = nc.declare_dram_parameter("x", [2, IM, IM], F32, isOutput=False)
    k_in = nc.declare_dram_parameter("kk", [2, K], F32, isOutput=False)
    c_in = nc.declare_dram_parameter("coil", [NC, 2, IM, IM], F32, isOutput=False)
    w_in = nc.declare_dram_parameter("wr", [128, NTILE * 128], F32, isOutput=False)
    art_in = nc.declare_dram_parameter("art", [IM, G], F32, isOutput=False)
    ait_in = nc.declare_dram_parameter("ait", [IM, G], F32, isOutput=False)
    aitn_in = nc.declare_dram_parameter("aitn", [IM, G], F32, isOutput=False)
    y_out = nc.declare_dram_parameter("yr", [128, NTILE * 128], F32, isOutput=True)

    BF16 = mybir.dt.bfloat16
    W2 = 2 * CELL          # paired-row cell: 2 v-rows x 16 cri = 32 el
    TW2 = PAD * W2         # table P-row width = 16544 el
    PROWS = 259            # pairs of table rows 0..517 (row 517 = v3 halo)
    T_dram = nc.dram_tensor("T0", [PROWS, TW2], BF16)

    CH = _CHEB
    with tile.TileContext(nc) as tc:
        with (
            tc.tile_pool(name="const", bufs=1) as constp,
            tc.tile_pool(name="work", bufs=1) as workp,
            tc.tile_pool(name="ctile", bufs=2) as coilp,
            tc.tile_pool(name="mtile", bufs=4) as mp,
            tc.tile_pool(name="bt", bufs=8) as btp,
            tc.tile_pool(name="stg", bufs=1) as stgp,
            tc.tile_pool(name="patch", bufs=2) as patchp,
            tc.tile_pool(name="w36", bufs=2) as w36p,
            tc.tile_pool(name="wp", bufs=2) as wpp,
            tc.tile_pool(name="ps1", bufs=4, space="PSUM") as ps1,
            tc.tile_pool(name="ps2", bufs=4, space="PSUM") as ps2,
        ):
            # ---------------- constants ----------------
            ident = constp.tile([128, 128], F32, tag="ident")
            make_identity(nc, ident[:])
            # A matrices: DMA f32, convert once to bf16 for PE
            art = []
            for name, src in (("art", art_in), ("ait", ait_in), ("aitn", aitn_in)):
                ts_ = []
                for xt in range(2):
                    tf = constp.tile([128, G], F32, tag=f"{name}f{xt}")
                    nc.sync.dma_start(out=tf[:], in_=src[xt * 128:(xt + 1) * 128, :])
                    tb = constp.tile([128, G], BF16, tag=f"{name}b{xt}")
                    nc.scalar.copy(out=tb[:], in_=tf[:])
                    ts_.append(tb)
                art.append(ts_)
            artT, aitT, aitnT = art

            offs = constp.tile([128, JT], F32, tag="offs")
            for a in range(JT):
                nc.vector.memset(offs[:, a:a + 1], float(3 - (a + 1)))
            ylat = constp.tile([128, 6], F32, tag="ylat")
            for a in range(6):
                nc.vector.memset(ylat[:, a:a + 1], float(-a))

            # ---------------- k -> [p, c] transpose ----------------
            kg = workp.tile([128, 256], F32, tag="kg")  # [p, (d, c)]
            for d in range(2):
                kt_in = workp.tile([128, 128], F32, tag="ktin")
                nc.sync.dma_start(
                    out=kt_in[:], in_=k_in[d].rearrange("(c p) -> c p", p=128)
                )
                ktp = ps2.tile([128, 128], F32, tag="psb")
                nc.tensor.transpose(ktp[:], kt_in[:], ident[:])
                nc.scalar.copy(out=kg[:, d * 128:(d + 1) * 128], in_=ktp[:])

            # ---------------- w load + sqrt ----------------
            wsq = workp.tile([128, NTILE * 128], F32, tag="wsq")
            nc.sync.dma_start(out=wsq[:], in_=w_in[:])
            nc.scalar.activation(
                out=wsq[:], in_=wsq[:],
                func=mybir.ActivationFunctionType.Sqrt,
            )

            # ---------------- index & weight math (DVE) ----------------
            # gx = om*(G/2pi); gx += 512 if gx < 0  -> [0, 512)
            gx0 = workp.tile([128, 256], F32, tag="gx0")
            nc.vector.tensor_scalar_mul(gx0[:], kg[:], float(G / TWO_PI))
            msk = workp.tile([128, 256], F32, tag="msk")
            nc.vector.tensor_scalar(
                out=msk[:], in0=gx0[:], scalar1=0.0, scalar2=None, op0=OP.is_lt
            )
            gxy = workp.tile([128, 256], F32, tag="gxy")
            nc.vector.scalar_tensor_tensor(
                out=gxy[:], in0=msk[:], scalar=float(G), in1=gx0[:],
                op0=OP.mult, op1=OP.add,
            )
            # gm3 = gxy - 3 ; f = rne(gm3 - 0.498) via 2^23 trick ; r = gm3 - f
            gm3 = workp.tile([128, 256], F32, tag="gm3")
            nc.vector.tensor_scalar(
                out=gm3[:], in0=gxy[:], scalar1=3.0, scalar2=None, op0=OP.subtract
            )
            fl = workp.tile([128, 256], F32, tag="fl")
            nc.vector.tensor_scalar(
                out=fl[:], in0=gm3[:],
                scalar1=0.0, scalar2=12582912.0,
                op0=OP.add, op1=OP.add,
            )
            nc.vector.tensor_scalar(
                out=fl[:], in0=fl[:], scalar1=12582912.0, scalar2=None,
                op0=OP.subtract,
            )
            rr = workp.tile([128, 256], F32, tag="rr")
            nc.vector.tensor_sub(rr[:], gm3[:], fl[:])

            # P0 = floor((fl_y + 3)/2): h = fl*0.5 + 1.5, rne(h - 0.498)
            fp = workp.tile([128, 128], F32, tag="fp")
            nc.vector.tensor_scalar(
                out=fp[:], in0=fl[:, 128:256], scalar1=0.5, scalar2=1.5,
                op0=OP.mult, op1=OP.add,
            )
            nc.vector.tensor_scalar(
                out=fp[:], in0=fp[:],
                scalar1=-0.498046875, scalar2=12582912.0,
                op0=OP.add, op1=OP.add,
            )
            nc.vector.tensor_scalar(
                out=fp[:], in0=fp[:], scalar1=12582912.0, scalar2=None,
                op0=OP.subtract,
            )
            # sY = gy + 2 - 2*P0 = gm3_y + 5 - 2*fp; y-tap j weight arg =
            # sY - j for gathered row-pair rows 2*P0 + j, j = 0..5
            sY = workp.tile([128, 128], F32, tag="sY")
            nc.vector.scalar_tensor_tensor(
                out=sY[:], in0=fp[:], scalar=-2.0, in1=gm3[:, 128:256],
                op0=OP.mult, op1=OP.add,
            )
            nc.vector.tensor_scalar_add(sY[:], sY[:], 5.0)

            # tap weight args: x: rr_x + offs (5); y: sY - j (6)
            NXC = 128 * JT
            ut = workp.tile([128, NXC + 128 * 6], F32, tag="ut")
            utx3 = ut[:, 0:NXC].rearrange("p (c a) -> p c a", a=JT)
            nc.vector.tensor_tensor(
                out=utx3,
                in0=rr[:, 0:128].unsqueeze(2).broadcast_to([128, 128, JT]),
                in1=offs[:].unsqueeze(1).broadcast_to([128, 128, JT]),
                op=OP.add,
            )
            uty3 = ut[:, NXC:].rearrange("p (c j) -> p c j", j=6)
            nc.vector.tensor_tensor(
                out=uty3,
                in0=sY[:].unsqueeze(2).broadcast_to([128, 128, 6]),
                in1=ylat[:].unsqueeze(1).broadcast_to([128, 128, 6]),
                op=OP.add,
            )
            # t = max(0, 1 - (U/3)^2)
            tsq = workp.tile([128, 128 * JT + 128 * 6], F32, tag="tsq")
            nc.vector.tensor_mul(tsq[:], ut[:], ut[:])
            nc.vector.tensor_scalar(
                out=tsq[:], in0=tsq[:], scalar1=float(-1.0 / 9.0), scalar2=1.0,
                op0=OP.mult, op1=OP.add,
            )
            nc.vector.tensor_scalar_max(tsq[:], tsq[:], 0.0)
            # Horner in t
            acc = workp.tile([128, 128 * JT + 128 * 6], F32, tag="acc")
            nc.vector.tensor_scalar(
                out=acc[:], in0=tsq[:], scalar1=float(CH[DEG]),
                scalar2=float(CH[DEG - 1]), op0=OP.mult, op1=OP.add,
            )
            for d in range(DEG - 2, -1, -1):
                nc.vector.tensor_mul(acc[:], acc[:], tsq[:])
                nc.vector.tensor_scalar_add(acc[:], acc[:], float(CH[d]))
            # acc = W_all [p, (d, c, a)]: d=0 -> wx taps, d=1 -> wy taps

            # gather cell2 indices: (P0 + b2)*517 + 3 + fx, b2 = 0..2
            cbt = constp.tile([128, 3], F32, tag="cbt")
            for a in range(3):
                nc.vector.memset(cbt[:, a:a + 1], float(a * PAD + 3))
            fy517 = workp.tile([128, 128], F32, tag="fy517")
            nc.vector.tensor_scalar_mul(fy517[:], fp[:], float(PAD))
            idxf = workp.tile([128, 128 * 3], F32, tag="idxf")
            idxf3 = idxf[:].rearrange("p (c b) -> p c b", b=3)
            nc.vector.tensor_tensor(
                out=idxf3,
                in0=fy517[:].unsqueeze(2).broadcast_to([128, 128, 3]),
                in1=cbt[:].unsqueeze(1).broadcast_to([128, 128, 3]),
                op=OP.add,
            )
            nc.vector.tensor_tensor(
                out=idxf3,
                in0=idxf3,
                in1=fl[:, 0:128].unsqueeze(2).broadcast_to([128, 128, 3]),
                op=OP.add,
            )
            idx32 = workp.tile([128, 128 * 3], I32, tag="idx32")
            nc.vector.tensor_copy(out=idx32[:], in_=idxf[:])

            # ---------------- res buffer ----------------
            res = workp.tile([128, NTILE * 128], F32, tag="res")

            # x image tiles (persist across all coils)
            xts = []
            for xt in range(2):
                xt_t = workp.tile([128, 2 * IM], F32, tag=f"xt{xt}")
                nc.sync.dma_start(
                    out=xt_t[:],
                    in_=x_in[:, xt * 128:(xt + 1) * 128, :]
                    .rearrange("ri x y -> x ri y"),
                )
                xts.append(xt_t)

            # 2 persistent bf16 stagings; partition p of staging S holds
            # the v-row pair (S*256 + 2p, S*256 + 2p + 1), cells2 (q, r2, e)
            stgs = []
            for S in range(2):
                stg = stgp.tile([128, G * W2], BF16, tag=f"stg{S}")
                stgs.append(stg)

            for c in range(NC):
                # ---- coil multiply (bf16 out for PE) ----
                mt = []
                for xt in range(2):
                    ct = coilp.tile([128, 2 * IM], F32, tag="ct")
                    nc.sync.dma_start(
                        out=ct[:],
                        in_=c_in[c, :, xt * 128:(xt + 1) * 128, :]
                        .rearrange("ri x y -> x ri y"),
                    )
                    xt_t = xts[xt]
                    m = mp.tile([128, 2 * IM], BF16, tag="m")
                    xr, xi = xt_t[:, 0:IM], xt_t[:, IM:2 * IM]
                    cr, ci = ct[:, 0:IM], ct[:, IM:2 * IM]
                    mr, mi = m[:, 0:IM], m[:, IM:2 * IM]
                    t1 = mp.tile([128, IM], F32, tag="cm1")
                    t2 = mp.tile([128, IM], F32, tag="cm2")
                    nc.vector.tensor_mul(t1[:], xr, cr)
                    nc.vector.tensor_mul(t2[:], xi, ci)
                    nc.vector.tensor_sub(mr, t1[:], t2[:])
                    nc.vector.tensor_mul(t1[:], xr, ci)
                    nc.vector.tensor_mul(t2[:], xi, cr)
                    nc.vector.tensor_add(mi, t1[:], t2[:])
                    mt.append(m)
                # ---- stage 1: BT[y, u] per (ri, Yt) ----
                bt = {}
                for yt in range(2):
                    pr = ps1.tile([128, G], F32, tag="psa")
                    pi = ps1.tile([128, G], F32, tag="psa")
                    for xt in range(2):
                        mrb = mt[xt][:, yt * 128:yt * 128 + 128]
                        mib = mt[xt][:, IM + yt * 128:IM + yt * 128 + 128]
                        st = xt == 0
                        sp = xt == 1
                        nc.tensor.matmul(pr[:], mrb, artT[xt][:], start=st, stop=False)
                        nc.tensor.matmul(pi[:], mrb, aitT[xt][:], start=st, stop=False)
                        nc.tensor.matmul(pr[:], mib, aitnT[xt][:], start=False, stop=sp)
                        nc.tensor.matmul(pi[:], mib, artT[xt][:], start=False, stop=sp)
                    btr = btp.tile([128, G], BF16, tag="bt")
                    bti = btp.tile([128, G], BF16, tag="bt")
                    nc.scalar.copy(out=btr[:], in_=pr[:])
                    nc.scalar.copy(out=bti[:], in_=pi[:])
                    bt[(0, yt)] = btr
                    bt[(1, yt)] = bti
                # ---- stage 2: G[v, u] with v = S*256 + 2p + r2 via
                # stride-2 A column slices; drain into paired-row stagings ----
                for half in range(4):
                    S, r2 = half // 2, half % 2
                    voff = S * 256 + r2
                    stg3 = stgs[S][:].rearrange("p (q w) -> p q w", w=W2)
                    gr = ps2.tile([128, G], F32, tag="psb")
                    gi = ps2.tile([128, G], F32, tag="psb")
                    for yt in range(2):
                        a0 = artT[yt][:]
                        av = bass.AP(a0.tensor, a0.offset + voff,
                                     [a0.ap[0], [2, 128]])
                        i0_ = aitT[yt][:]
                        aiv = bass.AP(i0_.tensor, i0_.offset + voff,
                                      [i0_.ap[0], [2, 128]])
                        n0 = aitnT[yt][:]
                        ainv = bass.AP(n0.tensor, n0.offset + voff,
                                       [n0.ap[0], [2, 128]])
                        btr = bt[(0, yt)]
                        bti = bt[(1, yt)]
                        st = yt == 0
                        sp = yt == 1
                        nc.tensor.matmul(gr[:], av, btr[:], start=st, stop=False)
                        nc.tensor.matmul(gi[:], aiv, btr[:], start=st, stop=False)
                        nc.tensor.matmul(gr[:], ainv, bti[:], start=False, stop=sp)
                        nc.tensor.matmul(gi[:], av, bti[:], start=False, stop=sp)
                    c2 = r2 * CELL + 2 * c
                    # split strided drains across Scalar and Vector engines
                    if c % 2 == 0:
                        nc.scalar.copy(out=stg3[:, :, c2:c2 + 1], in_=gr[:].unsqueeze(2))
                        nc.vector.tensor_copy(out=stg3[:, :, c2 + 1:c2 + 2], in_=gi[:].unsqueeze(2))
                    else:
                        nc.vector.tensor_copy(out=stg3[:, :, c2:c2 + 1], in_=gr[:].unsqueeze(2))
                        nc.scalar.copy(out=stg3[:, :, c2 + 1:c2 + 2], in_=gi[:].unsqueeze(2))

            # ---- table stores: mains (P 1..128, 129..256) + q halos,
            # then halo pair-rows P0 (v 510,511), P257 (v 0,1), P258 (v 2,3)
            t_stores = []
            Th = T_dram
            for S in range(2):
                stg = stgs[S]
                r0 = 1 + S * 128
                t_stores.append(nc.sync.dma_start(
                    out=Th[r0:r0 + 128, 2 * W2:2 * W2 + G * W2], in_=stg[:]
                ))
                t_stores.append(nc.sync.dma_start(
                    out=Th[r0:r0 + 128, 514 * W2:517 * W2],
                    in_=stg[:, 0:3 * W2],
                ))
                t_stores.append(nc.sync.dma_start(
                    out=Th[r0:r0 + 128, 0:2 * W2],
                    in_=stg[:, 510 * W2:512 * W2],
                ))
            for dst, ssrc, psrc in ((0, 1, 127), (257, 0, 0), (258, 0, 1)):
                stg = stgs[ssrc]
                t_stores += [
                    nc.sync.dma_start(
                        out=Th[dst:dst + 1, 2 * W2:2 * W2 + G * W2],
                        in_=stg[psrc:psrc + 1, :],
                    ),
                    nc.sync.dma_start(
                        out=Th[dst:dst + 1, 514 * W2:517 * W2],
                        in_=stg[psrc:psrc + 1, 0:3 * W2],
                    ),
                    nc.sync.dma_start(
                        out=Th[dst:dst + 1, 0:2 * W2],
                        in_=stg[psrc:psrc + 1, 510 * W2:512 * W2],
                    ),
                ]

            # ======== gather + combine ========
            # per index: 160 contiguous el (5 cells2 = 5q x 2rows x 16cri,
            # 320B); 3 pair-taps/point, 1 idx/partition/call -> 384 calls
            tab_flat = T_dram[:].rearrange("r (q e) -> (r q) e", e=W2)
            all_gathers = []
            for t in range(NTILE):
                # W[g, r2, b2, a] = wy[g, 2*b2 + r2] * wx[g, a]
                w240 = w36p.tile([128, GRP * 30], F32, tag="w36")
                for r2 in range(2):
                    ow = bass.AP(
                        w240[:].tensor, w240[:].offset + r2 * 15,
                        [w240[:].ap[0], [30, GRP], [5, 3], [1, 5]],
                    )
                    wyv = bass.AP(
                        acc[:].tensor,
                        acc[:].offset + 128 * JT + t * GRP * 6 + r2,
                        [acc[:].ap[0], [6, GRP], [2, 3], [0, 5]],
                    )
                    wxv = bass.AP(
                        acc[:].tensor, acc[:].offset + t * GRP * JT,
                        [acc[:].ap[0], [JT, GRP], [0, 3], [1, 5]],
                    )
                    nc.vector.tensor_tensor(out=ow, in0=wyv, in1=wxv, op=OP.mult)
                patch = patchp.tile([128, GRP * 3 * JT * W2], BF16, tag="patch")
                for g in range(GRP):
                    for b in range(3):
                        col = (t * GRP + g) * 3 + b
                        gi_ = nc.gpsimd.indirect_dma_start(
                            out=patch[:, (g * 3 + b) * JT * W2:
                                      (g * 3 + b + 1) * JT * W2],
                            out_offset=None,
                            in_=tab_flat,
                            in_offset=bass.IndirectOffsetOnAxis(
                                ap=idx32[:, col:col + 1], axis=0
                            ),
                        )
                        all_gathers.append(gi_)
                # WP[p, (g, cr, (r2, b2, a))] = patch[p, (g, b2, a, r2, cr)] * W
                wp = wpp.tile([128, GRP * 30 * CELL], BF16, tag="wpt")
                for r2 in range(2):
                    pv = bass.AP(
                        patch[:].tensor, patch[:].offset + r2 * CELL,
                        [patch[:].ap[0],
                         [3 * JT * W2, GRP], [1, CELL], [W2, 15]],
                    )
                    wv = bass.AP(
                        w240[:].tensor, w240[:].offset + r2 * 15,
                        [w240[:].ap[0], [30, GRP], [0, CELL], [1, 15]],
                    )
                    ov = bass.AP(
                        wp[:].tensor, wp[:].offset + r2 * 15,
                        [wp[:].ap[0],
                         [30 * CELL, GRP], [30, CELL], [1, 15]],
                    )
                    nc.vector.tensor_tensor(out=ov, in0=pv, in1=wv, op=OP.mult)
                # reduce innermost 30 -> res[:, t*128 + g*16 + cr]
                rv = bass.AP(
                    res[:].tensor, res[:].offset + t * 128,
                    [res[:].ap[0], [16, GRP], [1, CELL]],
                )
                wp3 = wp[:].rearrange("p (g cr ba) -> p g cr ba", cr=CELL, ba=30)
                nc.vector.tensor_reduce(out=rv, in_=wp3, axis=AX.X, op=OP.add)

            # explicit RAW edges: gathers after table stores
            for gi_ in all_gathers:
                for si in t_stores:
                    tile.add_dep_helper(gi_.ins, si.ins, reason="T RAW")

            # ======== sqrt(w) scale + store ========
            nc.vector.tensor_mul(res[:], res[:], wsq[:])
            nc.sync.dma_start(out=y_out[:], in_=res[:])

            if debug:
                dbg_outs = {
                    "kgo": kg, "acco": acc, "idxo": idx32, "flo": fl, "rro": rr,
                }
                for nm, t_ in dbg_outs.items():
                    o = nc.dram_tensor(nm, list(t_[:].shape), t_[:].dtype,
                                       kind="ExternalOutput")
                    nc.sync.dma_start(out=o[:], in_=t_[:])
                o = nc.dram_tensor("t0o", [PAD, TW], BF16, kind="ExternalOutput")
                di = nc.sync.dma_start(out=o[:], in_=T_dram[:])
                for si in t_stores:
                    tile.add_dep_helper(di.ins, si.ins, reason="T dump RAW")

    nc.compile()
    return nc


_NC_CACHE = None


def _get_nc():
    global _NC_CACHE
    if _NC_CACHE is None:
        _NC_CACHE = build_bass()
    return _NC_CACHE


# ---------------------------------------------------------------- host glue
def _shuffle_w(w_t):
    # w[c, ri, K] -> [p, (t, g, c, ri)] with K = t*1024 + g*128 + p
    v = w_t.reshape(NC, 2, NTILE, GRP, 128)
    return np.ascontiguousarray(v.transpose(4, 2, 3, 0, 1).reshape(128, NTILE * 128))


def _unshuffle_y(yr):
    # [p, (t, g, c, ri)] -> y[c, ri, K]
    v = yr.reshape(128, NTILE, GRP, NC, 2)
    return np.ascontiguousarray(v.transpose(3, 4, 1, 2, 0).reshape(NC, 2, K))


def make_in_maps(x, k, coil_sensitivities, w):
    in_maps = []
    coil0 = np.ascontiguousarray(coil_sensitivities[0], dtype=np.float32)
    for t in range(NT):
        in_maps.append({
            "x": np.ascontiguousarray(x[t], dtype=np.float32),
            "kk": np.ascontiguousarray(k[t], dtype=np.float32),
            "coil": coil0,
            "wr": _shuffle_w(np.asarray(w[t], dtype=np.float32)),
            "art": _ART, "ait": _AIT, "aitn": _AITN,
        })
    return in_maps


def run(x, k, coil_sensitivities, w, trace=False, **spmd_kwargs):
    nc = _get_nc()
    in_maps = make_in_maps(x, k, coil_sensitivities, w)
    r = run_bass_kernel_spmd(nc, in_maps, list(range(NT)), trace=trace, **spmd_kwargs)
    y = np.stack([_unshuffle_y(r.results[t]["yr"]) for t in range(NT)], axis=0)
    return y.astype(np.float32), r


def kernel(x, k, coil_sensitivities, w):
    y, _ = run(x, k, coil_sensitivities, w, trace=False)
    return y


# revision 18
# speedup vs baseline: 1.8531x; 1.0775x over previous
"""Trainium2 Bass kernel for nn_RadialModel (forward NUFFT, radial MRI).

Per-core (1 frame, all 8 coils):
  1. coil multiply (DVE, bf16 out)       cimage = (xr+ixi)*(cr+ici)
  2. DFT via PE bf16 matmuls (two stages): G[v,u] = A @ (M^T @ A^T) with
     apodization + fftshift phases folded into the constant A matrices
  3. store grid to a DRAM table (bf16), coil-interleaved cells
     [p=v_pad(517), q=u_pad(517), cri(16)] with 2/3-cell wraparound halo
  4. Kaiser-Bessel interpolation with round-to-nearest centering so the
     fractional offset is in [-0.5, 0.5]: a 5x5 tap window then captures
     every tap with weight > 2.6e-3 (the 6th tap of the reference's 6x6
     always has |U| >= 2.5 there).  Indirect-DMA gathers: one index per
     partition per call is a HW limit of the SWDGE indirect path (~1.25us
     fixed cost each); each index fetches a contiguous 5-cell x 16-cri
     160B run -> 640 calls, the dominant cost of the kernel.
  5. weighted reduce on DVE, sqrt(w) scale + store

Sharding: one frame (nt) per NeuronCore, 8 cores. Host does only
shard/reshape/unshuffle; all math on device.
"""
import math
import numpy as np

import concourse.bass as bass
import concourse.bacc as bacc
import concourse.mybir as mybir
import concourse.tile as tile
from concourse.bass_utils import run_bass_kernel_spmd
from concourse.masks import make_identity

F32 = mybir.dt.float32
I32 = mybir.dt.int32
AX = mybir.AxisListType
OP = mybir.AluOpType

IM = 256
G = 512
J = 6
JT = 5             # taps actually gathered per dim: the offs=-3 tap always
                   # has |U| >= 2.5 where the KB weight is <= 2.6e-3 -> drop
ALPHA = 2.34 * J
TWO_PI = 2.0 * np.pi
PAD = 517          # 512 + 2 left halo + 3 right halo
NT, NC, K = 8, 8, 16384
CELL = NC * 2      # floats per (p,q) cell = 16 (all coils interleaved)
TW = PAD * CELL    # table row width in elements = 8272
NTILE = 16         # point tiles of 1024 points (8 groups x 128 partitions)
GRP = 8            # groups per tile
DEG = 8            # KB weight polynomial degree (in t); abs err ~8.5e-6
NCELLS = PAD * PAD # flat cell count of the table


# ---------------------------------------------------------------- host consts
def _host_consts():
    # apodization correction 1/FT(kb)
    f = (np.arange(IM) - IM // 2) / G
    z = (np.pi * J * f) ** 2 - ALPHA ** 2
    s = np.sqrt(np.abs(z))
    val = np.where(z < 0, np.sinh(s) / np.maximum(s, 1e-12), np.sinc(s / np.pi))
    ftkb = (J / np.i0(ALPHA)) * val
    scal = 1.0 / ftkb
    # A[u, x'] = e^{i pi u/2 - 2 pi i u x'/G} * scal[x'] / sqrt(G)
    u = np.arange(G)[:, None].astype(np.float64)
    xp = np.arange(IM)[None, :].astype(np.float64)
    A = np.exp(1j * np.pi * u / 2 - 2j * np.pi * u * xp / G) * scal[None, :] / np.sqrt(G)
    art = np.ascontiguousarray(A.T.real, dtype=np.float32)   # [256, 512]
    ait = np.ascontiguousarray(A.T.imag, dtype=np.float32)
    aitn = np.ascontiguousarray(-A.T.imag, dtype=np.float32)
    # polynomial fit of w(t) = i0(ALPHA*sqrt(t))/i0(ALPHA) on t in [0,1]
    n = 512
    x = (1 - np.cos(np.pi * (np.arange(n) + 0.5) / n)) / 2
    w = np.i0(ALPHA * np.sqrt(x)) / np.i0(ALPHA)
    V = np.vander(x, DEG + 1, increasing=True)
    c, *_ = np.linalg.lstsq(V, w, rcond=None)
    return art, ait, aitn, c.astype(np.float64)


_ART, _AIT, _AITN, _CHEB = _host_consts()


# ---------------------------------------------------------------- bass build
def build_bass(debug=False):
    nc = bacc.Bacc()

    x_in = nc.declare_dram_parameter("x", [2, IM, IM], F32, isOutput=False)
    k_in = nc.declare_dram_parameter("kk", [2, K], F32, isOutput=False)
    c_in = nc.declare_dram_parameter("coil", [NC, 2, IM, IM], F32, isOutput=False)
    w_in = nc.declare_dram_parameter("wr", [128, NTILE * 128], F32, isOutput=False)
    art_in = nc.declare_dram_parameter("art", [IM, G], F32, isOutput=False)
    ait_in = nc.declare_dram_parameter("ait", [IM, G], F32, isOutput=False)
    aitn_in = nc.declare_dram_parameter("aitn", [IM, G], F32, isOutput=False)
    y_out = nc.declare_dram_parameter("yr", [128, NTILE * 128], F32, isOutput=True)

    BF16 = mybir.dt.bfloat16
    W2 = 4 * CELL          # quad-row cell: 4 v-rows x 16 cri = 64 el
    TW2 = PAD * W2         # table Q-row width = 33088 el
    PROWS = 130            # quads of table rows 0..519 (R = m + 4)
    T_dram = nc.dram_tensor("T0", [PROWS, TW2], BF16)

    CH = _CHEB
    with tile.TileContext(nc) as tc:
        with (
            tc.tile_pool(name="const", bufs=1) as constp,
            tc.tile_pool(name="work", bufs=1) as workp,
            tc.tile_pool(name="ctile", bufs=2) as coilp,
            tc.tile_pool(name="mtile", bufs=4) as mp,
            tc.tile_pool(name="bt", bufs=8) as btp,
            tc.tile_pool(name="stg", bufs=1) as stgp,
            tc.tile_pool(name="patch", bufs=2) as patchp,
            tc.tile_pool(name="w36", bufs=2) as w36p,
            tc.tile_pool(name="wp", bufs=2) as wpp,
            tc.tile_pool(name="ps1", bufs=4, space="PSUM") as ps1,
            tc.tile_pool(name="ps2", bufs=4, space="PSUM") as ps2,
        ):
            # ---------------- constants ----------------
            ident = constp.tile([128, 128], F32, tag="ident")
            make_identity(nc, ident[:])
            # A matrices: DMA f32, convert once to bf16 for PE
            art = []
            for name, src in (("art", art_in), ("ait", ait_in), ("aitn", aitn_in)):
                ts_ = []
                for xt in range(2):
                    tf = constp.tile([128, G], F32, tag=f"{name}f{xt}")
                    nc.sync.dma_start(out=tf[:], in_=src[xt * 128:(xt + 1) * 128, :])
                    tb = constp.tile([128, G], BF16, tag=f"{name}b{xt}")
                    nc.scalar.copy(out=tb[:], in_=tf[:])
                    ts_.append(tb)
                art.append(ts_)
            artT, aitT, aitnT = art

            offs = constp.tile([128, JT], F32, tag="offs")
            for a in range(JT):
                nc.vector.memset(offs[:, a:a + 1], float(3 - (a + 1)))
            ylat = constp.tile([128, 8], F32, tag="ylat")
            for a in range(8):
                nc.vector.memset(ylat[:, a:a + 1], float(-a))

            # ---------------- k -> [p, c] transpose ----------------
            kg = workp.tile([128, 256], F32, tag="kg")  # [p, (d, c)]
            for d in range(2):
                kt_in = workp.tile([128, 128], F32, tag="ktin")
                nc.sync.dma_start(
                    out=kt_in[:], in_=k_in[d].rearrange("(c p) -> c p", p=128)
                )
                ktp = ps2.tile([128, 128], F32, tag="psb")
                nc.tensor.transpose(ktp[:], kt_in[:], ident[:])
                nc.scalar.copy(out=kg[:, d * 128:(d + 1) * 128], in_=ktp[:])

            # ---------------- w load + sqrt ----------------
            wsq = workp.tile([128, NTILE * 128], F32, tag="wsq")
            nc.sync.dma_start(out=wsq[:], in_=w_in[:])
            nc.scalar.activation(
                out=wsq[:], in_=wsq[:],
                func=mybir.ActivationFunctionType.Sqrt,
            )

            # ---------------- index & weight math (DVE) ----------------
            # gx = om*(G/2pi); gx += 512 if gx < 0  -> [0, 512)
            gx0 = workp.tile([128, 256], F32, tag="gx0")
            nc.vector.tensor_scalar_mul(gx0[:], kg[:], float(G / TWO_PI))
            msk = workp.tile([128, 256], F32, tag="msk")
            nc.vector.tensor_scalar(
                out=msk[:], in0=gx0[:], scalar1=0.0, scalar2=None, op0=OP.is_lt
            )
            gxy = workp.tile([128, 256], F32, tag="gxy")
            nc.vector.scalar_tensor_tensor(
                out=gxy[:], in0=msk[:], scalar=float(G), in1=gx0[:],
                op0=OP.mult, op1=OP.add,
            )
            # gm3 = gxy - 3 ; f = rne(gm3 - 0.498) via 2^23 trick ; r = gm3 - f
            gm3 = workp.tile([128, 256], F32, tag="gm3")
            nc.vector.tensor_scalar(
                out=gm3[:], in0=gxy[:], scalar1=3.0, scalar2=None, op0=OP.subtract
            )
            fl = workp.tile([128, 256], F32, tag="fl")
            nc.vector.tensor_scalar(
                out=fl[:], in0=gm3[:],
                scalar1=0.0, scalar2=12582912.0,
                op0=OP.add, op1=OP.add,
            )
            nc.vector.tensor_scalar(
                out=fl[:], in0=fl[:], scalar1=12582912.0, scalar2=None,
                op0=OP.subtract,
            )
            rr = workp.tile([128, 256], F32, tag="rr")
            nc.vector.tensor_sub(rr[:], gm3[:], fl[:])

            # Q0 = floor((fl_y + 5)/4): h = fl*0.25 + 1.25, rne(h - 0.498)
            fp = workp.tile([128, 128], F32, tag="fp")
            nc.vector.tensor_scalar(
                out=fp[:], in0=fl[:, 128:256], scalar1=0.25, scalar2=1.25,
                op0=OP.mult, op1=OP.add,
            )
            nc.vector.tensor_scalar(
                out=fp[:], in0=fp[:],
                scalar1=-0.498046875, scalar2=12582912.0,
                op0=OP.add, op1=OP.add,
            )
            nc.vector.tensor_scalar(
                out=fp[:], in0=fp[:], scalar1=12582912.0, scalar2=None,
                op0=OP.subtract,
            )
            # sY = gy + 4 - 4*Q0 = gm3_y + 7 - 4*fp; y-tap j weight
            # arg = sY - j for gathered rows 4*Q0 + j, j = 0..7 (R = m + 4)
            sY = workp.tile([128, 128], F32, tag="sY")
            nc.vector.scalar_tensor_tensor(
                out=sY[:], in0=fp[:], scalar=-4.0, in1=gm3[:, 128:256],
                op0=OP.mult, op1=OP.add,
            )
            nc.vector.tensor_scalar_add(sY[:], sY[:], 7.0)

            # tap weight args: x: rr_x + offs (5); y: sY - j (6)
            NXC = 128 * JT
            ut = workp.tile([128, NXC + 128 * 8], F32, tag="ut")
            utx3 = ut[:, 0:NXC].rearrange("p (c a) -> p c a", a=JT)
            nc.vector.tensor_tensor(
                out=utx3,
                in0=rr[:, 0:128].unsqueeze(2).broadcast_to([128, 128, JT]),
                in1=offs[:].unsqueeze(1).broadcast_to([128, 128, JT]),
                op=OP.add,
            )
            uty3 = ut[:, NXC:].rearrange("p (c j) -> p c j", j=8)
            nc.vector.tensor_tensor(
                out=uty3,
                in0=sY[:].unsqueeze(2).broadcast_to([128, 128, 8]),
                in1=ylat[:].unsqueeze(1).broadcast_to([128, 128, 8]),
                op=OP.add,
            )
            # t = max(0, 1 - (U/3)^2)
            tsq = workp.tile([128, 128 * JT + 128 * 8], F32, tag="tsq")
            nc.vector.tensor_mul(tsq[:], ut[:], ut[:])
            nc.vector.tensor_scalar(
                out=tsq[:], in0=tsq[:], scalar1=float(-1.0 / 9.0), scalar2=1.0,
                op0=OP.mult, op1=OP.add,
            )
            nc.vector.tensor_scalar_max(tsq[:], tsq[:], 0.0)
            # Horner in t
            acc = workp.tile([128, 128 * JT + 128 * 8], F32, tag="acc")
            nc.vector.tensor_scalar(
                out=acc[:], in0=tsq[:], scalar1=float(CH[DEG]),
                scalar2=float(CH[DEG - 1]), op0=OP.mult, op1=OP.add,
            )
            for d in range(DEG - 2, -1, -1):
                nc.vector.tensor_mul(acc[:], acc[:], tsq[:])
                nc.vector.tensor_scalar_add(acc[:], acc[:], float(CH[d]))
            # acc = W_all [p, (d, c, a)]: d=0 -> wx taps, d=1 -> wy taps

            # gather cell4 indices: (Q0 + b2)*517 + 3 + fx, b2 = 0..1
            cbt = constp.tile([128, 2], F32, tag="cbt")
            for a in range(2):
                nc.vector.memset(cbt[:, a:a + 1], float(a * PAD + 3))
            fy517 = workp.tile([128, 128], F32, tag="fy517")
            nc.vector.tensor_scalar_mul(fy517[:], fp[:], float(PAD))
            idxf = workp.tile([128, 128 * 2], F32, tag="idxf")
            idxf3 = idxf[:].rearrange("p (c b) -> p c b", b=2)
            nc.vector.tensor_tensor(
                out=idxf3,
                in0=fy517[:].unsqueeze(2).broadcast_to([128, 128, 2]),
                in1=cbt[:].unsqueeze(1).broadcast_to([128, 128, 2]),
                op=OP.add,
            )
            nc.vector.tensor_tensor(
                out=idxf3,
                in0=idxf3,
                in1=fl[:, 0:128].unsqueeze(2).broadcast_to([128, 128, 2]),
                op=OP.add,
            )
            idx32 = workp.tile([128, 128 * 2], I32, tag="idx32")
            nc.vector.tensor_copy(out=idx32[:], in_=idxf[:])

            # ---------------- res buffer ----------------
            res = workp.tile([128, NTILE * 128], F32, tag="res")

            # x image tiles (persist across all coils)
            xts = []
            for xt in range(2):
                xt_t = workp.tile([128, 2 * IM], F32, tag=f"xt{xt}")
                nc.sync.dma_start(
                    out=xt_t[:],
                    in_=x_in[:, xt * 128:(xt + 1) * 128, :]
                    .rearrange("ri x y -> x ri y"),
                )
                xts.append(xt_t)

            # 1 persistent bf16 staging; partition p holds the v-row quad
            # (4p .. 4p+3), cells4 laid out (q, r4, e)
            stg0 = stgp.tile([128, G * W2], BF16, tag="stg0")

            for c in range(NC):
                # ---- coil multiply (bf16 out for PE) ----
                mt = []
                for xt in range(2):
                    ct = coilp.tile([128, 2 * IM], F32, tag="ct")
                    nc.sync.dma_start(
                        out=ct[:],
                        in_=c_in[c, :, xt * 128:(xt + 1) * 128, :]
                        .rearrange("ri x y -> x ri y"),
                    )
                    xt_t = xts[xt]
                    m = mp.tile([128, 2 * IM], BF16, tag="m")
                    xr, xi = xt_t[:, 0:IM], xt_t[:, IM:2 * IM]
                    cr, ci = ct[:, 0:IM], ct[:, IM:2 * IM]
                    mr, mi = m[:, 0:IM], m[:, IM:2 * IM]
                    t1 = mp.tile([128, IM], F32, tag="cm1")
                    t2 = mp.tile([128, IM], F32, tag="cm2")
                    nc.vector.tensor_mul(t1[:], xr, cr)
                    nc.vector.tensor_mul(t2[:], xi, ci)
                    nc.vector.tensor_sub(mr, t1[:], t2[:])
                    nc.vector.tensor_mul(t1[:], xr, ci)
                    nc.vector.tensor_mul(t2[:], xi, cr)
                    nc.vector.tensor_add(mi, t1[:], t2[:])
                    mt.append(m)
                # ---- stage 1: BT[y, u] per (ri, Yt) ----
                bt = {}
                for yt in range(2):
                    pr = ps1.tile([128, G], F32, tag="psa")
                    pi = ps1.tile([128, G], F32, tag="psa")
                    for xt in range(2):
                        mrb = mt[xt][:, yt * 128:yt * 128 + 128]
                        mib = mt[xt][:, IM + yt * 128:IM + yt * 128 + 128]
                        st = xt == 0
                        sp = xt == 1
                        nc.tensor.matmul(pr[:], mrb, artT[xt][:], start=st, stop=False)
                        nc.tensor.matmul(pi[:], mrb, aitT[xt][:], start=st, stop=False)
                        nc.tensor.matmul(pr[:], mib, aitnT[xt][:], start=False, stop=sp)
                        nc.tensor.matmul(pi[:], mib, artT[xt][:], start=False, stop=sp)
                    btr = btp.tile([128, G], BF16, tag="bt")
                    bti = btp.tile([128, G], BF16, tag="bt")
                    nc.scalar.copy(out=btr[:], in_=pr[:])
                    nc.scalar.copy(out=bti[:], in_=pi[:])
                    bt[(0, yt)] = btr
                    bt[(1, yt)] = bti
                # ---- stage 2: G[v, u] with v = 4p + r4 via stride-4 A
                # column slices; drain into quad-row staging ----
                for r2 in range(4):
                    stg3 = stg0[:].rearrange("p (q w) -> p q w", w=W2)
                    gr = ps2.tile([128, G], F32, tag="psb")
                    gi = ps2.tile([128, G], F32, tag="psb")
                    for yt in range(2):
                        a0 = artT[yt][:]
                        av = bass.AP(a0.tensor, a0.offset + r2,
                                     [a0.ap[0], [4, 128]])
                        i0_ = aitT[yt][:]
                        aiv = bass.AP(i0_.tensor, i0_.offset + r2,
                                      [i0_.ap[0], [4, 128]])
                        n0 = aitnT[yt][:]
                        ainv = bass.AP(n0.tensor, n0.offset + r2,
                                       [n0.ap[0], [4, 128]])
                        btr = bt[(0, yt)]
                        bti = bt[(1, yt)]
                        st = yt == 0
                        sp = yt == 1
                        nc.tensor.matmul(gr[:], av, btr[:], start=st, stop=False)
                        nc.tensor.matmul(gi[:], aiv, btr[:], start=st, stop=False)
                        nc.tensor.matmul(gr[:], ainv, bti[:], start=False, stop=sp)
                        nc.tensor.matmul(gi[:], av, bti[:], start=False, stop=sp)
                    c2 = r2 * CELL + 2 * c
                    # split strided drains across Scalar and Vector engines
                    if c % 2 == 0:
                        nc.scalar.copy(out=stg3[:, :, c2:c2 + 1], in_=gr[:].unsqueeze(2))
                        nc.vector.tensor_copy(out=stg3[:, :, c2 + 1:c2 + 2], in_=gi[:].unsqueeze(2))
                    else:
                        nc.vector.tensor_copy(out=stg3[:, :, c2:c2 + 1], in_=gr[:].unsqueeze(2))
                        nc.scalar.copy(out=stg3[:, :, c2 + 1:c2 + 2], in_=gi[:].unsqueeze(2))

            # ---- table stores: main (Q 1..128) + q halos, then halo
            # quads Q0 (v 508..511 <- stg[127]) and Q129 (v 0..3 <- stg[0])
            t_stores = []
            Th = T_dram
            t_stores.append(nc.sync.dma_start(
                out=Th[1:129, 2 * W2:2 * W2 + G * W2], in_=stg0[:]
            ))
            t_stores.append(nc.sync.dma_start(
                out=Th[1:129, 514 * W2:517 * W2], in_=stg0[:, 0:3 * W2],
            ))
            t_stores.append(nc.sync.dma_start(
                out=Th[1:129, 0:2 * W2], in_=stg0[:, 510 * W2:512 * W2],
            ))
            for dst, psrc in ((0, 127), (129, 0)):
                t_stores += [
                    nc.sync.dma_start(
                        out=Th[dst:dst + 1, 2 * W2:2 * W2 + G * W2],
                        in_=stg0[psrc:psrc + 1, :],
                    ),
                    nc.sync.dma_start(
                        out=Th[dst:dst + 1, 514 * W2:517 * W2],
                        in_=stg0[psrc:psrc + 1, 0:3 * W2],
                    ),
                    nc.sync.dma_start(
                        out=Th[dst:dst + 1, 0:2 * W2],
                        in_=stg0[psrc:psrc + 1, 510 * W2:512 * W2],
                    ),
                ]

            # ======== gather + combine ========
            # per index: 320 contiguous el (5 cells4 = 5q x 4rows x 16cri,
            # 640B); 2 quad-taps/point, 1 idx/partition/call -> 256 calls
            tab_flat = T_dram[:].rearrange("r (q e) -> (r q) e", e=W2)
            all_gathers = []
            for t in range(NTILE):
                # W[g, r4, b2, a] = wy[g, 4*b2 + r4] * wx[g, a]
                w240 = w36p.tile([128, GRP * 40], F32, tag="w36")
                for r2 in range(4):
                    ow = bass.AP(
                        w240[:].tensor, w240[:].offset + r2 * 10,
                        [w240[:].ap[0], [40, GRP], [5, 2], [1, 5]],
                    )
                    wyv = bass.AP(
                        acc[:].tensor,
                        acc[:].offset + 128 * JT + t * GRP * 8 + r2,
                        [acc[:].ap[0], [8, GRP], [4, 2], [0, 5]],
                    )
                    wxv = bass.AP(
                        acc[:].tensor, acc[:].offset + t * GRP * JT,
                        [acc[:].ap[0], [JT, GRP], [0, 2], [1, 5]],
                    )
                    nc.vector.tensor_tensor(out=ow, in0=wyv, in1=wxv, op=OP.mult)
                patch = patchp.tile([128, GRP * 2 * JT * W2], BF16, tag="patch")
                for g in range(GRP):
                    for b in range(2):
                        col = (t * GRP + g) * 2 + b
                        gi_ = nc.gpsimd.indirect_dma_start(
                            out=patch[:, (g * 2 + b) * JT * W2:
                                      (g * 2 + b + 1) * JT * W2],
                            out_offset=None,
                            in_=tab_flat,
                            in_offset=bass.IndirectOffsetOnAxis(
                                ap=idx32[:, col:col + 1], axis=0
                            ),
                        )
                        all_gathers.append(gi_)
                # WP[p, (g, cr, (r4, b2, a))] = patch[p, (g, b2, a, r4, cr)] * W
                wp = wpp.tile([128, GRP * 40 * CELL], BF16, tag="wpt")
                for r2 in range(4):
                    pv = bass.AP(
                        patch[:].tensor, patch[:].offset + r2 * CELL,
                        [patch[:].ap[0],
                         [2 * JT * W2, GRP], [1, CELL], [W2, 10]],
                    )
                    wv = bass.AP(
                        w240[:].tensor, w240[:].offset + r2 * 10,
                        [w240[:].ap[0], [40, GRP], [0, CELL], [1, 10]],
                    )
                    ov = bass.AP(
                        wp[:].tensor, wp[:].offset + r2 * 10,
                        [wp[:].ap[0],
                         [40 * CELL, GRP], [40, CELL], [1, 10]],
                    )
                    nc.vector.tensor_tensor(out=ov, in0=pv, in1=wv, op=OP.mult)
                # reduce innermost 40 -> res[:, t*128 + g*16 + cr]
                rv = bass.AP(
                    res[:].tensor, res[:].offset + t * 128,
                    [res[:].ap[0], [16, GRP], [1, CELL]],
                )
                wp3 = wp[:].rearrange("p (g cr ba) -> p g cr ba", cr=CELL, ba=40)
                nc.vector.tensor_reduce(out=rv, in_=wp3, axis=AX.X, op=OP.add)

            # explicit RAW edges: gathers after table stores
            for gi_ in all_gathers:
                for si in t_stores:
                    tile.add_dep_helper(gi_.ins, si.ins, reason="T RAW")

            # ======== sqrt(w) scale + store ========
            nc.vector.tensor_mul(res[:], res[:], wsq[:])
            nc.sync.dma_start(out=y_out[:], in_=res[:])

            if debug:
                dbg_outs = {
                    "kgo": kg, "acco": acc, "idxo": idx32, "flo": fl, "rro": rr,
                }
                for nm, t_ in dbg_outs.items():
                    o = nc.dram_tensor(nm, list(t_[:].shape), t_[:].dtype,
                                       kind="ExternalOutput")
                    nc.sync.dma_start(out=o[:], in_=t_[:])
                o = nc.dram_tensor("t0o", [PAD, TW], BF16, kind="ExternalOutput")
                di = nc.sync.dma_start(out=o[:], in_=T_dram[:])
                for si in t_stores:
                    tile.add_dep_helper(di.ins, si.ins, reason="T dump RAW")

    nc.compile()
    return nc


_NC_CACHE = None


def _get_nc():
    global _NC_CACHE
    if _NC_CACHE is None:
        _NC_CACHE = build_bass()
    return _NC_CACHE


# ---------------------------------------------------------------- host glue
def _shuffle_w(w_t):
    # w[c, ri, K] -> [p, (t, g, c, ri)] with K = t*1024 + g*128 + p
    v = w_t.reshape(NC, 2, NTILE, GRP, 128)
    return np.ascontiguousarray(v.transpose(4, 2, 3, 0, 1).reshape(128, NTILE * 128))


def _unshuffle_y(yr):
    # [p, (t, g, c, ri)] -> y[c, ri, K]
    v = yr.reshape(128, NTILE, GRP, NC, 2)
    return np.ascontiguousarray(v.transpose(3, 4, 1, 2, 0).reshape(NC, 2, K))


def make_in_maps(x, k, coil_sensitivities, w):
    in_maps = []
    coil0 = np.ascontiguousarray(coil_sensitivities[0], dtype=np.float32)
    for t in range(NT):
        in_maps.append({
            "x": np.ascontiguousarray(x[t], dtype=np.float32),
            "kk": np.ascontiguousarray(k[t], dtype=np.float32),
            "coil": coil0,
            "wr": _shuffle_w(np.asarray(w[t], dtype=np.float32)),
            "art": _ART, "ait": _AIT, "aitn": _AITN,
        })
    return in_maps


def run(x, k, coil_sensitivities, w, trace=False, **spmd_kwargs):
    nc = _get_nc()
    in_maps = make_in_maps(x, k, coil_sensitivities, w)
    r = run_bass_kernel_spmd(nc, in_maps, list(range(NT)), trace=trace, **spmd_kwargs)
    y = np.stack([_unshuffle_y(r.results[t]["yr"]) for t in range(NT)], axis=0)
    return y.astype(np.float32), r


def kernel(x, k, coil_sensitivities, w):
    y, _ = run(x, k, coil_sensitivities, w, trace=False)
    return y


# revision 20
# speedup vs baseline: 2.1936x; 1.1837x over previous
"""Trainium2 Bass kernel for nn_RadialModel (forward NUFFT, radial MRI).

Per-core (1 frame, all 8 coils):
  1. coil multiply (DVE, bf16 out)       cimage = (xr+ixi)*(cr+ici)
  2. DFT via PE bf16 matmuls (two stages): G[v,u] = A @ (M^T @ A^T) with
     apodization + fftshift phases folded into the constant A matrices
  3. store grid to a DRAM table (bf16), coil-interleaved cells
     [p=v_pad(517), q=u_pad(517), cri(16)] with 2/3-cell wraparound halo
  4. Kaiser-Bessel interpolation with round-to-nearest centering so the
     fractional offset is in [-0.5, 0.5]: a 5x5 tap window then captures
     every tap with weight > 2.6e-3 (the 6th tap of the reference's 6x6
     always has |U| >= 2.5 there).  The table packs FOUR v-rows per cell
     (quad-row cells, 64 el), so one gather index fetches 5 q-cells x 4
     rows x 16 cri = 640B and a point needs only TWO quad-row taps (the
     8-row window always covers the 5 needed rows; stray rows get ~4e-6
     weights from the clamped KB polynomial).  One index per partition
     per call is a HW limit of the SWDGE indirect path (~1.25us fixed
     cost each) -> 256 calls, the dominant cost of the kernel.
  5. per-r4 weighted multiply + 40-tap reduce on DVE, sqrt(w) scale + store

Sharding: one frame (nt) per NeuronCore, 8 cores. Host does only
shard/reshape/unshuffle; all math on device.
"""
import math
import numpy as np

import concourse.bass as bass
import concourse.bacc as bacc
import concourse.mybir as mybir
import concourse.tile as tile
from concourse.bass_utils import run_bass_kernel_spmd
from concourse.masks import make_identity

F32 = mybir.dt.float32
I32 = mybir.dt.int32
AX = mybir.AxisListType
OP = mybir.AluOpType

IM = 256
G = 512
J = 6
JT = 5             # live taps per dim (rne centering => the dropped 6th
                   # tap always has |U| >= 2.5, KB weight <= 2.6e-3)
ALPHA = 2.34 * J
TWO_PI = 2.0 * np.pi
PAD = 517          # 512 + 2 left halo + 3 right halo
NT, NC, K = 8, 8, 16384
CELL = NC * 2      # floats per (p,q) cell = 16 (all coils interleaved)
TW = PAD * CELL    # table row width in elements = 8272
NTILE = 16         # point tiles of 1024 points (8 groups x 128 partitions)
GRP = 8            # groups per tile
DEG = 8            # KB weight polynomial degree (in t); abs err ~8.5e-6
NCELLS = PAD * PAD # flat cell count of the table


# ---------------------------------------------------------------- host consts
def _host_consts():
    # apodization correction 1/FT(kb)
    f = (np.arange(IM) - IM // 2) / G
    z = (np.pi * J * f) ** 2 - ALPHA ** 2
    s = np.sqrt(np.abs(z))
    val = np.where(z < 0, np.sinh(s) / np.maximum(s, 1e-12), np.sinc(s / np.pi))
    ftkb = (J / np.i0(ALPHA)) * val
    scal = 1.0 / ftkb
    # A[u, x'] = e^{i pi u/2 - 2 pi i u x'/G} * scal[x'] / sqrt(G)
    u = np.arange(G)[:, None].astype(np.float64)
    xp = np.arange(IM)[None, :].astype(np.float64)
    A = np.exp(1j * np.pi * u / 2 - 2j * np.pi * u * xp / G) * scal[None, :] / np.sqrt(G)
    art = np.ascontiguousarray(A.T.real, dtype=np.float32)   # [256, 512]
    ait = np.ascontiguousarray(A.T.imag, dtype=np.float32)
    aitn = np.ascontiguousarray(-A.T.imag, dtype=np.float32)
    # polynomial fit of w(t) = i0(ALPHA*sqrt(t))/i0(ALPHA) on t in [0,1]
    n = 512
    x = (1 - np.cos(np.pi * (np.arange(n) + 0.5) / n)) / 2
    w = np.i0(ALPHA * np.sqrt(x)) / np.i0(ALPHA)
    V = np.vander(x, DEG + 1, increasing=True)
    c, *_ = np.linalg.lstsq(V, w, rcond=None)
    return art, ait, aitn, c.astype(np.float64)


_ART, _AIT, _AITN, _CHEB = _host_consts()


# ---------------------------------------------------------------- bass build
def build_bass(debug=False):
    nc = bacc.Bacc()

    x_in = nc.declare_dram_parameter("x", [2, IM, IM], F32, isOutput=False)
    k_in = nc.declare_dram_parameter("kk", [2, K], F32, isOutput=False)
    c_in = nc.declare_dram_parameter("coil", [NC, 2, IM, IM], F32, isOutput=False)
    w_in = nc.declare_dram_parameter("wr", [128, NTILE * 128], F32, isOutput=False)
    art_in = nc.declare_dram_parameter("art", [IM, G], F32, isOutput=False)
    ait_in = nc.declare_dram_parameter("ait", [IM, G], F32, isOutput=False)
    aitn_in = nc.declare_dram_parameter("aitn", [IM, G], F32, isOutput=False)
    y_out = nc.declare_dram_parameter("yr", [128, NTILE * 128], F32, isOutput=True)

    BF16 = mybir.dt.bfloat16
    W2 = 4 * CELL          # quad-row cell: 4 v-rows x 16 cri = 64 el
    TW2 = PAD * W2         # table Q-row width = 33088 el
    PROWS = 130            # quads of table rows 0..519 (R = m + 4)
    T_dram = nc.dram_tensor("T0", [PROWS, TW2], BF16)

    CH = _CHEB
    with tile.TileContext(nc) as tc:
        with (
            tc.tile_pool(name="const", bufs=1) as constp,
            tc.tile_pool(name="work", bufs=1) as workp,
            tc.tile_pool(name="ctile", bufs=2) as coilp,
            tc.tile_pool(name="mtile", bufs=4) as mp,
            tc.tile_pool(name="bt", bufs=8) as btp,
            tc.tile_pool(name="stg", bufs=1) as stgp,
            tc.tile_pool(name="patch", bufs=2) as patchp,
            tc.tile_pool(name="w36", bufs=2) as w36p,
            tc.tile_pool(name="wp", bufs=2) as wpp,
            tc.tile_pool(name="ps1", bufs=4, space="PSUM") as ps1,
            tc.tile_pool(name="ps2", bufs=4, space="PSUM") as ps2,
        ):
            # ---------------- constants ----------------
            ident = constp.tile([128, 128], F32, tag="ident")
            make_identity(nc, ident[:])
            # A matrices: DMA f32, convert once to bf16 for PE
            art = []
            for name, src in (("art", art_in), ("ait", ait_in), ("aitn", aitn_in)):
                ts_ = []
                for xt in range(2):
                    tf = constp.tile([128, G], F32, tag=f"{name}f{xt}")
                    nc.sync.dma_start(out=tf[:], in_=src[xt * 128:(xt + 1) * 128, :])
                    tb = constp.tile([128, G], BF16, tag=f"{name}b{xt}")
                    nc.scalar.copy(out=tb[:], in_=tf[:])
                    # stage-2 copy with v-columns regrouped (r4, p) so the
                    # stride-4 quad slices become contiguous weight loads
                    tp_ = constp.tile([128, G], BF16, tag=f"{name}p{xt}")
                    pin = bass.AP(
                        tf[:].tensor, tf[:].offset,
                        [tf[:].ap[0], [1, 4], [4, 128]],
                    )
                    nc.scalar.copy(out=tp_[:], in_=pin)
                    ts_.append((tb, tp_))
                art.append(ts_)
            artT = [a[0] for a in art[0]]
            aitT = [a[0] for a in art[1]]
            aitnT = [a[0] for a in art[2]]
            artP = [a[1] for a in art[0]]
            aitP = [a[1] for a in art[1]]
            aitnP = [a[1] for a in art[2]]

            offs = constp.tile([128, JT], F32, tag="offs")
            for a in range(JT):
                nc.vector.memset(offs[:, a:a + 1], float(3 - (a + 1)))
            ylat = constp.tile([128, 8], F32, tag="ylat")
            for a in range(8):
                nc.vector.memset(ylat[:, a:a + 1], float(-a))

            # ---------------- k -> [p, c] transpose ----------------
            kg = workp.tile([128, 256], F32, tag="kg")  # [p, (d, c)]
            for d in range(2):
                kt_in = workp.tile([128, 128], F32, tag="ktin")
                nc.sync.dma_start(
                    out=kt_in[:], in_=k_in[d].rearrange("(c p) -> c p", p=128)
                )
                ktp = ps2.tile([128, 128], F32, tag="psb")
                nc.tensor.transpose(ktp[:], kt_in[:], ident[:])
                nc.scalar.copy(out=kg[:, d * 128:(d + 1) * 128], in_=ktp[:])

            # ---------------- w load + sqrt ----------------
            wsq = workp.tile([128, NTILE * 128], F32, tag="wsq")
            nc.sync.dma_start(out=wsq[:], in_=w_in[:])
            nc.scalar.activation(
                out=wsq[:], in_=wsq[:],
                func=mybir.ActivationFunctionType.Sqrt,
            )

            # ---------------- index & weight math (DVE) ----------------
            # gx = om*(G/2pi); gx += 512 if gx < 0  -> [0, 512)
            gx0 = workp.tile([128, 256], F32, tag="gx0")
            nc.vector.tensor_scalar_mul(gx0[:], kg[:], float(G / TWO_PI))
            msk = workp.tile([128, 256], F32, tag="msk")
            nc.vector.tensor_scalar(
                out=msk[:], in0=gx0[:], scalar1=0.0, scalar2=None, op0=OP.is_lt
            )
            gxy = workp.tile([128, 256], F32, tag="gxy")
            nc.vector.scalar_tensor_tensor(
                out=gxy[:], in0=msk[:], scalar=float(G), in1=gx0[:],
                op0=OP.mult, op1=OP.add,
            )
            # gm3 = gxy - 3 ; f = rne(gm3 - 0.498) via 2^23 trick ; r = gm3 - f
            gm3 = workp.tile([128, 256], F32, tag="gm3")
            nc.vector.tensor_scalar(
                out=gm3[:], in0=gxy[:], scalar1=3.0, scalar2=None, op0=OP.subtract
            )
            fl = workp.tile([128, 256], F32, tag="fl")
            nc.vector.tensor_scalar(
                out=fl[:], in0=gm3[:],
                scalar1=0.0, scalar2=12582912.0,
                op0=OP.add, op1=OP.add,
            )
            nc.vector.tensor_scalar(
                out=fl[:], in0=fl[:], scalar1=12582912.0, scalar2=None,
                op0=OP.subtract,
            )
            rr = workp.tile([128, 256], F32, tag="rr")
            nc.vector.tensor_sub(rr[:], gm3[:], fl[:])

            # Q0 = floor((fl_y + 5)/4): h = fl*0.25 + 1.25, rne(h - 0.498)
            fp = workp.tile([128, 128], F32, tag="fp")
            nc.vector.tensor_scalar(
                out=fp[:], in0=fl[:, 128:256], scalar1=0.25, scalar2=1.25,
                op0=OP.mult, op1=OP.add,
            )
            nc.vector.tensor_scalar(
                out=fp[:], in0=fp[:],
                scalar1=-0.498046875, scalar2=12582912.0,
                op0=OP.add, op1=OP.add,
            )
            nc.vector.tensor_scalar(
                out=fp[:], in0=fp[:], scalar1=12582912.0, scalar2=None,
                op0=OP.subtract,
            )
            # sY = gy + 4 - 4*Q0 = gm3_y + 7 - 4*fp; y-tap j weight
            # arg = sY - j for gathered rows 4*Q0 + j, j = 0..7 (R = m + 4)
            sY = workp.tile([128, 128], F32, tag="sY")
            nc.vector.scalar_tensor_tensor(
                out=sY[:], in0=fp[:], scalar=-4.0, in1=gm3[:, 128:256],
                op0=OP.mult, op1=OP.add,
            )
            nc.vector.tensor_scalar_add(sY[:], sY[:], 7.0)

            # tap weight args: x: rr_x + offs (5); y: sY - j (6)
            NXC = 128 * JT
            ut = workp.tile([128, NXC + 128 * 8], F32, tag="ut")
            utx3 = ut[:, 0:NXC].rearrange("p (c a) -> p c a", a=JT)
            nc.vector.tensor_tensor(
                out=utx3,
                in0=rr[:, 0:128].unsqueeze(2).broadcast_to([128, 128, JT]),
                in1=offs[:].unsqueeze(1).broadcast_to([128, 128, JT]),
                op=OP.add,
            )
            uty3 = ut[:, NXC:].rearrange("p (c j) -> p c j", j=8)
            nc.vector.tensor_tensor(
                out=uty3,
                in0=sY[:].unsqueeze(2).broadcast_to([128, 128, 8]),
                in1=ylat[:].unsqueeze(1).broadcast_to([128, 128, 8]),
                op=OP.add,
            )
            # t = max(0, 1 - (U/3)^2)
            tsq = workp.tile([128, 128 * JT + 128 * 8], F32, tag="tsq")
            nc.vector.tensor_mul(tsq[:], ut[:], ut[:])
            nc.vector.tensor_scalar(
                out=tsq[:], in0=tsq[:], scalar1=float(-1.0 / 9.0), scalar2=1.0,
                op0=OP.mult, op1=OP.add,
            )
            nc.vector.tensor_scalar_max(tsq[:], tsq[:], 0.0)
            # Horner in t
            acc = workp.tile([128, 128 * JT + 128 * 8], F32, tag="acc")
            nc.vector.tensor_scalar(
                out=acc[:], in0=tsq[:], scalar1=float(CH[DEG]),
                scalar2=float(CH[DEG - 1]), op0=OP.mult, op1=OP.add,
            )
            for d in range(DEG - 2, -1, -1):
                nc.vector.tensor_mul(acc[:], acc[:], tsq[:])
                nc.vector.tensor_scalar_add(acc[:], acc[:], float(CH[d]))
            # acc = W_all [p, (d, c, a)]: d=0 -> wx taps, d=1 -> wy taps

            # gather cell4 indices: (Q0 + b2)*517 + 3 + fx, b2 = 0..1
            cbt = constp.tile([128, 2], F32, tag="cbt")
            for a in range(2):
                nc.vector.memset(cbt[:, a:a + 1], float(a * PAD + 3))
            fy517 = workp.tile([128, 128], F32, tag="fy517")
            nc.vector.tensor_scalar_mul(fy517[:], fp[:], float(PAD))
            idxf = workp.tile([128, 128 * 2], F32, tag="idxf")
            idxf3 = idxf[:].rearrange("p (c b) -> p c b", b=2)
            nc.vector.tensor_tensor(
                out=idxf3,
                in0=fy517[:].unsqueeze(2).broadcast_to([128, 128, 2]),
                in1=cbt[:].unsqueeze(1).broadcast_to([128, 128, 2]),
                op=OP.add,
            )
            nc.vector.tensor_tensor(
                out=idxf3,
                in0=idxf3,
                in1=fl[:, 0:128].unsqueeze(2).broadcast_to([128, 128, 2]),
                op=OP.add,
            )
            idx32 = workp.tile([128, 128 * 2], I32, tag="idx32")
            nc.vector.tensor_copy(out=idx32[:], in_=idxf[:])

            # ---------------- res buffer ----------------
            res = workp.tile([128, NTILE * 128], F32, tag="res")

            # x image tiles (persist across all coils)
            xts = []
            for xt in range(2):
                xt_t = workp.tile([128, 2 * IM], F32, tag=f"xt{xt}")
                nc.sync.dma_start(
                    out=xt_t[:],
                    in_=x_in[:, xt * 128:(xt + 1) * 128, :]
                    .rearrange("ri x y -> x ri y"),
                )
                xts.append(xt_t)

            # 1 persistent bf16 staging; partition p holds the v-row quad
            # (4p .. 4p+3), cells4 laid out (q, r4, e)
            stg0 = stgp.tile([128, G * W2], BF16, tag="stg0")

            for c in range(NC):
                # ---- coil multiply (bf16 out for PE) ----
                mt = []
                for xt in range(2):
                    ct = coilp.tile([128, 2 * IM], F32, tag="ct")
                    nc.sync.dma_start(
                        out=ct[:],
                        in_=c_in[c, :, xt * 128:(xt + 1) * 128, :]
                        .rearrange("ri x y -> x ri y"),
                    )
                    xt_t = xts[xt]
                    m = mp.tile([128, 2 * IM], BF16, tag="m")
                    xr, xi = xt_t[:, 0:IM], xt_t[:, IM:2 * IM]
                    cr, ci = ct[:, 0:IM], ct[:, IM:2 * IM]
                    mr, mi = m[:, 0:IM], m[:, IM:2 * IM]
                    t1 = mp.tile([128, IM], F32, tag="cm1")
                    t2 = mp.tile([128, IM], F32, tag="cm2")
                    nc.vector.tensor_mul(t1[:], xr, cr)
                    nc.vector.tensor_mul(t2[:], xi, ci)
                    nc.vector.tensor_sub(mr, t1[:], t2[:])
                    nc.vector.tensor_mul(t1[:], xr, ci)
                    nc.vector.tensor_mul(t2[:], xi, cr)
                    nc.vector.tensor_add(mi, t1[:], t2[:])
                    mt.append(m)
                # ---- stage 1: BT[y, u] per (ri, Yt) ----
                bt = {}
                for yt in range(2):
                    pr = ps1.tile([128, G], F32, tag="psa")
                    pi = ps1.tile([128, G], F32, tag="psa")
                    for xt in range(2):
                        mrb = mt[xt][:, yt * 128:yt * 128 + 128]
                        mib = mt[xt][:, IM + yt * 128:IM + yt * 128 + 128]
                        st = xt == 0
                        sp = xt == 1
                        nc.tensor.matmul(pr[:], mrb, artT[xt][:], start=st, stop=False)
                        nc.tensor.matmul(pi[:], mrb, aitT[xt][:], start=st, stop=False)
                        nc.tensor.matmul(pr[:], mib, aitnT[xt][:], start=False, stop=sp)
                        nc.tensor.matmul(pi[:], mib, artT[xt][:], start=False, stop=sp)
                    btr = btp.tile([128, G], BF16, tag="bt")
                    bti = btp.tile([128, G], BF16, tag="bt")
                    nc.scalar.copy(out=btr[:], in_=pr[:])
                    nc.scalar.copy(out=bti[:], in_=pi[:])
                    bt[(0, yt)] = btr
                    bt[(1, yt)] = bti
                # ---- stage 2: G[v, u] with v = 4p + r4 via stride-4 A
                # column slices; drain into quad-row staging ----
                for r2 in range(4):
                    stg3 = stg0[:].rearrange("p (q w) -> p q w", w=W2)
                    gr = ps2.tile([128, G], F32, tag="psb")
                    gi = ps2.tile([128, G], F32, tag="psb")
                    for yt in range(2):
                        av = artP[yt][:, r2 * 128:(r2 + 1) * 128]
                        aiv = aitP[yt][:, r2 * 128:(r2 + 1) * 128]
                        ainv = aitnP[yt][:, r2 * 128:(r2 + 1) * 128]
                        btr = bt[(0, yt)]
                        bti = bt[(1, yt)]
                        st = yt == 0
                        sp = yt == 1
                        nc.tensor.matmul(gr[:], av, btr[:], start=st, stop=False)
                        nc.tensor.matmul(gi[:], aiv, btr[:], start=st, stop=False)
                        nc.tensor.matmul(gr[:], ainv, bti[:], start=False, stop=sp)
                        nc.tensor.matmul(gi[:], av, bti[:], start=False, stop=sp)
                    c2 = r2 * CELL + 2 * c
                    # split strided drains across Scalar and Vector engines
                    if c % 2 == 0:
                        nc.scalar.copy(out=stg3[:, :, c2:c2 + 1], in_=gr[:].unsqueeze(2))
                        nc.vector.tensor_copy(out=stg3[:, :, c2 + 1:c2 + 2], in_=gi[:].unsqueeze(2))
                    else:
                        nc.vector.tensor_copy(out=stg3[:, :, c2:c2 + 1], in_=gr[:].unsqueeze(2))
                        nc.scalar.copy(out=stg3[:, :, c2 + 1:c2 + 2], in_=gi[:].unsqueeze(2))

            # ---- table stores: main (Q 1..128) + q halos, then halo
            # quads Q0 (v 508..511 <- stg[127]) and Q129 (v 0..3 <- stg[0])
            t_stores = []
            Th = T_dram
            t_stores.append(nc.sync.dma_start(
                out=Th[1:129, 2 * W2:2 * W2 + G * W2], in_=stg0[:]
            ))
            t_stores.append(nc.sync.dma_start(
                out=Th[1:129, 514 * W2:517 * W2], in_=stg0[:, 0:3 * W2],
            ))
            t_stores.append(nc.sync.dma_start(
                out=Th[1:129, 0:2 * W2], in_=stg0[:, 510 * W2:512 * W2],
            ))
            for dst, psrc in ((0, 127), (129, 0)):
                t_stores += [
                    nc.sync.dma_start(
                        out=Th[dst:dst + 1, 2 * W2:2 * W2 + G * W2],
                        in_=stg0[psrc:psrc + 1, :],
                    ),
                    nc.sync.dma_start(
                        out=Th[dst:dst + 1, 514 * W2:517 * W2],
                        in_=stg0[psrc:psrc + 1, 0:3 * W2],
                    ),
                    nc.sync.dma_start(
                        out=Th[dst:dst + 1, 0:2 * W2],
                        in_=stg0[psrc:psrc + 1, 510 * W2:512 * W2],
                    ),
                ]

            # ======== gather + combine ========
            # per index: 320 contiguous el (5 cells4 = 5q x 4rows x 16cri,
            # 640B); 2 quad-taps/point, 1 idx/partition/call -> 256 calls
            tab_flat = T_dram[:].rearrange("r (q e) -> (r q) e", e=W2)
            all_gathers = []
            for t in range(NTILE):
                # W[g, r4, b2, a] = wy[g, 4*b2 + r4] * wx[g, a]
                w240 = w36p.tile([128, GRP * 40], F32, tag="w36")
                for r2 in range(4):
                    ow = bass.AP(
                        w240[:].tensor, w240[:].offset + r2 * 10,
                        [w240[:].ap[0], [40, GRP], [5, 2], [1, 5]],
                    )
                    wyv = bass.AP(
                        acc[:].tensor,
                        acc[:].offset + 128 * JT + t * GRP * 8 + r2,
                        [acc[:].ap[0], [8, GRP], [4, 2], [0, 5]],
                    )
                    wxv = bass.AP(
                        acc[:].tensor, acc[:].offset + t * GRP * JT,
                        [acc[:].ap[0], [JT, GRP], [0, 2], [1, 5]],
                    )
                    nc.vector.tensor_tensor(out=ow, in0=wyv, in1=wxv, op=OP.mult)
                patch = patchp.tile([128, GRP * 2 * JT * W2], BF16, tag="patch")
                for g in range(GRP):
                    for b in range(2):
                        col = (t * GRP + g) * 2 + b
                        gi_ = nc.gpsimd.indirect_dma_start(
                            out=patch[:, (g * 2 + b) * JT * W2:
                                      (g * 2 + b + 1) * JT * W2],
                            out_offset=None,
                            in_=tab_flat,
                            in_offset=bass.IndirectOffsetOnAxis(
                                ap=idx32[:, col:col + 1], axis=0
                            ),
                        )
                        all_gathers.append(gi_)
                # WP[p, (g, cr, (r4, b2, a))] = patch[p, (g, b2, a, r4, cr)] * W
                wp = wpp.tile([128, GRP * 40 * CELL], BF16, tag="wpt")
                for r2 in range(4):
                    pv = bass.AP(
                        patch[:].tensor, patch[:].offset + r2 * CELL,
                        [patch[:].ap[0],
                         [2 * JT * W2, GRP], [1, CELL], [W2, 10]],
                    )
                    wv = bass.AP(
                        w240[:].tensor, w240[:].offset + r2 * 10,
                        [w240[:].ap[0], [40, GRP], [0, CELL], [1, 10]],
                    )
                    ov = bass.AP(
                        wp[:].tensor, wp[:].offset + r2 * 10,
                        [wp[:].ap[0],
                         [40 * CELL, GRP], [40, CELL], [1, 10]],
                    )
                    nc.vector.tensor_tensor(out=ov, in0=pv, in1=wv, op=OP.mult)
                # reduce innermost 40 -> res[:, t*128 + g*16 + cr]
                rv = bass.AP(
                    res[:].tensor, res[:].offset + t * 128,
                    [res[:].ap[0], [16, GRP], [1, CELL]],
                )
                wp3 = wp[:].rearrange("p (g cr ba) -> p g cr ba", cr=CELL, ba=40)
                nc.vector.tensor_reduce(out=rv, in_=wp3, axis=AX.X, op=OP.add)

            # explicit RAW edges: gathers after table stores
            for gi_ in all_gathers:
                for si in t_stores:
                    tile.add_dep_helper(gi_.ins, si.ins, reason="T RAW")

            # ======== sqrt(w) scale + store ========
            nc.vector.tensor_mul(res[:], res[:], wsq[:])
            nc.sync.dma_start(out=y_out[:], in_=res[:])

            if debug:
                dbg_outs = {
                    "kgo": kg, "acco": acc, "idxo": idx32, "flo": fl, "rro": rr,
                }
                for nm, t_ in dbg_outs.items():
                    o = nc.dram_tensor(nm, list(t_[:].shape), t_[:].dtype,
                                       kind="ExternalOutput")
                    nc.sync.dma_start(out=o[:], in_=t_[:])
                o = nc.dram_tensor("t0o", [PAD, TW], BF16, kind="ExternalOutput")
                di = nc.sync.dma_start(out=o[:], in_=T_dram[:])
                for si in t_stores:
                    tile.add_dep_helper(di.ins, si.ins, reason="T dump RAW")

    nc.compile()
    return nc


_NC_CACHE = None


def _get_nc():
    global _NC_CACHE
    if _NC_CACHE is None:
        _NC_CACHE = build_bass()
    return _NC_CACHE


# ---------------------------------------------------------------- host glue
def _shuffle_w(w_t):
    # w[c, ri, K] -> [p, (t, g, c, ri)] with K = t*1024 + g*128 + p
    v = w_t.reshape(NC, 2, NTILE, GRP, 128)
    return np.ascontiguousarray(v.transpose(4, 2, 3, 0, 1).reshape(128, NTILE * 128))


def _unshuffle_y(yr):
    # [p, (t, g, c, ri)] -> y[c, ri, K]
    v = yr.reshape(128, NTILE, GRP, NC, 2)
    return np.ascontiguousarray(v.transpose(3, 4, 1, 2, 0).reshape(NC, 2, K))


def make_in_maps(x, k, coil_sensitivities, w):
    in_maps = []
    coil0 = np.ascontiguousarray(coil_sensitivities[0], dtype=np.float32)
    for t in range(NT):
        in_maps.append({
            "x": np.ascontiguousarray(x[t], dtype=np.float32),
            "kk": np.ascontiguousarray(k[t], dtype=np.float32),
            "coil": coil0,
            "wr": _shuffle_w(np.asarray(w[t], dtype=np.float32)),
            "art": _ART, "ait": _AIT, "aitn": _AITN,
        })
    return in_maps


def run(x, k, coil_sensitivities, w, trace=False, **spmd_kwargs):
    nc = _get_nc()
    in_maps = make_in_maps(x, k, coil_sensitivities, w)
    r = run_bass_kernel_spmd(nc, in_maps, list(range(NT)), trace=trace, **spmd_kwargs)
    y = np.stack([_unshuffle_y(r.results[t]["yr"]) for t in range(NT)], axis=0)
    return y.astype(np.float32), r


def kernel(x, k, coil_sensitivities, w):
    y, _ = run(x, k, coil_sensitivities, w, trace=False)
    return y


# revision 24
# speedup vs baseline: 2.2319x; 1.0175x over previous
"""Trainium2 Bass kernel for nn_RadialModel (forward NUFFT, radial MRI).

Per-core (1 frame, all 8 coils):
  1. coil multiply (DVE, bf16 out)       cimage = (xr+ixi)*(cr+ici)
  2. DFT via PE bf16 matmuls (two stages): G[v,u] = A @ (M^T @ A^T) with
     apodization + fftshift phases folded into the constant A matrices
  3. store grid to a DRAM table (bf16), coil-interleaved cells
     [p=v_pad(517), q=u_pad(517), cri(16)] with 2/3-cell wraparound halo
  4. Kaiser-Bessel interpolation with round-to-nearest centering so the
     fractional offset is in [-0.5, 0.5]: a 5x5 tap window then captures
     every tap with weight > 2.6e-3 (the 6th tap of the reference's 6x6
     always has |U| >= 2.5 there).  The table packs FOUR v-rows per cell
     (quad-row cells, 64 el), so one gather index fetches 5 q-cells x 4
     rows x 16 cri = 640B and a point needs only TWO quad-row taps (the
     8-row window always covers the 5 needed rows; stray rows get ~4e-6
     weights from the clamped KB polynomial).  One index per partition
     per call is a HW limit of the SWDGE indirect path (~1.25us fixed
     cost each) -> 256 calls, the dominant cost of the kernel.
  5. per-r4 weighted multiply + 40-tap reduce on DVE, sqrt(w) scale + store

Sharding: one frame (nt) per NeuronCore, 8 cores. Host does only
shard/reshape/unshuffle; all math on device.
"""
import math
import numpy as np

import concourse.bass as bass
import concourse.bacc as bacc
import concourse.mybir as mybir
import concourse.tile as tile
from concourse.bass_utils import run_bass_kernel_spmd
from concourse.masks import make_identity

F32 = mybir.dt.float32
I32 = mybir.dt.int32
AX = mybir.AxisListType
OP = mybir.AluOpType

IM = 256
G = 512
J = 6
JT = 5             # live taps per dim (rne centering => the dropped 6th
                   # tap always has |U| >= 2.5, KB weight <= 2.6e-3)
ALPHA = 2.34 * J
TWO_PI = 2.0 * np.pi
PAD = 517          # 512 + 2 left halo + 3 right halo
NT, NC, K = 8, 8, 16384
CELL = NC * 2      # floats per (p,q) cell = 16 (all coils interleaved)
TW = PAD * CELL    # table row width in elements = 8272
NTILE = 16         # point tiles of 1024 points (8 groups x 128 partitions)
GRP = 8            # groups per tile
DEG = 8            # KB weight polynomial degree (in t); abs err ~8.5e-6
NCELLS = PAD * PAD # flat cell count of the table


# ---------------------------------------------------------------- host consts
def _host_consts():
    # apodization correction 1/FT(kb)
    f = (np.arange(IM) - IM // 2) / G
    z = (np.pi * J * f) ** 2 - ALPHA ** 2
    s = np.sqrt(np.abs(z))
    val = np.where(z < 0, np.sinh(s) / np.maximum(s, 1e-12), np.sinc(s / np.pi))
    ftkb = (J / np.i0(ALPHA)) * val
    scal = 1.0 / ftkb
    # A[u, x'] = e^{i pi u/2 - 2 pi i u x'/G} * scal[x'] / sqrt(G)
    u = np.arange(G)[:, None].astype(np.float64)
    xp = np.arange(IM)[None, :].astype(np.float64)
    A = np.exp(1j * np.pi * u / 2 - 2j * np.pi * u * xp / G) * scal[None, :] / np.sqrt(G)
    art = np.ascontiguousarray(A.T.real, dtype=np.float32)   # [256, 512]
    ait = np.ascontiguousarray(A.T.imag, dtype=np.float32)
    aitn = np.ascontiguousarray(-A.T.imag, dtype=np.float32)
    # polynomial fit of w(t) = i0(ALPHA*sqrt(t))/i0(ALPHA) on t in [0,1]
    n = 512
    x = (1 - np.cos(np.pi * (np.arange(n) + 0.5) / n)) / 2
    w = np.i0(ALPHA * np.sqrt(x)) / np.i0(ALPHA)
    V = np.vander(x, DEG + 1, increasing=True)
    c, *_ = np.linalg.lstsq(V, w, rcond=None)
    return art, ait, aitn, c.astype(np.float64)


_ART, _AIT, _AITN, _CHEB = _host_consts()


# ---------------------------------------------------------------- bass build
def build_bass(debug=False):
    nc = bacc.Bacc()

    x_in = nc.declare_dram_parameter("x", [2, IM, IM], F32, isOutput=False)
    k_in = nc.declare_dram_parameter("kk", [2, K], F32, isOutput=False)
    c_in = nc.declare_dram_parameter("coil", [NC, 2, IM, IM], F32, isOutput=False)
    w_in = nc.declare_dram_parameter("wr", [128, NTILE * 128], F32, isOutput=False)
    art_in = nc.declare_dram_parameter("art", [IM, G], F32, isOutput=False)
    ait_in = nc.declare_dram_parameter("ait", [IM, G], F32, isOutput=False)
    aitn_in = nc.declare_dram_parameter("aitn", [IM, G], F32, isOutput=False)
    y_out = nc.declare_dram_parameter("yr", [128, NTILE * 128], F32, isOutput=True)

    BF16 = mybir.dt.bfloat16
    W2 = 4 * CELL          # quad-row cell: 4 v-rows x 16 cri = 64 el
    TW2 = PAD * W2         # table Q-row width = 33088 el
    PROWS = 130            # quads of table rows 0..519 (R = m + 4)
    T_dram = nc.dram_tensor("T0", [PROWS, TW2], BF16)

    CH = _CHEB
    with tile.TileContext(nc) as tc:
        with (
            tc.tile_pool(name="const", bufs=1) as constp,
            tc.tile_pool(name="work", bufs=1) as workp,
            tc.tile_pool(name="ctile", bufs=2) as coilp,
            tc.tile_pool(name="mtile", bufs=4) as mp,
            tc.tile_pool(name="bt", bufs=8) as btp,
            tc.tile_pool(name="stg", bufs=1) as stgp,
            tc.tile_pool(name="patch", bufs=2) as patchp,
            tc.tile_pool(name="w36", bufs=2) as w36p,
            tc.tile_pool(name="wp", bufs=2) as wpp,
            tc.tile_pool(name="ps1", bufs=4, space="PSUM") as ps1,
            tc.tile_pool(name="ps2", bufs=4, space="PSUM") as ps2,
        ):
            # ---------------- constants ----------------
            ident = constp.tile([128, 128], F32, tag="ident")
            make_identity(nc, ident[:])
            # A matrices: DMA f32, convert once to bf16 for PE
            art = []
            for name, src in (("art", art_in), ("ait", ait_in), ("aitn", aitn_in)):
                ts_ = []
                for xt in range(2):
                    tf = constp.tile([128, G], F32, tag=f"{name}f{xt}")
                    nc.sync.dma_start(out=tf[:], in_=src[xt * 128:(xt + 1) * 128, :])
                    tb = constp.tile([128, G], BF16, tag=f"{name}b{xt}")
                    nc.scalar.copy(out=tb[:], in_=tf[:])
                    # stage-2 copy with v-columns regrouped (r4, p) so the
                    # stride-4 quad slices become contiguous weight loads
                    tp_ = constp.tile([128, G], BF16, tag=f"{name}p{xt}")
                    pin = bass.AP(
                        tf[:].tensor, tf[:].offset,
                        [tf[:].ap[0], [1, 4], [4, 128]],
                    )
                    nc.scalar.copy(out=tp_[:], in_=pin)
                    ts_.append((tb, tp_))
                art.append(ts_)
            artT = [a[0] for a in art[0]]
            aitT = [a[0] for a in art[1]]
            aitnT = [a[0] for a in art[2]]
            artP = [a[1] for a in art[0]]
            aitP = [a[1] for a in art[1]]
            aitnP = [a[1] for a in art[2]]

            offs = constp.tile([128, JT], F32, tag="offs")
            for a in range(JT):
                nc.vector.memset(offs[:, a:a + 1], float(3 - (a + 1)))
            ylat = constp.tile([128, 8], F32, tag="ylat")
            for a in range(8):
                nc.vector.memset(ylat[:, a:a + 1], float(-a))

            # ---------------- k -> [p, c] transpose ----------------
            kg = workp.tile([128, 256], F32, tag="kg")  # [p, (d, c)]
            for d in range(2):
                kt_in = workp.tile([128, 128], F32, tag="ktin")
                nc.sync.dma_start(
                    out=kt_in[:], in_=k_in[d].rearrange("(c p) -> c p", p=128)
                )
                ktp = ps2.tile([128, 128], F32, tag="psb")
                nc.tensor.transpose(ktp[:], kt_in[:], ident[:])
                nc.scalar.copy(out=kg[:, d * 128:(d + 1) * 128], in_=ktp[:])

            # ---------------- w load + sqrt ----------------
            wsq = workp.tile([128, NTILE * 128], F32, tag="wsq")
            nc.sync.dma_start(out=wsq[:], in_=w_in[:])
            nc.scalar.activation(
                out=wsq[:], in_=wsq[:],
                func=mybir.ActivationFunctionType.Sqrt,
            )

            # ---------------- index & weight math (DVE) ----------------
            # gx = om*(G/2pi); gx += 512 if gx < 0  -> [0, 512)
            gx0 = workp.tile([128, 256], F32, tag="gx0")
            nc.vector.tensor_scalar_mul(gx0[:], kg[:], float(G / TWO_PI))
            msk = workp.tile([128, 256], F32, tag="msk")
            nc.vector.tensor_scalar(
                out=msk[:], in0=gx0[:], scalar1=0.0, scalar2=None, op0=OP.is_lt
            )
            gxy = workp.tile([128, 256], F32, tag="gxy")
            nc.vector.scalar_tensor_tensor(
                out=gxy[:], in0=msk[:], scalar=float(G), in1=gx0[:],
                op0=OP.mult, op1=OP.add,
            )
            # gm3 = gxy - 3 ; f = rne(gm3 - 0.498) via 2^23 trick ; r = gm3 - f
            gm3 = workp.tile([128, 256], F32, tag="gm3")
            nc.vector.tensor_scalar(
                out=gm3[:], in0=gxy[:], scalar1=3.0, scalar2=None, op0=OP.subtract
            )
            fl = workp.tile([128, 256], F32, tag="fl")
            nc.vector.tensor_scalar(
                out=fl[:], in0=gm3[:],
                scalar1=0.0, scalar2=12582912.0,
                op0=OP.add, op1=OP.add,
            )
            nc.vector.tensor_scalar(
                out=fl[:], in0=fl[:], scalar1=12582912.0, scalar2=None,
                op0=OP.subtract,
            )
            rr = workp.tile([128, 256], F32, tag="rr")
            nc.vector.tensor_sub(rr[:], gm3[:], fl[:])

            # Q0 = floor((fl_y + 5)/4): h = fl*0.25 + 1.25, rne(h - 0.498)
            fp = workp.tile([128, 128], F32, tag="fp")
            nc.vector.tensor_scalar(
                out=fp[:], in0=fl[:, 128:256], scalar1=0.25, scalar2=1.25,
                op0=OP.mult, op1=OP.add,
            )
            nc.vector.tensor_scalar(
                out=fp[:], in0=fp[:],
                scalar1=-0.498046875, scalar2=12582912.0,
                op0=OP.add, op1=OP.add,
            )
            nc.vector.tensor_scalar(
                out=fp[:], in0=fp[:], scalar1=12582912.0, scalar2=None,
                op0=OP.subtract,
            )
            # sY = gy + 4 - 4*Q0 = gm3_y + 7 - 4*fp; y-tap j weight
            # arg = sY - j for gathered rows 4*Q0 + j, j = 0..7 (R = m + 4)
            sY = workp.tile([128, 128], F32, tag="sY")
            nc.vector.scalar_tensor_tensor(
                out=sY[:], in0=fp[:], scalar=-4.0, in1=gm3[:, 128:256],
                op0=OP.mult, op1=OP.add,
            )
            nc.vector.tensor_scalar_add(sY[:], sY[:], 7.0)

            # tap weight args: x: rr_x + offs (5); y: sY - j (6)
            NXC = 128 * JT
            ut = workp.tile([128, NXC + 128 * 8], F32, tag="ut")
            utx3 = ut[:, 0:NXC].rearrange("p (c a) -> p c a", a=JT)
            nc.vector.tensor_tensor(
                out=utx3,
                in0=rr[:, 0:128].unsqueeze(2).broadcast_to([128, 128, JT]),
                in1=offs[:].unsqueeze(1).broadcast_to([128, 128, JT]),
                op=OP.add,
            )
            uty3 = ut[:, NXC:].rearrange("p (c j) -> p c j", j=8)
            nc.vector.tensor_tensor(
                out=uty3,
                in0=sY[:].unsqueeze(2).broadcast_to([128, 128, 8]),
                in1=ylat[:].unsqueeze(1).broadcast_to([128, 128, 8]),
                op=OP.add,
            )
            # t = max(0, 1 - (U/3)^2)
            tsq = workp.tile([128, 128 * JT + 128 * 8], F32, tag="tsq")
            nc.vector.tensor_mul(tsq[:], ut[:], ut[:])
            nc.vector.tensor_scalar(
                out=tsq[:], in0=tsq[:], scalar1=float(-1.0 / 9.0), scalar2=1.0,
                op0=OP.mult, op1=OP.add,
            )
            nc.vector.tensor_scalar_max(tsq[:], tsq[:], 0.0)
            # Horner in t
            acc = workp.tile([128, 128 * JT + 128 * 8], F32, tag="acc")
            nc.vector.tensor_scalar(
                out=acc[:], in0=tsq[:], scalar1=float(CH[DEG]),
                scalar2=float(CH[DEG - 1]), op0=OP.mult, op1=OP.add,
            )
            for d in range(DEG - 2, -1, -1):
                nc.vector.tensor_mul(acc[:], acc[:], tsq[:])
                nc.vector.tensor_scalar_add(acc[:], acc[:], float(CH[d]))
            # acc = W_all [p, (d, c, a)]: d=0 -> wx taps, d=1 -> wy taps

            # gather cell4 indices: (Q0 + b2)*517 + 3 + fx, b2 = 0..1
            cbt = constp.tile([128, 2], F32, tag="cbt")
            for a in range(2):
                nc.vector.memset(cbt[:, a:a + 1], float(a * PAD + 3))
            fy517 = workp.tile([128, 128], F32, tag="fy517")
            nc.vector.tensor_scalar_mul(fy517[:], fp[:], float(PAD))
            idxf = workp.tile([128, 128 * 2], F32, tag="idxf")
            idxf3 = idxf[:].rearrange("p (c b) -> p c b", b=2)
            nc.vector.tensor_tensor(
                out=idxf3,
                in0=fy517[:].unsqueeze(2).broadcast_to([128, 128, 2]),
                in1=cbt[:].unsqueeze(1).broadcast_to([128, 128, 2]),
                op=OP.add,
            )
            nc.vector.tensor_tensor(
                out=idxf3,
                in0=idxf3,
                in1=fl[:, 0:128].unsqueeze(2).broadcast_to([128, 128, 2]),
                op=OP.add,
            )
            idx32 = workp.tile([128, 128 * 2], I32, tag="idx32")
            nc.vector.tensor_copy(out=idx32[:], in_=idxf[:])

            # ---------------- res buffer ----------------
            res = workp.tile([128, NTILE * 128], F32, tag="res")

            # x image tiles (persist across all coils)
            xts = []
            for xt in range(2):
                xt_t = workp.tile([128, 2 * IM], F32, tag=f"xt{xt}")
                nc.sync.dma_start(
                    out=xt_t[:],
                    in_=x_in[:, xt * 128:(xt + 1) * 128, :]
                    .rearrange("ri x y -> x ri y"),
                )
                xts.append(xt_t)

            # 1 persistent bf16 staging; partition p holds the v-row quad
            # (4p .. 4p+3), cells4 laid out (q, r4, e)
            stg0 = stgp.tile([128, G * W2], BF16, tag="stg0")

            for c in range(NC):
                # ---- coil multiply (bf16 out for PE) ----
                mt = []
                for xt in range(2):
                    ct = coilp.tile([128, 2 * IM], F32, tag="ct")
                    nc.sync.dma_start(
                        out=ct[:],
                        in_=c_in[c, :, xt * 128:(xt + 1) * 128, :]
                        .rearrange("ri x y -> x ri y"),
                    )
                    xt_t = xts[xt]
                    m = mp.tile([128, 2 * IM], BF16, tag="m")
                    xr, xi = xt_t[:, 0:IM], xt_t[:, IM:2 * IM]
                    cr, ci = ct[:, 0:IM], ct[:, IM:2 * IM]
                    mr, mi = m[:, 0:IM], m[:, IM:2 * IM]
                    t1 = mp.tile([128, IM], F32, tag="cm1")
                    t2 = mp.tile([128, IM], F32, tag="cm2")
                    nc.vector.tensor_mul(t1[:], xr, cr)
                    nc.vector.tensor_mul(t2[:], xi, ci)
                    nc.vector.tensor_sub(mr, t1[:], t2[:])
                    nc.vector.tensor_mul(t1[:], xr, ci)
                    nc.vector.tensor_mul(t2[:], xi, cr)
                    nc.vector.tensor_add(mi, t1[:], t2[:])
                    mt.append(m)
                # ---- stage 1: BT[y, u] per (ri, Yt) ----
                bt = {}
                for yt in range(2):
                    pr = ps1.tile([128, G], F32, tag="psa")
                    pi = ps1.tile([128, G], F32, tag="psa")
                    for xt in range(2):
                        mrb = mt[xt][:, yt * 128:yt * 128 + 128]
                        mib = mt[xt][:, IM + yt * 128:IM + yt * 128 + 128]
                        st = xt == 0
                        sp = xt == 1
                        nc.tensor.matmul(pr[:], mrb, artT[xt][:], start=st, stop=False)
                        nc.tensor.matmul(pi[:], mrb, aitT[xt][:], start=st, stop=False)
                        nc.tensor.matmul(pr[:], mib, aitnT[xt][:], start=False, stop=sp)
                        nc.tensor.matmul(pi[:], mib, artT[xt][:], start=False, stop=sp)
                    btr = btp.tile([128, G], BF16, tag="bt")
                    bti = btp.tile([128, G], BF16, tag="bt")
                    nc.scalar.copy(out=btr[:], in_=pr[:])
                    nc.scalar.copy(out=bti[:], in_=pi[:])
                    bt[(0, yt)] = btr
                    bt[(1, yt)] = bti
                # ---- stage 2: G[v, u] with v = 4p + r4 via stride-4 A
                # column slices; drain into quad-row staging ----
                for r2 in range(4):
                    stg3 = stg0[:].rearrange("p (q w) -> p q w", w=W2)
                    gr = ps2.tile([128, G], F32, tag="psb")
                    gi = ps2.tile([128, G], F32, tag="psb")
                    for yt in range(2):
                        av = artP[yt][:, r2 * 128:(r2 + 1) * 128]
                        aiv = aitP[yt][:, r2 * 128:(r2 + 1) * 128]
                        ainv = aitnP[yt][:, r2 * 128:(r2 + 1) * 128]
                        btr = bt[(0, yt)]
                        bti = bt[(1, yt)]
                        st = yt == 0
                        sp = yt == 1
                        nc.tensor.matmul(gr[:], av, btr[:], start=st, stop=False)
                        nc.tensor.matmul(gi[:], aiv, btr[:], start=st, stop=False)
                        nc.tensor.matmul(gr[:], ainv, bti[:], start=False, stop=sp)
                        nc.tensor.matmul(gi[:], av, bti[:], start=False, stop=sp)
                    c2 = r2 * CELL + 2 * c
                    # split strided drains across Scalar and Vector engines
                    if c % 2 == 0:
                        nc.scalar.copy(out=stg3[:, :, c2:c2 + 1], in_=gr[:].unsqueeze(2))
                        nc.vector.tensor_copy(out=stg3[:, :, c2 + 1:c2 + 2], in_=gi[:].unsqueeze(2))
                    else:
                        nc.vector.tensor_copy(out=stg3[:, :, c2:c2 + 1], in_=gr[:].unsqueeze(2))
                        nc.scalar.copy(out=stg3[:, :, c2 + 1:c2 + 2], in_=gi[:].unsqueeze(2))

            # ---- table stores: main (Q 1..128) + q halos, then halo
            # quads Q0 (v 508..511 <- stg[127]) and Q129 (v 0..3 <- stg[0])
            t_stores = []
            Th = T_dram
            t_stores.append(nc.sync.dma_start(
                out=Th[1:129, 2 * W2:2 * W2 + G * W2], in_=stg0[:]
            ))
            t_stores.append(nc.sync.dma_start(
                out=Th[1:129, 514 * W2:517 * W2], in_=stg0[:, 0:3 * W2],
            ))
            t_stores.append(nc.sync.dma_start(
                out=Th[1:129, 0:2 * W2], in_=stg0[:, 510 * W2:512 * W2],
            ))
            for dst, psrc in ((0, 127), (129, 0)):
                t_stores += [
                    nc.sync.dma_start(
                        out=Th[dst:dst + 1, 2 * W2:2 * W2 + G * W2],
                        in_=stg0[psrc:psrc + 1, :],
                    ),
                    nc.sync.dma_start(
                        out=Th[dst:dst + 1, 514 * W2:517 * W2],
                        in_=stg0[psrc:psrc + 1, 0:3 * W2],
                    ),
                    nc.sync.dma_start(
                        out=Th[dst:dst + 1, 0:2 * W2],
                        in_=stg0[psrc:psrc + 1, 510 * W2:512 * W2],
                    ),
                ]

            # ======== gather + combine ========
            # per index: 320 contiguous el (5 cells4 = 5q x 4rows x 16cri,
            # 640B); 2 quad-taps/point, 1 idx/partition/call -> 256 calls
            tab_flat = T_dram[:].rearrange("r (q e) -> (r q) e", e=W2)
            all_gathers = []
            for t in range(NTILE):
                # W[g, r4, b2, a] = wy[g, 4*b2 + r4] * wx[g, a]
                w240 = w36p.tile([128, GRP * 40], F32, tag="w36")
                for r2 in range(4):
                    ow = bass.AP(
                        w240[:].tensor, w240[:].offset + r2 * 10,
                        [w240[:].ap[0], [40, GRP], [5, 2], [1, 5]],
                    )
                    wyv = bass.AP(
                        acc[:].tensor,
                        acc[:].offset + 128 * JT + t * GRP * 8 + r2,
                        [acc[:].ap[0], [8, GRP], [4, 2], [0, 5]],
                    )
                    wxv = bass.AP(
                        acc[:].tensor, acc[:].offset + t * GRP * JT,
                        [acc[:].ap[0], [JT, GRP], [0, 2], [1, 5]],
                    )
                    nc.vector.tensor_tensor(out=ow, in0=wyv, in1=wxv, op=OP.mult)
                patch = patchp.tile([128, GRP * 2 * JT * W2], BF16, tag="patch")
                for g in range(GRP):
                    for b in range(2):
                        col = (t * GRP + g) * 2 + b
                        gi_ = nc.gpsimd.indirect_dma_start(
                            out=patch[:, (g * 2 + b) * JT * W2:
                                      (g * 2 + b + 1) * JT * W2],
                            out_offset=None,
                            in_=tab_flat,
                            in_offset=bass.IndirectOffsetOnAxis(
                                ap=idx32[:, col:col + 1], axis=0
                            ),
                        )
                        all_gathers.append(gi_)
                # WP[p, (g, cr, (r4, b2, a))] = patch[p, (g, b2, a, r4, cr)] * W
                wp = wpp.tile([128, GRP * 40 * CELL], BF16, tag="wpt")
                for r2 in range(4):
                    pv = bass.AP(
                        patch[:].tensor, patch[:].offset + r2 * CELL,
                        [patch[:].ap[0],
                         [2 * JT * W2, GRP], [1, CELL], [W2, 10]],
                    )
                    wv = bass.AP(
                        w240[:].tensor, w240[:].offset + r2 * 10,
                        [w240[:].ap[0], [40, GRP], [0, CELL], [1, 10]],
                    )
                    ov = bass.AP(
                        wp[:].tensor, wp[:].offset + r2 * 10,
                        [wp[:].ap[0],
                         [40 * CELL, GRP], [40, CELL], [1, 10]],
                    )
                    nc.vector.tensor_tensor(out=ov, in0=pv, in1=wv, op=OP.mult)
                # reduce innermost 40 -> res[:, t*128 + g*16 + cr]
                rv = bass.AP(
                    res[:].tensor, res[:].offset + t * 128,
                    [res[:].ap[0], [16, GRP], [1, CELL]],
                )
                wp3 = wp[:].rearrange("p (g cr ba) -> p g cr ba", cr=CELL, ba=40)
                nc.vector.tensor_reduce(out=rv, in_=wp3, axis=AX.X, op=OP.add)

            # explicit RAW edges: gathers after table stores
            for gi_ in all_gathers:
                for si in t_stores:
                    tile.add_dep_helper(gi_.ins, si.ins, reason="T RAW")

            # ======== sqrt(w) scale + store ========
            nc.vector.tensor_mul(res[:], res[:], wsq[:])
            nc.sync.dma_start(out=y_out[:], in_=res[:])



            if debug:
                dbg_outs = {
                    "kgo": kg, "acco": acc, "idxo": idx32, "flo": fl, "rro": rr,
                }
                for nm, t_ in dbg_outs.items():
                    o = nc.dram_tensor(nm, list(t_[:].shape), t_[:].dtype,
                                       kind="ExternalOutput")
                    nc.sync.dma_start(out=o[:], in_=t_[:])
                o = nc.dram_tensor("t0o", [PAD, TW], BF16, kind="ExternalOutput")
                di = nc.sync.dma_start(out=o[:], in_=T_dram[:])
                for si in t_stores:
                    tile.add_dep_helper(di.ins, si.ins, reason="T dump RAW")

    nc.compile()
    return nc


_NC_CACHE = None


def _get_nc():
    global _NC_CACHE
    if _NC_CACHE is None:
        _NC_CACHE = build_bass()
    return _NC_CACHE


# ---------------------------------------------------------------- host glue
def _shuffle_w(w_t):
    # w[c, ri, K] -> [p, (t, g, c, ri)] with K = t*1024 + g*128 + p
    v = w_t.reshape(NC, 2, NTILE, GRP, 128)
    return np.ascontiguousarray(v.transpose(4, 2, 3, 0, 1).reshape(128, NTILE * 128))


def _unshuffle_y(yr):
    # [p, (t, g, c, ri)] -> y[c, ri, K]
    v = yr.reshape(128, NTILE, GRP, NC, 2)
    return np.ascontiguousarray(v.transpose(3, 4, 1, 2, 0).reshape(NC, 2, K))


def make_in_maps(x, k, coil_sensitivities, w):
    in_maps = []
    coil0 = np.ascontiguousarray(coil_sensitivities[0], dtype=np.float32)
    for t in range(NT):
        in_maps.append({
            "x": np.ascontiguousarray(x[t], dtype=np.float32),
            "kk": np.ascontiguousarray(k[t], dtype=np.float32),
            "coil": coil0,
            "wr": _shuffle_w(np.asarray(w[t], dtype=np.float32)),
            "art": _ART, "ait": _AIT, "aitn": _AITN,
        })
    return in_maps


def run(x, k, coil_sensitivities, w, trace=False, **spmd_kwargs):
    nc = _get_nc()
    in_maps = make_in_maps(x, k, coil_sensitivities, w)
    r = run_bass_kernel_spmd(nc, in_maps, list(range(NT)), trace=trace, **spmd_kwargs)
    y = np.stack([_unshuffle_y(r.results[t]["yr"]) for t in range(NT)], axis=0)
    return y.astype(np.float32), r


def kernel(x, k, coil_sensitivities, w):
    y, _ = run(x, k, coil_sensitivities, w, trace=False)
    return y


# revision 25
# speedup vs baseline: 2.2543x; 1.0100x over previous
"""Trainium2 Bass kernel for nn_RadialModel (forward NUFFT, radial MRI).

Per-core (1 frame, all 8 coils):
  1. coil multiply (DVE, bf16 out)       cimage = (xr+ixi)*(cr+ici)
  2. DFT via PE bf16 matmuls (two stages): G[v,u] = A @ (M^T @ A^T) with
     apodization + fftshift phases folded into the constant A matrices
  3. store grid to a DRAM table (bf16), coil-interleaved cells
     [p=v_pad(517), q=u_pad(517), cri(16)] with 2/3-cell wraparound halo
  4. Kaiser-Bessel interpolation with round-to-nearest centering so the
     fractional offset is in [-0.5, 0.5]: a 5x5 tap window then captures
     every tap with weight > 2.6e-3 (the 6th tap of the reference's 6x6
     always has |U| >= 2.5 there).  The table packs FOUR v-rows per cell
     (quad-row cells, 64 el), so one gather index fetches 5 q-cells x 4
     rows x 16 cri = 640B and a point needs only TWO quad-row taps (the
     8-row window always covers the 5 needed rows; stray rows get ~4e-6
     weights from the clamped KB polynomial).  One index per partition
     per call is a HW limit of the SWDGE indirect path (~1.25us fixed
     cost each) -> 256 calls, the dominant cost of the kernel.
  5. per-r4 weighted multiply + 40-tap reduce on DVE, sqrt(w) scale + store

Sharding: one frame (nt) per NeuronCore, 8 cores. Host does only
shard/reshape/unshuffle; all math on device.
"""
import math
import numpy as np

import concourse.bass as bass
import concourse.bacc as bacc
import concourse.mybir as mybir
import concourse.tile as tile
from concourse.bass_utils import run_bass_kernel_spmd
from concourse.masks import make_identity

F32 = mybir.dt.float32
I32 = mybir.dt.int32
AX = mybir.AxisListType
OP = mybir.AluOpType

IM = 256
G = 512
J = 6
JT = 5             # live taps per dim (rne centering => the dropped 6th
                   # tap always has |U| >= 2.5, KB weight <= 2.6e-3)
ALPHA = 2.34 * J
TWO_PI = 2.0 * np.pi
PAD = 517          # 512 + 2 left halo + 3 right halo
NT, NC, K = 8, 8, 16384
CELL = NC * 2      # floats per (p,q) cell = 16 (all coils interleaved)
TW = PAD * CELL    # table row width in elements = 8272
NTILE = 16         # point tiles of 1024 points (8 groups x 128 partitions)
GRP = 8            # groups per tile
DEG = 8            # KB weight polynomial degree (in t); abs err ~8.5e-6
NCELLS = PAD * PAD # flat cell count of the table


# ---------------------------------------------------------------- host consts
def _host_consts():
    # apodization correction 1/FT(kb)
    f = (np.arange(IM) - IM // 2) / G
    z = (np.pi * J * f) ** 2 - ALPHA ** 2
    s = np.sqrt(np.abs(z))
    val = np.where(z < 0, np.sinh(s) / np.maximum(s, 1e-12), np.sinc(s / np.pi))
    ftkb = (J / np.i0(ALPHA)) * val
    scal = 1.0 / ftkb
    # A[u, x'] = e^{i pi u/2 - 2 pi i u x'/G} * scal[x'] / sqrt(G)
    u = np.arange(G)[:, None].astype(np.float64)
    xp = np.arange(IM)[None, :].astype(np.float64)
    A = np.exp(1j * np.pi * u / 2 - 2j * np.pi * u * xp / G) * scal[None, :] / np.sqrt(G)
    art = np.ascontiguousarray(A.T.real, dtype=np.float32)   # [256, 512]
    ait = np.ascontiguousarray(A.T.imag, dtype=np.float32)
    aitn = np.ascontiguousarray(-A.T.imag, dtype=np.float32)
    # polynomial fit of w(t) = i0(ALPHA*sqrt(t))/i0(ALPHA) on t in [0,1]
    n = 512
    x = (1 - np.cos(np.pi * (np.arange(n) + 0.5) / n)) / 2
    w = np.i0(ALPHA * np.sqrt(x)) / np.i0(ALPHA)
    V = np.vander(x, DEG + 1, increasing=True)
    c, *_ = np.linalg.lstsq(V, w, rcond=None)
    return art, ait, aitn, c.astype(np.float64)


_ART, _AIT, _AITN, _CHEB = _host_consts()


# ---------------------------------------------------------------- bass build
def build_bass(debug=False):
    nc = bacc.Bacc()

    x_in = nc.declare_dram_parameter("x", [2, IM, IM], F32, isOutput=False)
    k_in = nc.declare_dram_parameter("kk", [2, K], F32, isOutput=False)
    c_in = nc.declare_dram_parameter("coil", [NC, 2, IM, IM], F32, isOutput=False)
    w_in = nc.declare_dram_parameter("wr", [128, NTILE * 128], F32, isOutput=False)
    art_in = nc.declare_dram_parameter("art", [IM, G], F32, isOutput=False)
    ait_in = nc.declare_dram_parameter("ait", [IM, G], F32, isOutput=False)
    aitn_in = nc.declare_dram_parameter("aitn", [IM, G], F32, isOutput=False)
    y_out = nc.declare_dram_parameter("yr", [128, NTILE * 128], F32, isOutput=True)

    BF16 = mybir.dt.bfloat16
    W2 = 4 * CELL          # quad-row cell: 4 v-rows x 16 cri = 64 el
    TW2 = PAD * W2         # table Q-row width = 33088 el
    PROWS = 130            # quads of table rows 0..519 (R = m + 4)
    T_dram = nc.dram_tensor("T0", [PROWS, TW2], BF16)

    CH = _CHEB
    with tile.TileContext(nc) as tc:
        with (
            tc.tile_pool(name="const", bufs=1) as constp,
            tc.tile_pool(name="work", bufs=1) as workp,
            tc.tile_pool(name="ctile", bufs=2) as coilp,
            tc.tile_pool(name="mtile", bufs=4) as mp,
            tc.tile_pool(name="bt", bufs=8) as btp,
            tc.tile_pool(name="stg", bufs=1) as stgp,
            tc.tile_pool(name="patch", bufs=2) as patchp,
            tc.tile_pool(name="w36", bufs=2) as w36p,
            tc.tile_pool(name="wp", bufs=2) as wpp,
            tc.tile_pool(name="rt", bufs=3) as resp,
            tc.tile_pool(name="ps1", bufs=4, space="PSUM") as ps1,
            tc.tile_pool(name="ps2", bufs=4, space="PSUM") as ps2,
        ):
            # ---------------- constants ----------------
            ident = constp.tile([128, 128], F32, tag="ident")
            make_identity(nc, ident[:])
            # A matrices: DMA f32, convert once to bf16 for PE
            art = []
            for name, src in (("art", art_in), ("ait", ait_in), ("aitn", aitn_in)):
                ts_ = []
                for xt in range(2):
                    tf = constp.tile([128, G], F32, tag=f"{name}f{xt}")
                    nc.sync.dma_start(out=tf[:], in_=src[xt * 128:(xt + 1) * 128, :])
                    tb = constp.tile([128, G], BF16, tag=f"{name}b{xt}")
                    nc.scalar.copy(out=tb[:], in_=tf[:])
                    # stage-2 copy with v-columns regrouped (r4, p) so the
                    # stride-4 quad slices become contiguous weight loads
                    tp_ = constp.tile([128, G], BF16, tag=f"{name}p{xt}")
                    pin = bass.AP(
                        tf[:].tensor, tf[:].offset,
                        [tf[:].ap[0], [1, 4], [4, 128]],
                    )
                    nc.scalar.copy(out=tp_[:], in_=pin)
                    ts_.append((tb, tp_))
                art.append(ts_)
            artT = [a[0] for a in art[0]]
            aitT = [a[0] for a in art[1]]
            aitnT = [a[0] for a in art[2]]
            artP = [a[1] for a in art[0]]
            aitP = [a[1] for a in art[1]]
            aitnP = [a[1] for a in art[2]]

            offs = constp.tile([128, JT], F32, tag="offs")
            for a in range(JT):
                nc.vector.memset(offs[:, a:a + 1], float(3 - (a + 1)))
            ylat = constp.tile([128, 8], F32, tag="ylat")
            for a in range(8):
                nc.vector.memset(ylat[:, a:a + 1], float(-a))

            # ---------------- k -> [p, c] transpose ----------------
            kg = workp.tile([128, 256], F32, tag="kg")  # [p, (d, c)]
            for d in range(2):
                kt_in = workp.tile([128, 128], F32, tag="ktin")
                nc.sync.dma_start(
                    out=kt_in[:], in_=k_in[d].rearrange("(c p) -> c p", p=128)
                )
                ktp = ps2.tile([128, 128], F32, tag="psb")
                nc.tensor.transpose(ktp[:], kt_in[:], ident[:])
                nc.scalar.copy(out=kg[:, d * 128:(d + 1) * 128], in_=ktp[:])

            # ---------------- w load + sqrt ----------------
            wsq = workp.tile([128, NTILE * 128], F32, tag="wsq")
            nc.sync.dma_start(out=wsq[:], in_=w_in[:])
            nc.scalar.activation(
                out=wsq[:], in_=wsq[:],
                func=mybir.ActivationFunctionType.Sqrt,
            )

            # ---------------- index & weight math (DVE) ----------------
            # gx = om*(G/2pi); gx += 512 if gx < 0  -> [0, 512)
            gx0 = workp.tile([128, 256], F32, tag="gx0")
            nc.vector.tensor_scalar_mul(gx0[:], kg[:], float(G / TWO_PI))
            msk = workp.tile([128, 256], F32, tag="msk")
            nc.vector.tensor_scalar(
                out=msk[:], in0=gx0[:], scalar1=0.0, scalar2=None, op0=OP.is_lt
            )
            gxy = workp.tile([128, 256], F32, tag="gxy")
            nc.vector.scalar_tensor_tensor(
                out=gxy[:], in0=msk[:], scalar=float(G), in1=gx0[:],
                op0=OP.mult, op1=OP.add,
            )
            # gm3 = gxy - 3 ; f = rne(gm3 - 0.498) via 2^23 trick ; r = gm3 - f
            gm3 = workp.tile([128, 256], F32, tag="gm3")
            nc.vector.tensor_scalar(
                out=gm3[:], in0=gxy[:], scalar1=3.0, scalar2=None, op0=OP.subtract
            )
            fl = workp.tile([128, 256], F32, tag="fl")
            nc.vector.tensor_scalar(
                out=fl[:], in0=gm3[:],
                scalar1=0.0, scalar2=12582912.0,
                op0=OP.add, op1=OP.add,
            )
            nc.vector.tensor_scalar(
                out=fl[:], in0=fl[:], scalar1=12582912.0, scalar2=None,
                op0=OP.subtract,
            )
            rr = workp.tile([128, 256], F32, tag="rr")
            nc.vector.tensor_sub(rr[:], gm3[:], fl[:])

            # Q0 = floor((fl_y + 5)/4): h = fl*0.25 + 1.25, rne(h - 0.498)
            fp = workp.tile([128, 128], F32, tag="fp")
            nc.vector.tensor_scalar(
                out=fp[:], in0=fl[:, 128:256], scalar1=0.25, scalar2=1.25,
                op0=OP.mult, op1=OP.add,
            )
            nc.vector.tensor_scalar(
                out=fp[:], in0=fp[:],
                scalar1=-0.498046875, scalar2=12582912.0,
                op0=OP.add, op1=OP.add,
            )
            nc.vector.tensor_scalar(
                out=fp[:], in0=fp[:], scalar1=12582912.0, scalar2=None,
                op0=OP.subtract,
            )
            # sY = gy + 4 - 4*Q0 = gm3_y + 7 - 4*fp; y-tap j weight
            # arg = sY - j for gathered rows 4*Q0 + j, j = 0..7 (R = m + 4)
            sY = workp.tile([128, 128], F32, tag="sY")
            nc.vector.scalar_tensor_tensor(
                out=sY[:], in0=fp[:], scalar=-4.0, in1=gm3[:, 128:256],
                op0=OP.mult, op1=OP.add,
            )
            nc.vector.tensor_scalar_add(sY[:], sY[:], 7.0)

            # tap weight args: x: rr_x + offs (5); y: sY - j (6)
            NXC = 128 * JT
            ut = workp.tile([128, NXC + 128 * 8], F32, tag="ut")
            utx3 = ut[:, 0:NXC].rearrange("p (c a) -> p c a", a=JT)
            nc.vector.tensor_tensor(
                out=utx3,
                in0=rr[:, 0:128].unsqueeze(2).broadcast_to([128, 128, JT]),
                in1=offs[:].unsqueeze(1).broadcast_to([128, 128, JT]),
                op=OP.add,
            )
            uty3 = ut[:, NXC:].rearrange("p (c j) -> p c j", j=8)
            nc.vector.tensor_tensor(
                out=uty3,
                in0=sY[:].unsqueeze(2).broadcast_to([128, 128, 8]),
                in1=ylat[:].unsqueeze(1).broadcast_to([128, 128, 8]),
                op=OP.add,
            )
            # t = max(0, 1 - (U/3)^2)
            tsq = workp.tile([128, 128 * JT + 128 * 8], F32, tag="tsq")
            nc.vector.tensor_mul(tsq[:], ut[:], ut[:])
            nc.vector.tensor_scalar(
                out=tsq[:], in0=tsq[:], scalar1=float(-1.0 / 9.0), scalar2=1.0,
                op0=OP.mult, op1=OP.add,
            )
            nc.vector.tensor_scalar_max(tsq[:], tsq[:], 0.0)
            # Horner in t
            acc = workp.tile([128, 128 * JT + 128 * 8], F32, tag="acc")
            nc.vector.tensor_scalar(
                out=acc[:], in0=tsq[:], scalar1=float(CH[DEG]),
                scalar2=float(CH[DEG - 1]), op0=OP.mult, op1=OP.add,
            )
            for d in range(DEG - 2, -1, -1):
                nc.vector.tensor_mul(acc[:], acc[:], tsq[:])
                nc.vector.tensor_scalar_add(acc[:], acc[:], float(CH[d]))
            # acc = W_all [p, (d, c, a)]: d=0 -> wx taps, d=1 -> wy taps

            # gather cell4 indices: (Q0 + b2)*517 + 3 + fx, b2 = 0..1
            cbt = constp.tile([128, 2], F32, tag="cbt")
            for a in range(2):
                nc.vector.memset(cbt[:, a:a + 1], float(a * PAD + 3))
            fy517 = workp.tile([128, 128], F32, tag="fy517")
            nc.vector.tensor_scalar_mul(fy517[:], fp[:], float(PAD))
            idxf = workp.tile([128, 128 * 2], F32, tag="idxf")
            idxf3 = idxf[:].rearrange("p (c b) -> p c b", b=2)
            nc.vector.tensor_tensor(
                out=idxf3,
                in0=fy517[:].unsqueeze(2).broadcast_to([128, 128, 2]),
                in1=cbt[:].unsqueeze(1).broadcast_to([128, 128, 2]),
                op=OP.add,
            )
            nc.vector.tensor_tensor(
                out=idxf3,
                in0=idxf3,
                in1=fl[:, 0:128].unsqueeze(2).broadcast_to([128, 128, 2]),
                op=OP.add,
            )
            idx32 = workp.tile([128, 128 * 2], I32, tag="idx32")
            nc.vector.tensor_copy(out=idx32[:], in_=idxf[:])

            # x image tiles (persist across all coils)
            xts = []
            for xt in range(2):
                xt_t = workp.tile([128, 2 * IM], F32, tag=f"xt{xt}")
                nc.sync.dma_start(
                    out=xt_t[:],
                    in_=x_in[:, xt * 128:(xt + 1) * 128, :]
                    .rearrange("ri x y -> x ri y"),
                )
                xts.append(xt_t)

            # 1 persistent bf16 staging; partition p holds the v-row quad
            # (4p .. 4p+3), cells4 laid out (q, r4, e)
            stg0 = stgp.tile([128, G * W2], BF16, tag="stg0")

            for c in range(NC):
                # ---- coil multiply (bf16 out for PE) ----
                mt = []
                for xt in range(2):
                    ct = coilp.tile([128, 2 * IM], F32, tag="ct")
                    nc.sync.dma_start(
                        out=ct[:],
                        in_=c_in[c, :, xt * 128:(xt + 1) * 128, :]
                        .rearrange("ri x y -> x ri y"),
                    )
                    xt_t = xts[xt]
                    m = mp.tile([128, 2 * IM], BF16, tag="m")
                    xr, xi = xt_t[:, 0:IM], xt_t[:, IM:2 * IM]
                    cr, ci = ct[:, 0:IM], ct[:, IM:2 * IM]
                    mr, mi = m[:, 0:IM], m[:, IM:2 * IM]
                    t1 = mp.tile([128, IM], F32, tag="cm1")
                    t2 = mp.tile([128, IM], F32, tag="cm2")
                    nc.vector.tensor_mul(t1[:], xr, cr)
                    nc.vector.tensor_mul(t2[:], xi, ci)
                    nc.vector.tensor_sub(mr, t1[:], t2[:])
                    nc.vector.tensor_mul(t1[:], xr, ci)
                    nc.vector.tensor_mul(t2[:], xi, cr)
                    nc.vector.tensor_add(mi, t1[:], t2[:])
                    mt.append(m)
                # ---- stage 1: BT[y, u] per (ri, Yt) ----
                bt = {}
                for yt in range(2):
                    pr = ps1.tile([128, G], F32, tag="psa")
                    pi = ps1.tile([128, G], F32, tag="psa")
                    for xt in range(2):
                        mrb = mt[xt][:, yt * 128:yt * 128 + 128]
                        mib = mt[xt][:, IM + yt * 128:IM + yt * 128 + 128]
                        st = xt == 0
                        sp = xt == 1
                        nc.tensor.matmul(pr[:], mrb, artT[xt][:], start=st, stop=False)
                        nc.tensor.matmul(pi[:], mrb, aitT[xt][:], start=st, stop=False)
                        nc.tensor.matmul(pr[:], mib, aitnT[xt][:], start=False, stop=sp)
                        nc.tensor.matmul(pi[:], mib, artT[xt][:], start=False, stop=sp)
                    btr = btp.tile([128, G], BF16, tag="bt")
                    bti = btp.tile([128, G], BF16, tag="bt")
                    nc.scalar.copy(out=btr[:], in_=pr[:])
                    nc.scalar.copy(out=bti[:], in_=pi[:])
                    bt[(0, yt)] = btr
                    bt[(1, yt)] = bti
                # ---- stage 2: G[v, u] with v = 4p + r4 via stride-4 A
                # column slices; drain into quad-row staging ----
                for r2 in range(4):
                    stg3 = stg0[:].rearrange("p (q w) -> p q w", w=W2)
                    gr = ps2.tile([128, G], F32, tag="psb")
                    gi = ps2.tile([128, G], F32, tag="psb")
                    for yt in range(2):
                        av = artP[yt][:, r2 * 128:(r2 + 1) * 128]
                        aiv = aitP[yt][:, r2 * 128:(r2 + 1) * 128]
                        ainv = aitnP[yt][:, r2 * 128:(r2 + 1) * 128]
                        btr = bt[(0, yt)]
                        bti = bt[(1, yt)]
                        st = yt == 0
                        sp = yt == 1
                        nc.tensor.matmul(gr[:], av, btr[:], start=st, stop=False)
                        nc.tensor.matmul(gi[:], aiv, btr[:], start=st, stop=False)
                        nc.tensor.matmul(gr[:], ainv, bti[:], start=False, stop=sp)
                        nc.tensor.matmul(gi[:], av, bti[:], start=False, stop=sp)
                    c2 = r2 * CELL + 2 * c
                    # split strided drains across Scalar and Vector engines
                    if c % 2 == 0:
                        nc.scalar.copy(out=stg3[:, :, c2:c2 + 1], in_=gr[:].unsqueeze(2))
                        nc.vector.tensor_copy(out=stg3[:, :, c2 + 1:c2 + 2], in_=gi[:].unsqueeze(2))
                    else:
                        nc.vector.tensor_copy(out=stg3[:, :, c2:c2 + 1], in_=gr[:].unsqueeze(2))
                        nc.scalar.copy(out=stg3[:, :, c2 + 1:c2 + 2], in_=gi[:].unsqueeze(2))

            # ---- table stores: main (Q 1..128) + q halos, then halo
            # quads Q0 (v 508..511 <- stg[127]) and Q129 (v 0..3 <- stg[0])
            t_stores = []
            Th = T_dram
            t_stores.append(nc.sync.dma_start(
                out=Th[1:129, 2 * W2:2 * W2 + G * W2], in_=stg0[:]
            ))
            t_stores.append(nc.sync.dma_start(
                out=Th[1:129, 514 * W2:517 * W2], in_=stg0[:, 0:3 * W2],
            ))
            t_stores.append(nc.sync.dma_start(
                out=Th[1:129, 0:2 * W2], in_=stg0[:, 510 * W2:512 * W2],
            ))
            for dst, psrc in ((0, 127), (129, 0)):
                t_stores += [
                    nc.sync.dma_start(
                        out=Th[dst:dst + 1, 2 * W2:2 * W2 + G * W2],
                        in_=stg0[psrc:psrc + 1, :],
                    ),
                    nc.sync.dma_start(
                        out=Th[dst:dst + 1, 514 * W2:517 * W2],
                        in_=stg0[psrc:psrc + 1, 0:3 * W2],
                    ),
                    nc.sync.dma_start(
                        out=Th[dst:dst + 1, 0:2 * W2],
                        in_=stg0[psrc:psrc + 1, 510 * W2:512 * W2],
                    ),
                ]

            # ======== gather + combine ========
            # per index: 320 contiguous el (5 cells4 = 5q x 4rows x 16cri,
            # 640B); 2 quad-taps/point, 1 idx/partition/call -> 256 calls
            tab_flat = T_dram[:].rearrange("r (q e) -> (r q) e", e=W2)
            all_gathers = []
            for t in range(NTILE):
                # W[g, r4, b2, a] = wy[g, 4*b2 + r4] * wx[g, a]
                w240 = w36p.tile([128, GRP * 40], F32, tag="w36")
                for r2 in range(4):
                    ow = bass.AP(
                        w240[:].tensor, w240[:].offset + r2 * 10,
                        [w240[:].ap[0], [40, GRP], [5, 2], [1, 5]],
                    )
                    wyv = bass.AP(
                        acc[:].tensor,
                        acc[:].offset + 128 * JT + t * GRP * 8 + r2,
                        [acc[:].ap[0], [8, GRP], [4, 2], [0, 5]],
                    )
                    wxv = bass.AP(
                        acc[:].tensor, acc[:].offset + t * GRP * JT,
                        [acc[:].ap[0], [JT, GRP], [0, 2], [1, 5]],
                    )
                    nc.vector.tensor_tensor(out=ow, in0=wyv, in1=wxv, op=OP.mult)
                patch = patchp.tile([128, GRP * 2 * JT * W2], BF16, tag="patch")
                for g in range(GRP):
                    for b in range(2):
                        col = (t * GRP + g) * 2 + b
                        gi_ = nc.gpsimd.indirect_dma_start(
                            out=patch[:, (g * 2 + b) * JT * W2:
                                      (g * 2 + b + 1) * JT * W2],
                            out_offset=None,
                            in_=tab_flat,
                            in_offset=bass.IndirectOffsetOnAxis(
                                ap=idx32[:, col:col + 1], axis=0
                            ),
                        )
                        all_gathers.append(gi_)
                # WP[p, (g, cr, (r4, b2, a))] = patch[p, (g, b2, a, r4, cr)] * W
                wp = wpp.tile([128, GRP * 40 * CELL], BF16, tag="wpt")
                for r2 in range(4):
                    pv = bass.AP(
                        patch[:].tensor, patch[:].offset + r2 * CELL,
                        [patch[:].ap[0],
                         [2 * JT * W2, GRP], [1, CELL], [W2, 10]],
                    )
                    wv = bass.AP(
                        w240[:].tensor, w240[:].offset + r2 * 10,
                        [w240[:].ap[0], [40, GRP], [0, CELL], [1, 10]],
                    )
                    ov = bass.AP(
                        wp[:].tensor, wp[:].offset + r2 * 10,
                        [wp[:].ap[0],
                         [40 * CELL, GRP], [40, CELL], [1, 10]],
                    )
                    nc.vector.tensor_tensor(out=ov, in0=pv, in1=wv, op=OP.mult)
                # reduce innermost 40 -> private per-tile result tile
                # (slicing a shared accumulator would serialize the gather
                # pipeline on whole-tile WAR hazards)
                rt = resp.tile([128, 128], F32, tag="rt")
                rv = bass.AP(
                    rt[:].tensor, rt[:].offset,
                    [rt[:].ap[0], [16, GRP], [1, CELL]],
                )
                wp3 = wp[:].rearrange("p (g cr ba) -> p g cr ba", cr=CELL, ba=40)
                nc.vector.tensor_reduce(out=rv, in_=wp3, axis=AX.X, op=OP.add)
                ts_ = slice(t * 128, (t + 1) * 128)
                nc.vector.tensor_mul(rt[:], rt[:], wsq[:, ts_])
                nc.sync.dma_start(out=y_out[:, ts_], in_=rt[:])

            # explicit RAW edges: gathers after table stores
            for gi_ in all_gathers:
                for si in t_stores:
                    tile.add_dep_helper(gi_.ins, si.ins, reason="T RAW")



            if debug:
                dbg_outs = {
                    "kgo": kg, "acco": acc, "idxo": idx32, "flo": fl, "rro": rr,
                }
                for nm, t_ in dbg_outs.items():
                    o = nc.dram_tensor(nm, list(t_[:].shape), t_[:].dtype,
                                       kind="ExternalOutput")
                    nc.sync.dma_start(out=o[:], in_=t_[:])
                o = nc.dram_tensor("t0o", [PAD, TW], BF16, kind="ExternalOutput")
                di = nc.sync.dma_start(out=o[:], in_=T_dram[:])
                for si in t_stores:
                    tile.add_dep_helper(di.ins, si.ins, reason="T dump RAW")

    nc.compile()
    return nc


_NC_CACHE = None


def _get_nc():
    global _NC_CACHE
    if _NC_CACHE is None:
        _NC_CACHE = build_bass()
    return _NC_CACHE


# ---------------------------------------------------------------- host glue
def _shuffle_w(w_t):
    # w[c, ri, K] -> [p, (t, g, c, ri)] with K = t*1024 + g*128 + p
    v = w_t.reshape(NC, 2, NTILE, GRP, 128)
    return np.ascontiguousarray(v.transpose(4, 2, 3, 0, 1).reshape(128, NTILE * 128))


def _unshuffle_y(yr):
    # [p, (t, g, c, ri)] -> y[c, ri, K]
    v = yr.reshape(128, NTILE, GRP, NC, 2)
    return np.ascontiguousarray(v.transpose(3, 4, 1, 2, 0).reshape(NC, 2, K))


def make_in_maps(x, k, coil_sensitivities, w):
    in_maps = []
    coil0 = np.ascontiguousarray(coil_sensitivities[0], dtype=np.float32)
    for t in range(NT):
        in_maps.append({
            "x": np.ascontiguousarray(x[t], dtype=np.float32),
            "kk": np.ascontiguousarray(k[t], dtype=np.float32),
            "coil": coil0,
            "wr": _shuffle_w(np.asarray(w[t], dtype=np.float32)),
            "art": _ART, "ait": _AIT, "aitn": _AITN,
        })
    return in_maps


def run(x, k, coil_sensitivities, w, trace=False, **spmd_kwargs):
    nc = _get_nc()
    in_maps = make_in_maps(x, k, coil_sensitivities, w)
    r = run_bass_kernel_spmd(nc, in_maps, list(range(NT)), trace=trace, **spmd_kwargs)
    y = np.stack([_unshuffle_y(r.results[t]["yr"]) for t in range(NT)], axis=0)
    return y.astype(np.float32), r


def kernel(x, k, coil_sensitivities, w):
    y, _ = run(x, k, coil_sensitivities, w, trace=False)
    return y


# revision 28
# speedup vs baseline: 2.2666x; 1.0055x over previous
"""Trainium2 Bass kernel for nn_RadialModel (forward NUFFT, radial MRI).

Per-core (1 frame, all 8 coils):
  1. coil multiply (DVE, bf16 out)       cimage = (xr+ixi)*(cr+ici)
  2. DFT via PE bf16 matmuls (two stages): G[v,u] = A @ (M^T @ A^T) with
     apodization + fftshift phases folded into the constant A matrices
  3. store grid to a DRAM table (bf16), coil-interleaved cells
     [p=v_pad(517), q=u_pad(517), cri(16)] with 2/3-cell wraparound halo
  4. Kaiser-Bessel interpolation with round-to-nearest centering so the
     fractional offset is in [-0.5, 0.5]: a 5x5 tap window then captures
     every tap with weight > 2.6e-3 (the 6th tap of the reference's 6x6
     always has |U| >= 2.5 there).  The table packs FOUR v-rows per cell
     (quad-row cells, 64 el), so one gather index fetches 5 q-cells x 4
     rows x 16 cri = 640B and a point needs only TWO quad-row taps (the
     8-row window always covers the 5 needed rows; stray rows get ~4e-6
     weights from the clamped KB polynomial).  One index per partition
     per call is a HW limit of the SWDGE indirect path (~1.25us fixed
     cost each) -> 256 calls, the dominant cost of the kernel.
  5. per-r4 weighted multiply + 40-tap reduce on DVE, sqrt(w) scale + store

Sharding: one frame (nt) per NeuronCore, 8 cores. Host does only
shard/reshape/unshuffle; all math on device.
"""
import math
import numpy as np

import concourse.bass as bass
import concourse.bacc as bacc
import concourse.mybir as mybir
import concourse.tile as tile
from concourse.bass_utils import run_bass_kernel_spmd
from concourse.masks import make_identity

F32 = mybir.dt.float32
I32 = mybir.dt.int32
AX = mybir.AxisListType
OP = mybir.AluOpType

IM = 256
G = 512
J = 6
JT = 5             # live taps per dim (rne centering => the dropped 6th
                   # tap always has |U| >= 2.5, KB weight <= 2.6e-3)
ALPHA = 2.34 * J
TWO_PI = 2.0 * np.pi
PAD = 517          # 512 + 2 left halo + 3 right halo
NT, NC, K = 8, 8, 16384
CELL = NC * 2      # floats per (p,q) cell = 16 (all coils interleaved)
TW = PAD * CELL    # table row width in elements = 8272
NTILE = 16         # point tiles of 1024 points (8 groups x 128 partitions)
GRP = 8            # groups per tile
DEG = 8            # KB weight polynomial degree (in t); abs err ~8.5e-6
NCELLS = PAD * PAD # flat cell count of the table


# ---------------------------------------------------------------- host consts
def _host_consts():
    # apodization correction 1/FT(kb)
    f = (np.arange(IM) - IM // 2) / G
    z = (np.pi * J * f) ** 2 - ALPHA ** 2
    s = np.sqrt(np.abs(z))
    val = np.where(z < 0, np.sinh(s) / np.maximum(s, 1e-12), np.sinc(s / np.pi))
    ftkb = (J / np.i0(ALPHA)) * val
    scal = 1.0 / ftkb
    # A[u, x'] = e^{i pi u/2 - 2 pi i u x'/G} * scal[x'] / sqrt(G)
    u = np.arange(G)[:, None].astype(np.float64)
    xp = np.arange(IM)[None, :].astype(np.float64)
    A = np.exp(1j * np.pi * u / 2 - 2j * np.pi * u * xp / G) * scal[None, :] / np.sqrt(G)
    art = np.ascontiguousarray(A.T.real, dtype=np.float32)   # [256, 512]
    ait = np.ascontiguousarray(A.T.imag, dtype=np.float32)
    aitn = np.ascontiguousarray(-A.T.imag, dtype=np.float32)
    # polynomial fit of w(t) = i0(ALPHA*sqrt(t))/i0(ALPHA) on t in [0,1]
    n = 512
    x = (1 - np.cos(np.pi * (np.arange(n) + 0.5) / n)) / 2
    w = np.i0(ALPHA * np.sqrt(x)) / np.i0(ALPHA)
    V = np.vander(x, DEG + 1, increasing=True)
    c, *_ = np.linalg.lstsq(V, w, rcond=None)
    return art, ait, aitn, c.astype(np.float64)


_ART, _AIT, _AITN, _CHEB = _host_consts()


# ---------------------------------------------------------------- bass build
def build_bass(debug=False):
    nc = bacc.Bacc()

    x_in = nc.declare_dram_parameter("x", [2, IM, IM], F32, isOutput=False)
    k_in = nc.declare_dram_parameter("kk", [2, K], F32, isOutput=False)
    c_in = nc.declare_dram_parameter("coil", [NC, 2, IM, IM], F32, isOutput=False)
    w_in = nc.declare_dram_parameter("wr", [128, NTILE * 128], F32, isOutput=False)
    art_in = nc.declare_dram_parameter("art", [IM, G], F32, isOutput=False)
    ait_in = nc.declare_dram_parameter("ait", [IM, G], F32, isOutput=False)
    aitn_in = nc.declare_dram_parameter("aitn", [IM, G], F32, isOutput=False)
    y_out = nc.declare_dram_parameter("yr", [128, NTILE * 128], F32, isOutput=True)

    BF16 = mybir.dt.bfloat16
    W2 = 4 * CELL          # quad-row cell: 4 v-rows x 16 cri = 64 el
    TW2 = PAD * W2         # table Q-row width = 33088 el
    PROWS = 130            # quads of table rows 0..519 (R = m + 4)
    T_dram = nc.dram_tensor("T0", [PROWS, TW2], BF16)

    CH = _CHEB
    with tile.TileContext(nc) as tc:
        with (
            tc.tile_pool(name="const", bufs=1) as constp,
            tc.tile_pool(name="work", bufs=1) as workp,
            tc.tile_pool(name="ctile", bufs=2) as coilp,
            tc.tile_pool(name="mtile", bufs=4) as mp,
            tc.tile_pool(name="bt", bufs=8) as btp,
            tc.tile_pool(name="stg", bufs=1) as stgp,
            tc.tile_pool(name="patch", bufs=2) as patchp,
            tc.tile_pool(name="w36", bufs=2) as w36p,
            tc.tile_pool(name="wp", bufs=2) as wpp,
            tc.tile_pool(name="rt", bufs=3) as resp,
            tc.tile_pool(name="ps1", bufs=4, space="PSUM") as ps1,
            tc.tile_pool(name="ps2", bufs=4, space="PSUM") as ps2,
        ):
            # ---------------- constants ----------------
            ident = constp.tile([128, 128], F32, tag="ident")
            make_identity(nc, ident[:])
            # A matrices: DMA f32, convert once to bf16 for PE
            art = []
            for name, src in (("art", art_in), ("ait", ait_in), ("aitn", aitn_in)):
                ts_ = []
                for xt in range(2):
                    tf = constp.tile([128, G], F32, tag=f"{name}f{xt}")
                    nc.sync.dma_start(out=tf[:], in_=src[xt * 128:(xt + 1) * 128, :])
                    tb = constp.tile([128, G], BF16, tag=f"{name}b{xt}")
                    nc.scalar.copy(out=tb[:], in_=tf[:])
                    # stage-2 copy with v-columns regrouped (r4, p) so the
                    # stride-4 quad slices become contiguous weight loads
                    tp_ = constp.tile([128, G], BF16, tag=f"{name}p{xt}")
                    pin = bass.AP(
                        tf[:].tensor, tf[:].offset,
                        [tf[:].ap[0], [1, 4], [4, 128]],
                    )
                    nc.scalar.copy(out=tp_[:], in_=pin)
                    ts_.append((tb, tp_))
                art.append(ts_)
            artT = [a[0] for a in art[0]]
            aitT = [a[0] for a in art[1]]
            aitnT = [a[0] for a in art[2]]
            artP = [a[1] for a in art[0]]
            aitP = [a[1] for a in art[1]]
            aitnP = [a[1] for a in art[2]]

            offs = constp.tile([128, JT], F32, tag="offs")
            for a in range(JT):
                nc.vector.memset(offs[:, a:a + 1], float(3 - (a + 1)))
            ylat = constp.tile([128, 8], F32, tag="ylat")
            for a in range(8):
                nc.vector.memset(ylat[:, a:a + 1], float(-a))

            # ---------------- k -> [p, c] transpose ----------------
            kg = workp.tile([128, 256], F32, tag="kg")  # [p, (d, c)]
            for d in range(2):
                kt_in = workp.tile([128, 128], F32, tag="ktin")
                nc.sync.dma_start(
                    out=kt_in[:], in_=k_in[d].rearrange("(c p) -> c p", p=128)
                )
                ktp = ps2.tile([128, 128], F32, tag="psb")
                nc.tensor.transpose(ktp[:], kt_in[:], ident[:])
                nc.scalar.copy(out=kg[:, d * 128:(d + 1) * 128], in_=ktp[:])

            # ---------------- w load + sqrt ----------------
            wsq = workp.tile([128, NTILE * 128], F32, tag="wsq")
            nc.sync.dma_start(out=wsq[:], in_=w_in[:])
            nc.scalar.activation(
                out=wsq[:], in_=wsq[:],
                func=mybir.ActivationFunctionType.Sqrt,
            )

            # ---------------- index & weight math (DVE) ----------------
            # gx = om*(G/2pi); gx += 512 if gx < 0  -> [0, 512)
            gx0 = workp.tile([128, 256], F32, tag="gx0")
            nc.vector.tensor_scalar_mul(gx0[:], kg[:], float(G / TWO_PI))
            msk = workp.tile([128, 256], F32, tag="msk")
            nc.vector.tensor_scalar(
                out=msk[:], in0=gx0[:], scalar1=0.0, scalar2=None, op0=OP.is_lt
            )
            gxy = workp.tile([128, 256], F32, tag="gxy")
            nc.vector.scalar_tensor_tensor(
                out=gxy[:], in0=msk[:], scalar=float(G), in1=gx0[:],
                op0=OP.mult, op1=OP.add,
            )
            # gm3 = gxy - 3 ; f = rne(gm3 - 0.498) via 2^23 trick ; r = gm3 - f
            gm3 = workp.tile([128, 256], F32, tag="gm3")
            nc.vector.tensor_scalar(
                out=gm3[:], in0=gxy[:], scalar1=3.0, scalar2=None, op0=OP.subtract
            )
            fl = workp.tile([128, 256], F32, tag="fl")
            nc.vector.tensor_scalar(
                out=fl[:], in0=gm3[:],
                scalar1=0.0, scalar2=12582912.0,
                op0=OP.add, op1=OP.add,
            )
            nc.vector.tensor_scalar(
                out=fl[:], in0=fl[:], scalar1=12582912.0, scalar2=None,
                op0=OP.subtract,
            )
            rr = workp.tile([128, 256], F32, tag="rr")
            nc.vector.tensor_sub(rr[:], gm3[:], fl[:])

            # Q0 = floor((fl_y + 5)/4): h = fl*0.25 + 1.25, rne(h - 0.498)
            fp = workp.tile([128, 128], F32, tag="fp")
            nc.vector.tensor_scalar(
                out=fp[:], in0=fl[:, 128:256], scalar1=0.25, scalar2=1.25,
                op0=OP.mult, op1=OP.add,
            )
            nc.vector.tensor_scalar(
                out=fp[:], in0=fp[:],
                scalar1=-0.498046875, scalar2=12582912.0,
                op0=OP.add, op1=OP.add,
            )
            nc.vector.tensor_scalar(
                out=fp[:], in0=fp[:], scalar1=12582912.0, scalar2=None,
                op0=OP.subtract,
            )
            # sY = gy + 4 - 4*Q0 = gm3_y + 7 - 4*fp; y-tap j weight
            # arg = sY - j for gathered rows 4*Q0 + j, j = 0..7 (R = m + 4)
            sY = workp.tile([128, 128], F32, tag="sY")
            nc.vector.scalar_tensor_tensor(
                out=sY[:], in0=fp[:], scalar=-4.0, in1=gm3[:, 128:256],
                op0=OP.mult, op1=OP.add,
            )
            nc.vector.tensor_scalar_add(sY[:], sY[:], 7.0)

            # tap weight args: x: rr_x + offs (5); y: sY - j (6)
            NXC = 128 * JT
            ut = workp.tile([128, NXC + 128 * 8], F32, tag="ut")
            utx3 = ut[:, 0:NXC].rearrange("p (c a) -> p c a", a=JT)
            nc.vector.tensor_tensor(
                out=utx3,
                in0=rr[:, 0:128].unsqueeze(2).broadcast_to([128, 128, JT]),
                in1=offs[:].unsqueeze(1).broadcast_to([128, 128, JT]),
                op=OP.add,
            )
            uty3 = ut[:, NXC:].rearrange("p (c j) -> p c j", j=8)
            nc.vector.tensor_tensor(
                out=uty3,
                in0=sY[:].unsqueeze(2).broadcast_to([128, 128, 8]),
                in1=ylat[:].unsqueeze(1).broadcast_to([128, 128, 8]),
                op=OP.add,
            )
            # t = max(0, 1 - (U/3)^2)
            tsq = workp.tile([128, 128 * JT + 128 * 8], F32, tag="tsq")
            nc.vector.tensor_mul(tsq[:], ut[:], ut[:])
            nc.vector.tensor_scalar(
                out=tsq[:], in0=tsq[:], scalar1=float(-1.0 / 9.0), scalar2=1.0,
                op0=OP.mult, op1=OP.add,
            )
            nc.vector.tensor_scalar_max(tsq[:], tsq[:], 0.0)
            # Horner in t
            acc = workp.tile([128, 128 * JT + 128 * 8], F32, tag="acc")
            nc.vector.tensor_scalar(
                out=acc[:], in0=tsq[:], scalar1=float(CH[DEG]),
                scalar2=float(CH[DEG - 1]), op0=OP.mult, op1=OP.add,
            )
            for d in range(DEG - 2, -1, -1):
                nc.vector.tensor_mul(acc[:], acc[:], tsq[:])
                nc.vector.tensor_scalar_add(acc[:], acc[:], float(CH[d]))
            # acc = W_all [p, (d, c, a)]: d=0 -> wx taps, d=1 -> wy taps

            # gather cell4 indices: (Q0 + b2)*517 + 3 + fx, b2 = 0..1
            cbt = constp.tile([128, 2], F32, tag="cbt")
            for a in range(2):
                nc.vector.memset(cbt[:, a:a + 1], float(a * PAD + 3))
            fy517 = workp.tile([128, 128], F32, tag="fy517")
            nc.vector.tensor_scalar_mul(fy517[:], fp[:], float(PAD))
            idxf = workp.tile([128, 128 * 2], F32, tag="idxf")
            idxf3 = idxf[:].rearrange("p (c b) -> p c b", b=2)
            nc.vector.tensor_tensor(
                out=idxf3,
                in0=fy517[:].unsqueeze(2).broadcast_to([128, 128, 2]),
                in1=cbt[:].unsqueeze(1).broadcast_to([128, 128, 2]),
                op=OP.add,
            )
            nc.vector.tensor_tensor(
                out=idxf3,
                in0=idxf3,
                in1=fl[:, 0:128].unsqueeze(2).broadcast_to([128, 128, 2]),
                op=OP.add,
            )
            idx32 = workp.tile([128, 128 * 2], I32, tag="idx32")
            nc.vector.tensor_copy(out=idx32[:], in_=idxf[:])

            # x image tiles (persist across all coils)
            xts = []
            for xt in range(2):
                xt_t = workp.tile([128, 2 * IM], F32, tag=f"xt{xt}")
                nc.sync.dma_start(
                    out=xt_t[:],
                    in_=x_in[:, xt * 128:(xt + 1) * 128, :]
                    .rearrange("ri x y -> x ri y"),
                )
                xts.append(xt_t)

            # 1 persistent bf16 staging; partition p holds the v-row quad
            # (4p .. 4p+3), cells4 laid out (q, r4, e)
            stg0 = stgp.tile([128, G * W2], BF16, tag="stg0")

            for c in range(NC):
                # ---- coil multiply (bf16 out for PE) ----
                mt = []
                for xt in range(2):
                    ct = coilp.tile([128, 2 * IM], F32, tag="ct")
                    nc.sync.dma_start(
                        out=ct[:],
                        in_=c_in[c, :, xt * 128:(xt + 1) * 128, :]
                        .rearrange("ri x y -> x ri y"),
                    )
                    xt_t = xts[xt]
                    m = mp.tile([128, 2 * IM], BF16, tag="m")
                    xr, xi = xt_t[:, 0:IM], xt_t[:, IM:2 * IM]
                    cr, ci = ct[:, 0:IM], ct[:, IM:2 * IM]
                    mr, mi = m[:, 0:IM], m[:, IM:2 * IM]
                    t1 = mp.tile([128, IM], F32, tag="cm1")
                    t2 = mp.tile([128, IM], F32, tag="cm2")
                    nc.vector.tensor_mul(t1[:], xr, cr)
                    nc.vector.tensor_mul(t2[:], xi, ci)
                    nc.vector.tensor_sub(mr, t1[:], t2[:])
                    nc.vector.tensor_mul(t1[:], xr, ci)
                    nc.vector.tensor_mul(t2[:], xi, cr)
                    nc.vector.tensor_add(mi, t1[:], t2[:])
                    mt.append(m)
                # ---- stage 1: BT[y, u] per (ri, Yt) ----
                bt = {}
                for yt in range(2):
                    pr = ps1.tile([128, G], F32, tag="psa")
                    pi = ps1.tile([128, G], F32, tag="psa")
                    for xt in range(2):
                        mrb = mt[xt][:, yt * 128:yt * 128 + 128]
                        mib = mt[xt][:, IM + yt * 128:IM + yt * 128 + 128]
                        st = xt == 0
                        sp = xt == 1
                        nc.tensor.matmul(pr[:], mrb, artT[xt][:], start=st, stop=False)
                        nc.tensor.matmul(pi[:], mrb, aitT[xt][:], start=st, stop=False)
                        nc.tensor.matmul(pr[:], mib, aitnT[xt][:], start=False, stop=sp)
                        nc.tensor.matmul(pi[:], mib, artT[xt][:], start=False, stop=sp)
                    btr = btp.tile([128, G], BF16, tag="bt")
                    bti = btp.tile([128, G], BF16, tag="bt")
                    nc.scalar.copy(out=btr[:], in_=pr[:])
                    nc.scalar.copy(out=bti[:], in_=pi[:])
                    bt[(0, yt)] = btr
                    bt[(1, yt)] = bti
                # ---- stage 2: G[v, u] with v = 4p + r4 via stride-4 A
                # column slices; drain into quad-row staging ----
                for r2 in range(4):
                    stg3 = stg0[:].rearrange("p (q w) -> p q w", w=W2)
                    gr = ps2.tile([128, G], F32, tag="psb")
                    gi = ps2.tile([128, G], F32, tag="psb")
                    for yt in range(2):
                        av = artP[yt][:, r2 * 128:(r2 + 1) * 128]
                        aiv = aitP[yt][:, r2 * 128:(r2 + 1) * 128]
                        ainv = aitnP[yt][:, r2 * 128:(r2 + 1) * 128]
                        btr = bt[(0, yt)]
                        bti = bt[(1, yt)]
                        st = yt == 0
                        sp = yt == 1
                        nc.tensor.matmul(gr[:], av, btr[:], start=st, stop=False)
                        nc.tensor.matmul(gi[:], aiv, btr[:], start=st, stop=False)
                        nc.tensor.matmul(gr[:], ainv, bti[:], start=False, stop=sp)
                        nc.tensor.matmul(gi[:], av, bti[:], start=False, stop=sp)
                    c2 = r2 * CELL + 2 * c
                    # split strided drains across Scalar and Vector engines
                    if c % 2 == 0:
                        nc.scalar.copy(out=stg3[:, :, c2:c2 + 1], in_=gr[:].unsqueeze(2))
                        nc.vector.tensor_copy(out=stg3[:, :, c2 + 1:c2 + 2], in_=gi[:].unsqueeze(2))
                    else:
                        nc.vector.tensor_copy(out=stg3[:, :, c2:c2 + 1], in_=gr[:].unsqueeze(2))
                        nc.scalar.copy(out=stg3[:, :, c2 + 1:c2 + 2], in_=gi[:].unsqueeze(2))

            # ---- table stores: main (Q 1..128) + q halos, then halo
            # quads Q0 (v 508..511 <- stg[127]) and Q129 (v 0..3 <- stg[0])
            t_stores = []
            Th = T_dram
            t_stores.append(nc.sync.dma_start(
                out=Th[1:129, 2 * W2:2 * W2 + G * W2], in_=stg0[:]
            ))
            t_stores.append(nc.sync.dma_start(
                out=Th[1:129, 514 * W2:517 * W2], in_=stg0[:, 0:3 * W2],
            ))
            t_stores.append(nc.sync.dma_start(
                out=Th[1:129, 0:2 * W2], in_=stg0[:, 510 * W2:512 * W2],
            ))
            for dst, psrc in ((0, 127), (129, 0)):
                t_stores += [
                    nc.sync.dma_start(
                        out=Th[dst:dst + 1, 2 * W2:2 * W2 + G * W2],
                        in_=stg0[psrc:psrc + 1, :],
                    ),
                    nc.sync.dma_start(
                        out=Th[dst:dst + 1, 514 * W2:517 * W2],
                        in_=stg0[psrc:psrc + 1, 0:3 * W2],
                    ),
                    nc.sync.dma_start(
                        out=Th[dst:dst + 1, 0:2 * W2],
                        in_=stg0[psrc:psrc + 1, 510 * W2:512 * W2],
                    ),
                ]

            # ======== gather + combine ========
            # per index: 320 contiguous el (5 cells4 = 5q x 4rows x 16cri,
            # 640B); 2 quad-taps/point, 1 idx/partition/call -> 256 calls
            tab_flat = T_dram[:].rearrange("r (q e) -> (r q) e", e=W2)
            all_gathers = []
            for t in range(NTILE):
                # W[g, r4, b2, a] = wy[g, 4*b2 + r4] * wx[g, a]
                w240 = w36p.tile([128, GRP * 40], F32, tag="w36")
                for r2 in range(4):
                    ow = bass.AP(
                        w240[:].tensor, w240[:].offset + r2 * 10,
                        [w240[:].ap[0], [40, GRP], [5, 2], [1, 5]],
                    )
                    wyv = bass.AP(
                        acc[:].tensor,
                        acc[:].offset + 128 * JT + t * GRP * 8 + r2,
                        [acc[:].ap[0], [8, GRP], [4, 2], [0, 5]],
                    )
                    wxv = bass.AP(
                        acc[:].tensor, acc[:].offset + t * GRP * JT,
                        [acc[:].ap[0], [JT, GRP], [0, 2], [1, 5]],
                    )
                    nc.vector.tensor_tensor(out=ow, in0=wyv, in1=wxv, op=OP.mult)
                patch = patchp.tile([128, GRP * 2 * JT * W2], BF16, tag="patch")
                for g in range(GRP):
                    for b in range(2):
                        col = (t * GRP + g) * 2 + b
                        gi_ = nc.gpsimd.indirect_dma_start(
                            out=patch[:, (g * 2 + b) * JT * W2:
                                      (g * 2 + b + 1) * JT * W2],
                            out_offset=None,
                            in_=tab_flat,
                            in_offset=bass.IndirectOffsetOnAxis(
                                ap=idx32[:, col:col + 1], axis=0
                            ),
                        )
                        all_gathers.append(gi_)
                # WP[p, (g, cr, (r4, b2, a))] = patch[p, (g, b2, a, r4, cr)] * W
                wp = wpp.tile([128, GRP * 40 * CELL], BF16, tag="wpt")
                for r2 in range(4):
                    pv = bass.AP(
                        patch[:].tensor, patch[:].offset + r2 * CELL,
                        [patch[:].ap[0],
                         [2 * JT * W2, GRP], [1, CELL], [W2, 10]],
                    )
                    wv = bass.AP(
                        w240[:].tensor, w240[:].offset + r2 * 10,
                        [w240[:].ap[0], [40, GRP], [0, CELL], [1, 10]],
                    )
                    ov = bass.AP(
                        wp[:].tensor, wp[:].offset + r2 * 10,
                        [wp[:].ap[0],
                         [40 * CELL, GRP], [40, CELL], [1, 10]],
                    )
                    nc.vector.tensor_tensor(out=ov, in0=pv, in1=wv, op=OP.mult)
                # reduce innermost 40 -> private per-tile result tile
                # (slicing a shared accumulator would serialize the gather
                # pipeline on whole-tile WAR hazards)
                rt = resp.tile([128, 128], F32, tag="rt")
                rv = bass.AP(
                    rt[:].tensor, rt[:].offset,
                    [rt[:].ap[0], [16, GRP], [1, CELL]],
                )
                wp3 = wp[:].rearrange("p (g cr ba) -> p g cr ba", cr=CELL, ba=40)
                nc.vector.tensor_reduce(out=rv, in_=wp3, axis=AX.X, op=OP.add)
                ts_ = slice(t * 128, (t + 1) * 128)
                nc.vector.tensor_mul(rt[:], rt[:], wsq[:, ts_])
                nc.sync.dma_start(out=y_out[:, ts_], in_=rt[:])

            # explicit RAW edges: gathers after table stores
            for gi_ in all_gathers:
                for si in t_stores:
                    tile.add_dep_helper(gi_.ins, si.ins, reason="T RAW")



            if debug:
                dbg_outs = {
                    "kgo": kg, "acco": acc, "idxo": idx32, "flo": fl, "rro": rr,
                }
                for nm, t_ in dbg_outs.items():
                    o = nc.dram_tensor(nm, list(t_[:].shape), t_[:].dtype,
                                       kind="ExternalOutput")
                    nc.sync.dma_start(out=o[:], in_=t_[:])
                o = nc.dram_tensor("t0o", [PAD, TW], BF16, kind="ExternalOutput")
                di = nc.sync.dma_start(out=o[:], in_=T_dram[:])
                for si in t_stores:
                    tile.add_dep_helper(di.ins, si.ins, reason="T dump RAW")

    nc.compile()
    return nc


_NC_CACHE = None


def _get_nc():
    global _NC_CACHE
    if _NC_CACHE is None:
        _NC_CACHE = build_bass()
    return _NC_CACHE


# ---------------------------------------------------------------- host glue
def _shuffle_w(w_t):
    # w[c, ri, K] -> [p, (t, g, c, ri)] with K = t*1024 + g*128 + p
    v = w_t.reshape(NC, 2, NTILE, GRP, 128)
    return np.ascontiguousarray(v.transpose(4, 2, 3, 0, 1).reshape(128, NTILE * 128))


def _unshuffle_y(yr):
    # [p, (t, g, c, ri)] -> y[c, ri, K]
    v = yr.reshape(128, NTILE, GRP, NC, 2)
    return np.ascontiguousarray(v.transpose(3, 4, 1, 2, 0).reshape(NC, 2, K))


def make_in_maps(x, k, coil_sensitivities, w):
    in_maps = []
    coil0 = np.ascontiguousarray(coil_sensitivities[0], dtype=np.float32)
    for t in range(NT):
        in_maps.append({
            "x": np.ascontiguousarray(x[t], dtype=np.float32),
            "kk": np.ascontiguousarray(k[t], dtype=np.float32),
            "coil": coil0,
            "wr": _shuffle_w(np.asarray(w[t], dtype=np.float32)),
            "art": _ART, "ait": _AIT, "aitn": _AITN,
        })
    return in_maps


def run(x, k, coil_sensitivities, w, trace=False, **spmd_kwargs):
    nc = _get_nc()
    in_maps = make_in_maps(x, k, coil_sensitivities, w)
    r = run_bass_kernel_spmd(nc, in_maps, list(range(NT)), trace=trace, **spmd_kwargs)
    y = np.stack([_unshuffle_y(r.results[t]["yr"]) for t in range(NT)], axis=0)
    return y.astype(np.float32), r


def kernel(x, k, coil_sensitivities, w):
    y, _ = run(x, k, coil_sensitivities, w, trace=False)
    return y
